# revision 6
# baseline (speedup 1.0000x reference)
"""Trainium2 Bass kernel for nn_CGT_21354577396059 (GPS-style GNN, 3 streams x 3 layers).

Strategy (8 NeuronCores, SPMD):
- Node-shard: core c owns nodes [2048c, 2048c+2048) = 8 graphs of 256 nodes.
- Activations feature-major in SBUF: hT [C=128 partitions, 2048 nodes] fp32,
  bf16 copies as matmul inputs.
- GIN segment_sum: edges dst-sorted per core, deduped by src per 128-dst
  window (multi-hot count columns); src rows gathered from a bf16 node-major
  DRAM table (gpsimd dma_gather, 256B rows, one call per window-group);
  scatter via count-matrix matmuls aggT += gathered_chunk.T @ onehot_chunk.
- The bf16 node table is stored P-MAJOR per core block (row p*16+k holds node
  k*128+p) so the SBUF->DRAM table write is contiguous; gather indices are
  host-permuted to match. Table rebuilt each layer via PE transpose + 8-core
  AllGather.
- One-hot count matrices are stored partition-major in DRAM so their loads are
  contiguous (4KB+ per-partition descriptors instead of 256B).
- Attention is graph-local: scoresT = kT.T @ qT per (graph, head, key-chunk)
  masked-K; exp on ACT; softmax sums via col-tiled ones-matmuls; o via
  col-tiled matmuls contracted over keys; normalization via ACT reciprocal.

kernel(**inputs) takes the FULL unsharded inputs and returns
(pool(h0), pool(ha), pool(hb)) -- tuple of [64, 128] float32 -- like the reference.
"""
import sys
import numpy as np
import ml_dtypes

if "/opt/trn_rl_repo" not in sys.path:
    sys.path.insert(0, "/opt/trn_rl_repo")

import concourse.bass as bass  # noqa: F401
import concourse.tile as tile
from concourse import bacc, mybir
from concourse.bass_utils import run_bass_kernel_spmd

BF = ml_dtypes.bfloat16

# Problem constants (self-contained; no reads of /root/problem/*)
N_NODES = 16384
N_GRAPHS = 64
NPG = 256
FEA_DIM = 32
PE_DIM = 20
C = 128
HEADS = 4
HD = C // HEADS
L = 3
BN_EPS = 1e-5
S_BN = float(1.0 / np.sqrt(1.0 + BN_EPS))

N_CORES = 8
NPC = N_NODES // N_CORES   # 2048
NG_C = NPC // NPG          # 8 graphs per core
WIN = 128
NWIN = NPC // WIN          # 16
NCHUNK = NPC // 128        # 16
WGRP = 1                   # windows per dma_gather call

fdt = mybir.dt.float32
bdt = mybir.dt.bfloat16
i16 = mybir.dt.int16
AF = mybir.ActivationFunctionType
AX = mybir.AxisListType
ALU = mybir.AluOpType
AG_GROUPS = [list(range(N_CORES))]


# ---------------------------------------------------------------------------
# Host-side data prep
# ---------------------------------------------------------------------------

def _perm_row(n):
    """Global table row for node n with p-major per-core blocks."""
    cb, m = n // NPC, n % NPC
    return cb * NPC + (m % 128) * NCHUNK + m // 128


def _wrap_idxs(idx):
    """dma_gather idx layout [128, n/16] int16: idx i at (i%16, i//16),
    replicated across the 8 16-partition blocks."""
    n = len(idx)
    a = np.asarray(idx, np.int16).reshape(n // 16, 16).T
    return np.ascontiguousarray(np.tile(a, (8, 1)))


def _prep_edges_stream(edge_index):
    """Dedup by src per (core, 128-dst window); count-matrix columns.

    Returns (cap_chunks, [(gidx_wrapped, oh_pmajor)] per core) where
    oh_pmajor is [128, NWIN*cap*WIN] bf16 (partition-major onehot)."""
    src = np.asarray(edge_index[0]).astype(np.int64)
    dst = np.asarray(edge_index[1]).astype(np.int64)
    per_core_wins = []
    max_w = 0
    for c in range(N_CORES):
        m = (dst >= c * NPC) & (dst < (c + 1) * NPC)
        s, d = src[m], dst[m] - c * NPC
        wins = []
        for w in range(NWIN):
            mw = (d >= w * WIN) & (d < (w + 1) * WIN)
            sw, dw = s[mw], d[mw] - w * WIN
            # dedup srcs within the window; multi-hot count columns
            uniq, inv = np.unique(sw, return_inverse=True)
            cnt = np.zeros((len(uniq), WIN), np.float32)
            np.add.at(cnt, (inv, dw), 1.0)
            wins.append((uniq, cnt))
            max_w = max(max_w, len(uniq))
        per_core_wins.append(wins)
    cap_e = ((max_w + 127) // 128) * 128     # rows per window, padded
    cap = cap_e // 128
    out = []
    for c in range(N_CORES):
        srcs = np.zeros(NWIN * cap_e, np.int64)
        oh = np.zeros((NWIN, cap_e, WIN), np.float32)
        for w in range(NWIN):
            uniq, cnt = per_core_wins[c][w]
            n = len(uniq)
            srcs[w * cap_e:w * cap_e + n] = _perm_row(uniq)
            oh[w, :n, :] = cnt
        # partition-major: oh_pm[p, (w, t, c)] = oh[w, t*128+p, c]
        oh_pm = np.ascontiguousarray(
            oh.reshape(NWIN, cap, 128, WIN).transpose(2, 0, 1, 3)
            .reshape(128, NWIN * cap * WIN).astype(BF))
        out.append((_wrap_idxs(srcs), oh_pm))
    return cap, out


def _pack_host(inputs):
    inp = {k: np.asarray(v) for k, v in inputs.items()}
    rt2 = 1.0 / np.sqrt(HD)

    blocks, offs = [], {}

    def add(name, arr):
        arr = np.asarray(arr, np.float32)
        k, m = arr.shape
        buf = np.zeros((128, m), BF)
        buf[:k] = arr.astype(BF)
        offs[name] = sum(b.shape[1] for b in blocks)
        blocks.append(buf)

    add("emb", inp["node_emb_w"])
    add("pe", inp["pe_lin_w"])
    add("I2", 2.0 * np.eye(C))       # h2 fold: ACT scale s gives 2s*h
    add("I1", np.eye(C))             # transpose identity + acc fold
    add("ones32", np.ones((C, HD)))
    for l in range(L):
        aw = inp["attn_in_w"][l]
        add(f"gw1_{l}", inp["gin_w1"][l])
        add(f"gw2_{l}", inp["gin_w2"][l])
        add(f"qT_{l}", (aw[0:C] * rt2).T)
        for h in range(HEADS):
            km = aw[C:2 * C].T.copy()
            mask = np.zeros(C); mask[32 * h:32 * h + 32] = 1.0
            add(f"kTm{h}_{l}", km * mask[None, :])
        add(f"vT_{l}", aw[2 * C:3 * C].T)
        add(f"ow_{l}", inp["attn_out_w"][l].T)
        add(f"m1_{l}", inp["mlp_w1"][l])
        add(f"m2a_{l}", inp["mlp_w2"][l][0:C])
        add(f"m2b_{l}", inp["mlp_w2"][l][C:2 * C])
    wts = np.ascontiguousarray(np.concatenate(blocks, axis=1))

    bvecs, boffs = [], {}

    def addb(name, vec):
        vec = np.asarray(vec, np.float32).reshape(-1)
        assert vec.shape == (C,)
        boffs[name] = len(bvecs)
        bvecs.append(vec)

    addb("eb", inp["node_emb_b"] + inp["pe_lin_b"])
    for l in range(L):
        ab = inp["attn_in_b"][l]
        addb(f"gb1_{l}", inp["gin_b1"][l])
        addb(f"sgb2_{l}", S_BN * inp["gin_b2"][l])
        addb(f"qb_{l}", ab[0:C] * rt2)
        for h in range(HEADS):
            mask = np.zeros(C); mask[32 * h:32 * h + 32] = 1.0
            addb(f"kbm{h}_{l}", ab[C:2 * C] * mask)
        addb(f"sob_{l}", S_BN * inp["attn_out_b"][l])
        addb(f"m1ba_{l}", inp["mlp_b1"][l][0:C])
        addb(f"m1bb_{l}", inp["mlp_b1"][l][C:2 * C])
        addb(f"smb2_{l}", S_BN * inp["mlp_b2"][l])
    biases = np.ascontiguousarray(np.stack(bvecs, axis=1).astype(np.float32))

    vbias = np.ascontiguousarray(np.stack(
        [np.tile(inp["attn_in_b"][l][2 * C:3 * C], (128, 1)) for l in range(L)]
    ).astype(np.float32))

    caps, edges = [], []
    for key in ("edge_index", "edge_index1", "edge_index2"):
        cap, per_core = _prep_edges_stream(inp[key])
        caps.append(cap)
        edges.append(per_core)

    xs = [inp["x"], inp["x1"], inp["x2"]]
    in_maps = []
    for c in range(N_CORES):
        m = {"wts": wts, "biases": biases, "vbias": vbias}
        sl = slice(c * NPC, (c + 1) * NPC)
        for s in range(3):
            m[f"xT{s}"] = np.ascontiguousarray(xs[s][sl].T.astype(BF))
            m[f"gidx{s}"] = edges[s][c][0]
            m[f"onehot{s}"] = edges[s][c][1]
        m["peT"] = np.ascontiguousarray(inp["pe"][sl].T.astype(BF))
        in_maps.append(m)

    return in_maps, caps, offs, boffs, wts.shape[1], biases.shape[1]


# ---------------------------------------------------------------------------
# Kernel builder
# ---------------------------------------------------------------------------


def _mm(nc, out, lhsT, rhs, start, stop, nmax=512):
    """matmul with moving free dim split to <=512 (ISA limit)."""
    n = rhs.shape[-1]
    assert out.shape[-1] == n
    for i in range(0, n, nmax):
        j = min(i + nmax, n)
        nc.tensor.matmul(out[:, i:j], lhsT, rhs[:, i:j], start=start, stop=stop)


def build_kernel(caps, offs, boffs, wcols, bcols, n_layers=L, n_streams=3,
                 gcall=1024, dbg_stream=None):
    nc = bacc.Bacc("TRN2", target_bir_lowering=False, num_devices=N_CORES,
                   num_swdge_queues=4)

    wts_d = nc.dram_tensor("wts", [128, wcols], bdt, kind="ExternalInput")
    bias_d = nc.dram_tensor("biases", [128, bcols], fdt, kind="ExternalInput")
    vbias_d = nc.dram_tensor("vbias", [L, 128, 128], fdt, kind="ExternalInput")
    peT_d = nc.dram_tensor("peT", [PE_DIM, NPC], bdt, kind="ExternalInput")
    xT_d, gidx_d, oh_d = [], [], []
    for s in range(3):
        cap = caps[s]
        xT_d.append(nc.dram_tensor(f"xT{s}", [FEA_DIM, NPC], bdt,
                                   kind="ExternalInput"))
        gidx_d.append(nc.dram_tensor(f"gidx{s}", [128, NWIN * cap * 8], i16,
                                     kind="ExternalInput"))
        oh_d.append(nc.dram_tensor(f"onehot{s}", [128, NWIN * cap * WIN], bdt,
                                   kind="ExternalInput"))
    pool_out = nc.dram_tensor("pool_out", [3, C, NG_C], fdt,
                              kind="ExternalOutput")
    dbg_out = None
    if dbg_stream is not None:
        dbg_out = nc.dram_tensor("dbg_out", [C, NPC], fdt, kind="ExternalOutput")

    with tile.TileContext(nc) as tc:
        with (
            tc.tile_pool(name="const", bufs=1) as const_p,
            tc.tile_pool(name="hstate", bufs=1) as hstate_p,
            tc.tile_pool(name="big", bufs=1) as big_p,       # full-width tiles
            tc.tile_pool(name="chunk", bufs=2) as chunk_p,   # [C,1024]-ish tiles
            tc.tile_pool(name="gath", bufs=2) as gath_p,
            tc.tile_pool(name="ohp", bufs=2) as oh_p,
            tc.tile_pool(name="ps2", bufs=3, space="PSUM") as ps2,   # 2-bank
            tc.tile_pool(name="ps1", bufs=2, space="PSUM") as ps1,   # 1-bank
            tc.tile_pool(name="dram", bufs=4, space="DRAM") as dram_p,
        ):
            wts = const_p.tile([128, wcols], bdt)
            nc.sync.dma_start(wts[:], wts_d[:])
            bia = const_p.tile([128, bcols], fdt)
            nc.sync.dma_start(bia[:], bias_d[:])
            vbia = const_p.tile([128, L, 128], fdt)
            nc.sync.dma_start(vbia[:], vbias_d[:].rearrange("l p c -> p l c"))

            def W(name, width=128, rows=128):
                return wts[0:rows, offs[name]:offs[name] + width]

            def B(name):
                return bia[:, boffs[name]:boffs[name] + 1]

            hT = [hstate_p.tile([C, NPC], fdt, tag=f"hT{s}", name=f"hT{s}")
                  for s in range(3)]
            hB = [hstate_p.tile([C, NPC], bdt, tag=f"hB{s}", name=f"hB{s}")
                  for s in range(3)]
            gidx = []
            for s in range(n_streams):
                t = hstate_p.tile([128, NWIN * caps[s] * 8], i16,
                                  tag=f"gidx{s}", name=f"gidx{s}")
                nc.sync.dma_start(t[:], gidx_d[s][:])
                gidx.append(t)

            def emit_table(src_bf):
                """feature-major bf16 [C, NPC] -> p-major node table ->
                AllGather. Block layout: DRAM row p*NCHUNK+k = node k*128+p."""
                nm = big_p.tile([128, NCHUNK, C], bdt, tag="nm")
                for k in range(NCHUNK):
                    pt = ps1.tile([128, 128], bdt, tag="ps1", name="pt")
                    nc.tensor.transpose(pt[:], src_bf[:, k * 128:(k + 1) * 128],
                                        W("I1"))
                    nc.vector.tensor_copy(nm[:, k, :], pt[:])
                agi = dram_p.tile([128, NCHUNK * C], bdt, tag="agi")
                nc.sync.dma_start(agi[:], nm[:].rearrange("p k c -> p (k c)"))
                tab = dram_p.tile([N_NODES, C], bdt, tag="tab", name="tab")
                nc.gpsimd.collective_compute(
                    "AllGather", ALU.bypass, replica_groups=AG_GROUPS,
                    ins=[agi.opt()], outs=[tab.opt()])
                return tab

            # ---------------- embedding ----------------
            tabs = [None] * 3
            for s in range(n_streams):
                xT = chunk_p.tile([FEA_DIM, NPC], bdt, tag="xT")
                nc.sync.dma_start(xT[:], xT_d[s][:])
                peT = chunk_p.tile([PE_DIM, NPC], bdt, tag="peT")
                nc.sync.dma_start(peT[:], peT_d[:])
                for ch in range(2):
                    sl = slice(ch * 1024, (ch + 1) * 1024)
                    ep = ps2.tile([C, 1024], fdt, tag="ps2")
                    _mm(nc, ep, W("emb", rows=FEA_DIM), xT[:, sl],
                        start=True, stop=False)
                    _mm(nc, ep, W("pe", rows=PE_DIM), peT[:, sl],
                        start=False, stop=True)
                    nc.scalar.activation(hT[s][:, sl], ep[:], AF.Identity,
                                         bias=B("eb"))
                    nc.vector.tensor_copy(hB[s][:, sl], hT[s][:, sl])
                tabs[s] = emit_table(hB[s])

            # ---------------- layers ----------------
            for l in range(n_layers):
                for s in range(n_streams):
                    _gps_layer(nc, l, caps[s], hT[s], hB[s], tabs, s,
                               gidx[s], oh_d[s], W, B, vbia,
                               big_p, chunk_p, gath_p, oh_p, ps2, ps1,
                               emit_table, gcall, last=(l == n_layers - 1))

            # ---------------- pooling ----------------
            for s in range(n_streams):
                po = chunk_p.tile([C, NG_C], fdt, tag="pool")
                nc.vector.reduce_sum(
                    out=po[:],
                    in_=hT[s][:].rearrange("c (g n) -> c g n", g=NG_C),
                    axis=AX.X)
                nc.sync.dma_start(pool_out[s], po[:])

            if dbg_stream is not None:
                nc.sync.dma_start(dbg_out[:], hT[dbg_stream][:])

    nc.compile()
    return nc


def _gps_layer(nc, l, cap, hT, hB, tabs, s, gidx, oh_d, W, B, vbia,
               big_p, chunk_p, gath_p, oh_p, ps2, ps1, emit_table, gcall,
               last):
    AFI = AF.Identity

    # ---------------- attention (graph-local) ----------------
    qT = big_p.tile([C, NPC], bdt, tag="qT", name="qT")
    for ch in range(2):
        sl = slice(ch * 1024, (ch + 1) * 1024)
        qp = ps2.tile([C, 1024], fdt, tag="ps2")
        _mm(nc, qp, W(f"qT_{l}"), hB[:, sl], start=True, stop=True)
        nc.scalar.activation(qT[:, sl], qp[:], AFI, bias=B(f"qb_{l}"))
    v_sb = big_p.tile([128, NCHUNK, C], bdt, tag="v_sb", name="v_sb")
    for k in range(NCHUNK):
        vp = ps1.tile([128, C], fdt, tag="ps1")
        nc.tensor.matmul(vp[:], hB[:, k * 128:(k + 1) * 128], W(f"vT_{l}"),
                         start=True, stop=True)
        nc.vector.tensor_add(v_sb[:, k, :], vp[:], vbia[:, l, :])

    expT = big_p.tile([128, 2, HEADS, NG_C, NPG], bdt, tag="expT", name="expT")
    for h in range(HEADS):
        kTm = big_p.tile([C, NPC], bdt, tag="kTm", name="kTm")
        for ch in range(2):
            sl = slice(ch * 1024, (ch + 1) * 1024)
            kp = ps2.tile([C, 1024], fdt, tag="ps2")
            _mm(nc, kp, W(f"kTm{h}_{l}"), hB[:, sl], start=True, stop=True)
            nc.scalar.activation(kTm[:, sl], kp[:], AFI, bias=B(f"kbm{h}_{l}"))
        for kc in range(2):
            for gh in range(2):
                sp = ps2.tile([128, 4, NPG], fdt, tag="ps2")
                for gi in range(4):
                    g = gh * 4 + gi
                    ksl = kTm[:, g * NPG + kc * 128:g * NPG + kc * 128 + 128]
                    qsl = qT[:, g * NPG:(g + 1) * NPG]
                    nc.tensor.matmul(sp[:, gi, :], ksl, qsl,
                                     start=True, stop=True)
                nc.scalar.activation(expT[:, kc, h, gh * 4:gh * 4 + 4, :],
                                     sp[:], AF.Exp)

    recip = big_p.tile([128, NPC], bdt, tag="recip", name="recip")
    for half in range(2):
        sm = ps2.tile([128, 1024], fdt, tag="ps2")
        for h in range(HEADS):
            for qc in range(2):
                osl = sm[32 * h:32 * h + 32, qc * 512:(qc + 1) * 512]
                for kc in range(2):
                    rhs = expT[:, kc, h, :, :].rearrange("p g q -> p (g q)")
                    nc.tensor.matmul(
                        osl, W("ones32", width=HD),
                        rhs[:, half * 1024 + qc * 512:half * 1024 + (qc + 1) * 512],
                        start=(kc == 0), stop=(kc == 1),
                        tile_position=(0, 32 * h))
        # 1/x = exp(-ln(x)) on ACT (Reciprocal AF is blocked; DVE recip is slow)
        for rc in range(2):
            lg = chunk_p.tile([128, 512], fdt, tag="lg")
            nc.scalar.activation(lg[:], sm[:, rc * 512:(rc + 1) * 512], AF.Ln)
            with nc.allow_low_precision(reason="softmax recip ok at 2e-2 gate"):
                nc.scalar.activation(
                    recip[:, half * 1024 + rc * 512:half * 1024 + (rc + 1) * 512],
                    lg[:], AF.Exp, scale=-1.0)

    o_bf = big_p.tile([C, NPC], bdt, tag="o_bf", name="o_bf")
    for g in range(NG_C):
        op = ps1.tile([128, NPG], fdt, tag="ps1")
        for h in range(HEADS):
            for kc in range(2):
                lhs = v_sb[:, g * 2 + kc, 32 * h:32 * h + 32]
                rhs = expT[:, kc, h, g, :]
                nc.tensor.matmul(op[32 * h:32 * h + 32, :], lhs, rhs,
                                 start=(kc == 0), stop=(kc == 1),
                                 tile_position=(0, 32 * h))
        nc.vector.tensor_mul(o_bf[:, g * NPG:(g + 1) * NPG], op[:],
                             recip[:, g * NPG:(g + 1) * NPG])

    # ---------------- GIN ----------------
    g_bf = big_p.tile([C, NPC], bdt, tag="g_bf")
    tab = tabs[s]
    cap_e = cap * 128                       # gathered rows per window
    for wg in range(NWIN // WGRP):
        nidx = WGRP * cap_e
        gath = gath_p.tile([128, WGRP * cap, C], bdt, tag="gath")
        base = wg * nidx
        off = 0
        qn = wg % 4
        while off < nidx:
            n = min(gcall, nidx - off)
            isl = gidx[:, (base + off) // 16:(base + off + n) // 16]
            nc.gpsimd.dma_gather(
                gath[:, off // 128:(off + n) // 128, :], tab[:], isl,
                n, n, C, queue_num=qn)
            off += n
        for wi in range(WGRP):
            w = wg * WGRP + wi
            ohs = oh_p.tile([128, cap, WIN], bdt, tag="ohs")
            nc.sync.dma_start(
                ohs[:].rearrange("p t c -> p (t c)"),
                oh_d[:][:, w * cap * WIN:(w + 1) * cap * WIN])
            ap = ps1.tile([C, WIN], fdt, tag="ps1")
            for t in range(cap):
                nc.tensor.matmul(ap[:], gath[:, wi * cap + t, :], ohs[:, t, :],
                                 start=(t == 0), stop=(t == cap - 1))
            nc.vector.tensor_add(g_bf[:, w * WIN:(w + 1) * WIN], ap[:],
                                 hT[:, w * WIN:(w + 1) * WIN])

    # GIN MLP + combine with attention: acc = h1 + h2
    acc_bf = big_p.tile([C, NPC], bdt, tag="acc_bf")
    r_bf = big_p.tile([C, NPC], bdt, tag="r_bf")
    for ch in range(2):
        sl = slice(ch * 1024, (ch + 1) * 1024)
        tp = ps2.tile([C, 1024], fdt, tag="ps2")
        _mm(nc, tp, W(f"gw1_{l}"), g_bf[:, sl], start=True, stop=True)
        nc.scalar.activation(r_bf[:, sl], tp[:], AF.Relu, bias=B(f"gb1_{l}"))
        up = ps2.tile([C, 1024], fdt, tag="ps2")
        _mm(nc, up, W(f"gw2_{l}"), r_bf[:, sl], start=True, stop=True)
        h1 = chunk_p.tile([C, 1024], fdt, tag="h1")
        nc.scalar.activation(h1[:], up[:], AFI, bias=B(f"sgb2_{l}"), scale=S_BN)
        ap2 = ps2.tile([C, 1024], fdt, tag="ps2")
        _mm(nc, ap2, W(f"ow_{l}"), o_bf[:, sl], start=True, stop=False)
        _mm(nc, ap2, W("I2"), hB[:, sl], start=False, stop=True)
        h2 = chunk_p.tile([C, 1024], fdt, tag="h2")
        nc.scalar.activation(h2[:], ap2[:], AFI, bias=B(f"sob_{l}"), scale=S_BN)
        nc.vector.tensor_add(acc_bf[:, sl], h1[:], h2[:])

    # MLP
    r2_bf = big_p.tile([C, 2, NPC], bdt, tag="r2_bf")
    for mh in range(2):
        for ch in range(2):
            sl = slice(ch * 1024, (ch + 1) * 1024)
            mp = ps2.tile([C, 1024], fdt, tag="ps2")
            _mm(nc, mp, W(f"m1_{l}", width=256)[:, mh * 128:(mh + 1) * 128],
                acc_bf[:, sl], start=True, stop=True)
            bname = f"m1ba_{l}" if mh == 0 else f"m1bb_{l}"
            nc.scalar.activation(r2_bf[:, mh, sl], mp[:], AF.Relu, bias=B(bname))
    for ch in range(2):
        sl = slice(ch * 1024, (ch + 1) * 1024)
        m2p = ps2.tile([C, 1024], fdt, tag="ps2")
        _mm(nc, m2p, W(f"m2a_{l}"), r2_bf[:, 0, sl], start=True, stop=False)
        _mm(nc, m2p, W(f"m2b_{l}"), r2_bf[:, 1, sl], start=False, stop=False)
        _mm(nc, m2p, W("I1"), acc_bf[:, sl], start=False, stop=True)
        dh = chunk_p.tile([C, 1024], fdt, tag="dh")
        nc.scalar.activation(dh[:], m2p[:], AFI, bias=B(f"smb2_{l}"), scale=S_BN)
        nc.vector.tensor_add(hT[:, sl], hT[:, sl], dh[:])
        nc.vector.tensor_copy(hB[:, sl], hT[:, sl])

    if not last:
        tabs[s] = emit_table(hB)


# ---------------------------------------------------------------------------
# Entry point
# ---------------------------------------------------------------------------

_CACHE = {}


def _get_kernel(caps, offs, boffs, wcols, bcols, **kw):
    key = (tuple(caps), wcols, bcols, tuple(sorted(kw.items())))
    if key not in _CACHE:
        _CACHE[key] = build_kernel(caps, offs, boffs, wcols, bcols, **kw)
    return _CACHE[key]


def kernel(**inputs):
    in_maps, caps, offs, boffs, wcols, bcols = _pack_host(inputs)
    nc = _get_kernel(caps, offs, boffs, wcols, bcols)
    res = run_bass_kernel_spmd(nc, in_maps, core_ids=list(range(N_CORES)))
    pools = []
    for si in range(3):
        parts = [np.asarray(res.results[c]["pool_out"][si])
                 for c in range(N_CORES)]
        full = np.concatenate(parts, axis=1)          # [C, 64]
        pools.append(np.ascontiguousarray(full.T).astype(np.float32))
    return tuple(pools)


# revision 8
# speedup vs baseline: 1.0886x; 1.0886x over previous
"""Trainium2 Bass kernel for nn_CGT_21354577396059 (GPS-style GNN, 3 streams x 3 layers).

Strategy (8 NeuronCores, SPMD):
- Node-shard: core c owns nodes [2048c, 2048c+2048) = 8 graphs of 256 nodes.
- Activations feature-major in SBUF: hT [C=128 partitions, 2048 nodes] fp32,
  bf16 copies as matmul inputs.
- GIN segment_sum: edges dst-sorted per core, deduped by src per 128-dst
  window (multi-hot count columns); src rows gathered from a bf16 node-major
  DRAM table (gpsimd dma_gather, 256B rows, one call per window-group);
  scatter via count-matrix matmuls aggT += gathered_chunk.T @ onehot_chunk.
- The bf16 node table is stored P-MAJOR per core block (row p*16+k holds node
  k*128+p) so the SBUF->DRAM table write is contiguous; gather indices are
  host-permuted to match. Table rebuilt each layer via PE transpose + 8-core
  AllGather.
- One-hot count matrices are stored partition-major in DRAM so their loads are
  contiguous (4KB+ per-partition descriptors instead of 256B).
- Attention is graph-local: scoresT = kT.T @ qT per (graph, head, key-chunk)
  masked-K; exp on ACT; softmax sums via col-tiled ones-matmuls; o via
  col-tiled matmuls contracted over keys; normalization via ACT reciprocal.

kernel(**inputs) takes the FULL unsharded inputs and returns
(pool(h0), pool(ha), pool(hb)) -- tuple of [64, 128] float32 -- like the reference.
"""
import sys
import numpy as np
import ml_dtypes

if "/opt/trn_rl_repo" not in sys.path:
    sys.path.insert(0, "/opt/trn_rl_repo")

import concourse.bass as bass  # noqa: F401
import concourse.tile as tile
from concourse import bacc, mybir
from concourse.bass_utils import run_bass_kernel_spmd

BF = ml_dtypes.bfloat16

# Problem constants (self-contained; no reads of /root/problem/*)
N_NODES = 16384
N_GRAPHS = 64
NPG = 256
FEA_DIM = 32
PE_DIM = 20
C = 128
HEADS = 4
HD = C // HEADS
L = 3
BN_EPS = 1e-5
S_BN = float(1.0 / np.sqrt(1.0 + BN_EPS))

N_CORES = 8
NPC = N_NODES // N_CORES   # 2048
NG_C = NPC // NPG          # 8 graphs per core
WIN = 128
NWIN = NPC // WIN          # 16
NCHUNK = NPC // 128        # 16
WGRP = 1                   # windows per dma_gather call

fdt = mybir.dt.float32
bdt = mybir.dt.bfloat16
i16 = mybir.dt.int16
AF = mybir.ActivationFunctionType
AX = mybir.AxisListType
ALU = mybir.AluOpType
AG_GROUPS = [list(range(N_CORES))]


# ---------------------------------------------------------------------------
# Host-side data prep
# ---------------------------------------------------------------------------

def _perm_row(n):
    """Global table row for node n with p-major per-core blocks."""
    cb, m = n // NPC, n % NPC
    return cb * NPC + (m % 128) * NCHUNK + m // 128


def _wrap_idxs(idx):
    """dma_gather idx layout [128, n/16] int16: idx i at (i%16, i//16),
    replicated across the 8 16-partition blocks."""
    n = len(idx)
    a = np.asarray(idx, np.int16).reshape(n // 16, 16).T
    return np.ascontiguousarray(np.tile(a, (8, 1)))


def _prep_edges_stream(edge_index):
    """Dedup by src per (core, 128-dst window); count-matrix columns.

    Returns (cap_chunks, [(gidx_wrapped, oh_pmajor)] per core) where
    oh_pmajor is [128, NWIN*cap*WIN] bf16 (partition-major onehot)."""
    src = np.asarray(edge_index[0]).astype(np.int64)
    dst = np.asarray(edge_index[1]).astype(np.int64)
    per_core_wins = []
    max_w = 0
    for c in range(N_CORES):
        m = (dst >= c * NPC) & (dst < (c + 1) * NPC)
        s, d = src[m], dst[m] - c * NPC
        wins = []
        for w in range(NWIN):
            mw = (d >= w * WIN) & (d < (w + 1) * WIN)
            sw, dw = s[mw], d[mw] - w * WIN
            # dedup srcs within the window; multi-hot count columns
            uniq, inv = np.unique(sw, return_inverse=True)
            cnt = np.zeros((len(uniq), WIN), np.float32)
            np.add.at(cnt, (inv, dw), 1.0)
            wins.append((uniq, cnt))
            max_w = max(max_w, len(uniq))
        per_core_wins.append(wins)
    cap_e = ((max_w + 127) // 128) * 128     # rows per window, padded
    cap = cap_e // 128
    out = []
    for c in range(N_CORES):
        srcs = np.zeros(NWIN * cap_e, np.int64)
        oh = np.zeros((NWIN, cap_e, WIN), np.float32)
        for w in range(NWIN):
            uniq, cnt = per_core_wins[c][w]
            n = len(uniq)
            srcs[w * cap_e:w * cap_e + n] = _perm_row(uniq)
            oh[w, :n, :] = cnt
        # partition-major: oh_pm[p, (w, t, c)] = oh[w, t*128+p, c]
        oh_pm = np.ascontiguousarray(
            oh.reshape(NWIN, cap, 128, WIN).transpose(2, 0, 1, 3)
            .reshape(128, NWIN * cap * WIN).astype(BF))
        out.append((_wrap_idxs(srcs), oh_pm))
    return cap, out


def _pack_host(inputs):
    inp = {k: np.asarray(v) for k, v in inputs.items()}
    rt2 = 1.0 / np.sqrt(HD)

    blocks, offs = [], {}

    def add(name, arr):
        arr = np.asarray(arr, np.float32)
        k, m = arr.shape
        buf = np.zeros((128, m), BF)
        buf[:k] = arr.astype(BF)
        offs[name] = sum(b.shape[1] for b in blocks)
        blocks.append(buf)

    add("emb", inp["node_emb_w"])
    add("pe", inp["pe_lin_w"])
    add("I2", 2.0 * np.eye(C))       # h2 fold: ACT scale s gives 2s*h
    add("I1", np.eye(C))             # transpose identity + acc fold
    add("ones32", np.ones((C, HD)))
    for l in range(L):
        aw = inp["attn_in_w"][l]
        add(f"gw1_{l}", inp["gin_w1"][l])
        add(f"gw2_{l}", inp["gin_w2"][l])
        add(f"qT_{l}", (aw[0:C] * rt2).T)
        for h in range(HEADS):
            km = aw[C:2 * C].T.copy()
            mask = np.zeros(C); mask[32 * h:32 * h + 32] = 1.0
            add(f"kTm{h}_{l}", km * mask[None, :])
        add(f"vT_{l}", aw[2 * C:3 * C].T)
        add(f"ow_{l}", inp["attn_out_w"][l].T)
        add(f"m1_{l}", inp["mlp_w1"][l])
        add(f"m2a_{l}", inp["mlp_w2"][l][0:C])
        add(f"m2b_{l}", inp["mlp_w2"][l][C:2 * C])
    wts = np.ascontiguousarray(np.concatenate(blocks, axis=1))

    bvecs, boffs = [], {}

    def addb(name, vec):
        vec = np.asarray(vec, np.float32).reshape(-1)
        assert vec.shape == (C,)
        boffs[name] = len(bvecs)
        bvecs.append(vec)

    addb("eb", inp["node_emb_b"] + inp["pe_lin_b"])
    for l in range(L):
        ab = inp["attn_in_b"][l]
        addb(f"gb1_{l}", inp["gin_b1"][l])
        addb(f"sgb2_{l}", S_BN * inp["gin_b2"][l])
        addb(f"qb_{l}", ab[0:C] * rt2)
        for h in range(HEADS):
            mask = np.zeros(C); mask[32 * h:32 * h + 32] = 1.0
            addb(f"kbm{h}_{l}", ab[C:2 * C] * mask)
        addb(f"sob_{l}", S_BN * inp["attn_out_b"][l])
        addb(f"m1ba_{l}", inp["mlp_b1"][l][0:C])
        addb(f"m1bb_{l}", inp["mlp_b1"][l][C:2 * C])
        addb(f"smb2_{l}", S_BN * inp["mlp_b2"][l])
    biases = np.ascontiguousarray(np.stack(bvecs, axis=1).astype(np.float32))

    vbias = np.ascontiguousarray(np.stack(
        [np.tile(inp["attn_in_b"][l][2 * C:3 * C], (128, 1)) for l in range(L)]
    ).astype(np.float32))

    caps, edges = [], []
    for key in ("edge_index", "edge_index1", "edge_index2"):
        cap, per_core = _prep_edges_stream(inp[key])
        caps.append(cap)
        edges.append(per_core)

    xs = [inp["x"], inp["x1"], inp["x2"]]
    in_maps = []
    for c in range(N_CORES):
        m = {"wts": wts, "biases": biases, "vbias": vbias}
        sl = slice(c * NPC, (c + 1) * NPC)
        for s in range(3):
            m[f"xT{s}"] = np.ascontiguousarray(xs[s][sl].T.astype(BF))
            m[f"gidx{s}"] = edges[s][c][0]
            m[f"onehot{s}"] = edges[s][c][1]
        m["peT"] = np.ascontiguousarray(inp["pe"][sl].T.astype(BF))
        in_maps.append(m)

    return in_maps, caps, offs, boffs, wts.shape[1], biases.shape[1]


# ---------------------------------------------------------------------------
# Kernel builder
# ---------------------------------------------------------------------------


def _mm(nc, out, lhsT, rhs, start, stop, nmax=512):
    """matmul with moving free dim split to <=512 (ISA limit)."""
    n = rhs.shape[-1]
    assert out.shape[-1] == n
    for i in range(0, n, nmax):
        j = min(i + nmax, n)
        nc.tensor.matmul(out[:, i:j], lhsT, rhs[:, i:j], start=start, stop=stop)


def build_kernel(caps, offs, boffs, wcols, bcols, n_layers=L, n_streams=3,
                 gcall=1024, dbg_stream=None):
    nc = bacc.Bacc("TRN2", target_bir_lowering=False, num_devices=N_CORES,
                   num_swdge_queues=4)

    wts_d = nc.dram_tensor("wts", [128, wcols], bdt, kind="ExternalInput")
    bias_d = nc.dram_tensor("biases", [128, bcols], fdt, kind="ExternalInput")
    vbias_d = nc.dram_tensor("vbias", [L, 128, 128], fdt, kind="ExternalInput")
    peT_d = nc.dram_tensor("peT", [PE_DIM, NPC], bdt, kind="ExternalInput")
    xT_d, gidx_d, oh_d = [], [], []
    for s in range(3):
        cap = caps[s]
        xT_d.append(nc.dram_tensor(f"xT{s}", [FEA_DIM, NPC], bdt,
                                   kind="ExternalInput"))
        gidx_d.append(nc.dram_tensor(f"gidx{s}", [128, NWIN * cap * 8], i16,
                                     kind="ExternalInput"))
        oh_d.append(nc.dram_tensor(f"onehot{s}", [128, NWIN * cap * WIN], bdt,
                                   kind="ExternalInput"))
    pool_out = nc.dram_tensor("pool_out", [3, C, NG_C], fdt,
                              kind="ExternalOutput")
    dbg_out = None
    if dbg_stream is not None:
        dbg_out = nc.dram_tensor("dbg_out", [C, NPC], fdt, kind="ExternalOutput")

    with tile.TileContext(nc) as tc:
        with (
            tc.tile_pool(name="const", bufs=1) as const_p,
            tc.tile_pool(name="hstate", bufs=1) as hstate_p,
            tc.tile_pool(name="big", bufs=1) as big_p,       # full-width tiles
            tc.tile_pool(name="chunk", bufs=2) as chunk_p,   # [C,1024]-ish tiles
            tc.tile_pool(name="gath", bufs=2) as gath_p,
            tc.tile_pool(name="ohp", bufs=2) as oh_p,
            tc.tile_pool(name="ps2", bufs=3, space="PSUM") as ps2,   # 2-bank
            tc.tile_pool(name="ps1", bufs=2, space="PSUM") as ps1,   # 1-bank
            tc.tile_pool(name="dram", bufs=4, space="DRAM") as dram_p,
        ):
            wts = const_p.tile([128, wcols], bdt)
            nc.sync.dma_start(wts[:], wts_d[:])
            bia = const_p.tile([128, bcols], fdt)
            nc.sync.dma_start(bia[:], bias_d[:])
            vbia = const_p.tile([128, L, 128], fdt)
            nc.sync.dma_start(vbia[:], vbias_d[:].rearrange("l p c -> p l c"))

            def W(name, width=128, rows=128):
                return wts[0:rows, offs[name]:offs[name] + width]

            def B(name):
                return bia[:, boffs[name]:boffs[name] + 1]

            hT = [hstate_p.tile([C, NPC], fdt, tag=f"hT{s}", name=f"hT{s}")
                  for s in range(3)]
            hB = [hstate_p.tile([C, NPC], bdt, tag=f"hB{s}", name=f"hB{s}")
                  for s in range(3)]
            gidx = []
            for s in range(n_streams):
                t = hstate_p.tile([128, NWIN * caps[s] * 8], i16,
                                  tag=f"gidx{s}", name=f"gidx{s}")
                nc.sync.dma_start(t[:], gidx_d[s][:])
                gidx.append(t)

            def emit_table(src_bf):
                """feature-major bf16 [C, NPC] -> p-major node table ->
                AllGather. Block layout: DRAM row p*NCHUNK+k = node k*128+p."""
                nm = big_p.tile([128, NCHUNK, C], bdt, tag="nm")
                for k in range(NCHUNK):
                    pt = ps1.tile([128, 128], bdt, tag="ps1", name="pt")
                    nc.tensor.transpose(pt[:], src_bf[:, k * 128:(k + 1) * 128],
                                        W("I1"))
                    nc.vector.tensor_copy(nm[:, k, :], pt[:])
                agi = dram_p.tile([128, NCHUNK * C], bdt, tag="agi")
                nc.sync.dma_start(agi[:], nm[:].rearrange("p k c -> p (k c)"))
                tab = dram_p.tile([N_NODES, C], bdt, tag="tab", name="tab")
                nc.gpsimd.collective_compute(
                    "AllGather", ALU.bypass, replica_groups=AG_GROUPS,
                    ins=[agi.opt()], outs=[tab.opt()])
                return tab

            # ---------------- embedding ----------------
            tabs = [None] * 3
            for s in range(n_streams):
                xT = chunk_p.tile([FEA_DIM, NPC], bdt, tag="xT")
                nc.sync.dma_start(xT[:], xT_d[s][:])
                peT = chunk_p.tile([PE_DIM, NPC], bdt, tag="peT")
                nc.sync.dma_start(peT[:], peT_d[:])
                for ch in range(2):
                    sl = slice(ch * 1024, (ch + 1) * 1024)
                    ep = ps2.tile([C, 1024], fdt, tag="ps2")
                    _mm(nc, ep, W("emb", rows=FEA_DIM), xT[:, sl],
                        start=True, stop=False)
                    _mm(nc, ep, W("pe", rows=PE_DIM), peT[:, sl],
                        start=False, stop=True)
                    nc.scalar.activation(hT[s][:, sl], ep[:], AF.Identity,
                                         bias=B("eb"))
                    nc.vector.tensor_copy(hB[s][:, sl], hT[s][:, sl])
                tabs[s] = emit_table(hB[s])

            # ---------------- layers ----------------
            for l in range(n_layers):
                for s in range(n_streams):
                    _gps_layer(nc, l, caps[s], hT[s], hB[s], tabs, s,
                               gidx[s], oh_d[s], W, B, vbia,
                               big_p, chunk_p, gath_p, oh_p, ps2, ps1,
                               emit_table, gcall, last=(l == n_layers - 1))

            # ---------------- pooling ----------------
            for s in range(n_streams):
                po = chunk_p.tile([C, NG_C], fdt, tag="pool")
                nc.vector.reduce_sum(
                    out=po[:],
                    in_=hT[s][:].rearrange("c (g n) -> c g n", g=NG_C),
                    axis=AX.X)
                nc.sync.dma_start(pool_out[s], po[:])

            if dbg_stream is not None:
                nc.sync.dma_start(dbg_out[:], hT[dbg_stream][:])

    nc.compile()
    return nc


def _gps_layer(nc, l, cap, hT, hB, tabs, s, gidx, oh_d, W, B, vbia,
               big_p, chunk_p, gath_p, oh_p, ps2, ps1, emit_table, gcall,
               last):
    AFI = AF.Identity

    # ---------------- attention (graph-local) ----------------
    qT = big_p.tile([C, NPC], bdt, tag="qT", name="qT")
    for ch in range(2):
        sl = slice(ch * 1024, (ch + 1) * 1024)
        qp = ps2.tile([C, 1024], fdt, tag="ps2")
        _mm(nc, qp, W(f"qT_{l}"), hB[:, sl], start=True, stop=True)
        nc.scalar.activation(qT[:, sl], qp[:], AFI, bias=B(f"qb_{l}"))
    v_sb = big_p.tile([128, NCHUNK, C], bdt, tag="v_sb", name="v_sb")
    for k in range(NCHUNK):
        vp = ps1.tile([128, C], fdt, tag="ps1")
        nc.tensor.matmul(vp[:], hB[:, k * 128:(k + 1) * 128], W(f"vT_{l}"),
                         start=True, stop=True)
        nc.vector.tensor_add(v_sb[:, k, :], vp[:], vbia[:, l, :])

    expT = big_p.tile([128, 2, HEADS, NG_C, NPG], bdt, tag="expT", name="expT")
    for h in range(HEADS):
        kTm = big_p.tile([C, NPC], bdt, tag="kTm", name="kTm")
        for ch in range(2):
            sl = slice(ch * 1024, (ch + 1) * 1024)
            kp = ps2.tile([C, 1024], fdt, tag="ps2")
            _mm(nc, kp, W(f"kTm{h}_{l}"), hB[:, sl], start=True, stop=True)
            nc.scalar.activation(kTm[:, sl], kp[:], AFI, bias=B(f"kbm{h}_{l}"))
        for kc in range(2):
            for gh in range(2):
                sp = ps2.tile([128, 4, NPG], fdt, tag="ps2")
                for gi in range(4):
                    g = gh * 4 + gi
                    ksl = kTm[:, g * NPG + kc * 128:g * NPG + kc * 128 + 128]
                    qsl = qT[:, g * NPG:(g + 1) * NPG]
                    nc.tensor.matmul(sp[:, gi, :], ksl, qsl,
                                     start=True, stop=True)
                nc.scalar.activation(expT[:, kc, h, gh * 4:gh * 4 + 4, :],
                                     sp[:], AF.Exp)

    recip = big_p.tile([128, NPC], bdt, tag="recip", name="recip")
    for half in range(2):
        sm = ps2.tile([128, 1024], fdt, tag="ps2")
        for h in range(HEADS):
            for qc in range(2):
                osl = sm[32 * h:32 * h + 32, qc * 512:(qc + 1) * 512]
                for kc in range(2):
                    rhs = expT[:, kc, h, :, :].rearrange("p g q -> p (g q)")
                    nc.tensor.matmul(
                        osl, W("ones32", width=HD),
                        rhs[:, half * 1024 + qc * 512:half * 1024 + (qc + 1) * 512],
                        start=(kc == 0), stop=(kc == 1),
                        tile_position=(0, 32 * h))
        with nc.allow_low_precision(reason="softmax recip bf16 ok at 2e-2 gate"):
            nc.vector.reciprocal(recip[:, half * 1024:(half + 1) * 1024], sm[:])

    o_bf = big_p.tile([C, NPC], bdt, tag="o_bf", name="o_bf")
    for g in range(NG_C):
        op = ps1.tile([128, NPG], fdt, tag="ps1")
        for h in range(HEADS):
            for kc in range(2):
                lhs = v_sb[:, g * 2 + kc, 32 * h:32 * h + 32]
                rhs = expT[:, kc, h, g, :]
                nc.tensor.matmul(op[32 * h:32 * h + 32, :], lhs, rhs,
                                 start=(kc == 0), stop=(kc == 1),
                                 tile_position=(0, 32 * h))
        nc.vector.tensor_mul(o_bf[:, g * NPG:(g + 1) * NPG], op[:],
                             recip[:, g * NPG:(g + 1) * NPG])

    # ---------------- GIN ----------------
    g_bf = big_p.tile([C, NPC], bdt, tag="g_bf")
    tab = tabs[s]
    cap_e = cap * 128                       # gathered rows per window
    qn = [0]
    for wg in range(NWIN // WGRP):
        nidx = WGRP * cap_e
        gath = gath_p.tile([128, WGRP * cap, C], bdt, tag="gath")
        base = wg * nidx
        off = 0
        while off < nidx:
            n = min(gcall, nidx - off)
            isl = gidx[:, (base + off) // 16:(base + off + n) // 16]
            nc.gpsimd.dma_gather(
                gath[:, off // 128:(off + n) // 128, :], tab[:], isl,
                n, n, C, queue_num=qn[0])
            qn[0] = (qn[0] + 1) % 4
            off += n
        for wi in range(WGRP):
            w = wg * WGRP + wi
            ohs = oh_p.tile([128, cap, WIN], bdt, tag="ohs")
            nc.sync.dma_start(
                ohs[:].rearrange("p t c -> p (t c)"),
                oh_d[:][:, w * cap * WIN:(w + 1) * cap * WIN])
            ap = ps1.tile([C, WIN], fdt, tag="ps1")
            for t in range(cap):
                nc.tensor.matmul(ap[:], gath[:, wi * cap + t, :], ohs[:, t, :],
                                 start=(t == 0), stop=(t == cap - 1))
            nc.vector.tensor_add(g_bf[:, w * WIN:(w + 1) * WIN], ap[:],
                                 hT[:, w * WIN:(w + 1) * WIN])

    # GIN MLP + combine with attention: acc = h1 + h2
    acc_bf = big_p.tile([C, NPC], bdt, tag="acc_bf")
    r_bf = big_p.tile([C, NPC], bdt, tag="r_bf")
    for ch in range(2):
        sl = slice(ch * 1024, (ch + 1) * 1024)
        tp = ps2.tile([C, 1024], fdt, tag="ps2")
        _mm(nc, tp, W(f"gw1_{l}"), g_bf[:, sl], start=True, stop=True)
        nc.scalar.activation(r_bf[:, sl], tp[:], AF.Relu, bias=B(f"gb1_{l}"))
        up = ps2.tile([C, 1024], fdt, tag="ps2")
        _mm(nc, up, W(f"gw2_{l}"), r_bf[:, sl], start=True, stop=True)
        h1 = chunk_p.tile([C, 1024], fdt, tag="h1")
        nc.scalar.activation(h1[:], up[:], AFI, bias=B(f"sgb2_{l}"), scale=S_BN)
        ap2 = ps2.tile([C, 1024], fdt, tag="ps2")
        _mm(nc, ap2, W(f"ow_{l}"), o_bf[:, sl], start=True, stop=False)
        _mm(nc, ap2, W("I2"), hB[:, sl], start=False, stop=True)
        h2 = chunk_p.tile([C, 1024], fdt, tag="h2")
        nc.scalar.activation(h2[:], ap2[:], AFI, bias=B(f"sob_{l}"), scale=S_BN)
        nc.vector.tensor_add(acc_bf[:, sl], h1[:], h2[:])

    # MLP
    r2_bf = big_p.tile([C, 2, NPC], bdt, tag="r2_bf")
    for mh in range(2):
        for ch in range(2):
            sl = slice(ch * 1024, (ch + 1) * 1024)
            mp = ps2.tile([C, 1024], fdt, tag="ps2")
            _mm(nc, mp, W(f"m1_{l}", width=256)[:, mh * 128:(mh + 1) * 128],
                acc_bf[:, sl], start=True, stop=True)
            bname = f"m1ba_{l}" if mh == 0 else f"m1bb_{l}"
            nc.scalar.activation(r2_bf[:, mh, sl], mp[:], AF.Relu, bias=B(bname))
    for ch in range(2):
        sl = slice(ch * 1024, (ch + 1) * 1024)
        m2p = ps2.tile([C, 1024], fdt, tag="ps2")
        _mm(nc, m2p, W(f"m2a_{l}"), r2_bf[:, 0, sl], start=True, stop=False)
        _mm(nc, m2p, W(f"m2b_{l}"), r2_bf[:, 1, sl], start=False, stop=False)
        _mm(nc, m2p, W("I1"), acc_bf[:, sl], start=False, stop=True)
        dh = chunk_p.tile([C, 1024], fdt, tag="dh")
        nc.scalar.activation(dh[:], m2p[:], AFI, bias=B(f"smb2_{l}"), scale=S_BN)
        nc.vector.tensor_add(hT[:, sl], hT[:, sl], dh[:])
        nc.vector.tensor_copy(hB[:, sl], hT[:, sl])

    if not last:
        tabs[s] = emit_table(hB)


# ---------------------------------------------------------------------------
# Entry point
# ---------------------------------------------------------------------------

_CACHE = {}


def _get_kernel(caps, offs, boffs, wcols, bcols, **kw):
    key = (tuple(caps), wcols, bcols, tuple(sorted(kw.items())))
    if key not in _CACHE:
        _CACHE[key] = build_kernel(caps, offs, boffs, wcols, bcols, **kw)
    return _CACHE[key]


def kernel(**inputs):
    in_maps, caps, offs, boffs, wcols, bcols = _pack_host(inputs)
    nc = _get_kernel(caps, offs, boffs, wcols, bcols)
    res = run_bass_kernel_spmd(nc, in_maps, core_ids=list(range(N_CORES)))
    pools = []
    for si in range(3):
        parts = [np.asarray(res.results[c]["pool_out"][si])
                 for c in range(N_CORES)]
        full = np.concatenate(parts, axis=1)          # [C, 64]
        pools.append(np.ascontiguousarray(full.T).astype(np.float32))
    return tuple(pools)


# revision 11
# speedup vs baseline: 1.2981x; 1.1925x over previous
"""Trainium2 Bass kernel for nn_CGT_21354577396059 (GPS-style GNN, 3 streams x 3 layers).

Strategy (8 NeuronCores, SPMD):
- Node-shard: core c owns nodes [2048c, 2048c+2048) = 8 graphs of 256 nodes.
- Activations feature-major in SBUF: hT [C=128 partitions, 2048 nodes] fp32,
  bf16 copies as matmul inputs.
- GIN segment_sum: edges dst-sorted per core, deduped by src per 128-dst
  window (multi-hot count columns); src rows gathered from a bf16 node-major
  DRAM table (gpsimd dma_gather, 256B rows, one call per window-group);
  scatter via count-matrix matmuls aggT += gathered_chunk.T @ onehot_chunk.
- The bf16 node table is stored P-MAJOR per core block (row p*16+k holds node
  k*128+p) so the SBUF->DRAM table write is contiguous; gather indices are
  host-permuted to match. Table rebuilt each layer via PE transpose + 8-core
  AllGather.
- One-hot count matrices are stored partition-major in DRAM so their loads are
  contiguous (4KB+ per-partition descriptors instead of 256B).
- Attention is graph-local: scoresT = kT.T @ qT per (graph, head, key-chunk)
  masked-K; exp on ACT; softmax sums via col-tiled ones-matmuls; o via
  col-tiled matmuls contracted over keys; normalization via ACT reciprocal.

kernel(**inputs) takes the FULL unsharded inputs and returns
(pool(h0), pool(ha), pool(hb)) -- tuple of [64, 128] float32 -- like the reference.
"""
import sys
import numpy as np
import ml_dtypes

if "/opt/trn_rl_repo" not in sys.path:
    sys.path.insert(0, "/opt/trn_rl_repo")

import concourse.bass as bass  # noqa: F401
import concourse.tile as tile
from concourse import bacc, mybir
from concourse.bass_utils import run_bass_kernel_spmd

BF = ml_dtypes.bfloat16

# Problem constants (self-contained; no reads of /root/problem/*)
N_NODES = 16384
N_GRAPHS = 64
NPG = 256
FEA_DIM = 32
PE_DIM = 20
C = 128
HEADS = 4
HD = C // HEADS
L = 3
BN_EPS = 1e-5
S_BN = float(1.0 / np.sqrt(1.0 + BN_EPS))

N_CORES = 8
NPC = N_NODES // N_CORES   # 2048
NG_C = NPC // NPG          # 8 graphs per core
WIN = 128
NWIN = NPC // WIN          # 16
NCHUNK = NPC // 128        # 16
WGRP = 1                   # windows per dma_gather call

fdt = mybir.dt.float32
bdt = mybir.dt.bfloat16
i16 = mybir.dt.int16
AF = mybir.ActivationFunctionType
AX = mybir.AxisListType
ALU = mybir.AluOpType
AG_GROUPS = [list(range(N_CORES))]


# ---------------------------------------------------------------------------
# Host-side data prep
# ---------------------------------------------------------------------------

def _perm_row(n):
    """Global table row for node n with p-major per-core blocks."""
    cb, m = n // NPC, n % NPC
    return cb * NPC + (m % 128) * NCHUNK + m // 128


def _wrap_idxs(idx):
    """dma_gather idx layout [128, n/16] int16: idx i at (i%16, i//16),
    replicated across the 8 16-partition blocks."""
    n = len(idx)
    a = np.asarray(idx, np.int16).reshape(n // 16, 16).T
    return np.ascontiguousarray(np.tile(a, (8, 1)))


def _prep_edges_stream(edge_index):
    """Dedup by src per (core, 128-dst window); count-matrix columns.

    Returns (cap_chunks, [(gidx_wrapped, oh_pmajor)] per core) where
    oh_pmajor is [128, NWIN*cap*WIN] bf16 (partition-major onehot)."""
    src = np.asarray(edge_index[0]).astype(np.int64)
    dst = np.asarray(edge_index[1]).astype(np.int64)
    per_core_wins = []
    max_w = 0
    for c in range(N_CORES):
        m = (dst >= c * NPC) & (dst < (c + 1) * NPC)
        s, d = src[m], dst[m] - c * NPC
        wins = []
        for w in range(NWIN):
            mw = (d >= w * WIN) & (d < (w + 1) * WIN)
            sw, dw = s[mw], d[mw] - w * WIN
            # dedup srcs within the window; multi-hot count columns
            uniq, inv = np.unique(sw, return_inverse=True)
            cnt = np.zeros((len(uniq), WIN), np.float32)
            np.add.at(cnt, (inv, dw), 1.0)
            # sort rows by permuted table address for DMA page locality
            order = np.argsort(_perm_row(uniq), kind="stable")
            uniq, cnt = uniq[order], cnt[order]
            wins.append((uniq, cnt))
            max_w = max(max_w, len(uniq))
        per_core_wins.append(wins)
    cap_e = ((max_w + 127) // 128) * 128     # rows per window, padded
    cap = cap_e // 128
    out = []
    for c in range(N_CORES):
        srcs = np.zeros(NWIN * cap_e, np.int64)
        oh = np.zeros((NWIN, cap_e, WIN), np.float32)
        for w in range(NWIN):
            uniq, cnt = per_core_wins[c][w]
            n = len(uniq)
            srcs[w * cap_e:w * cap_e + n] = _perm_row(uniq)
            oh[w, :n, :] = cnt
        # partition-major: oh_pm[p, (w, t, c)] = oh[w, t*128+p, c]
        oh_pm = np.ascontiguousarray(
            oh.reshape(NWIN, cap, 128, WIN).transpose(2, 0, 1, 3)
            .reshape(128, NWIN * cap * WIN).astype(BF))
        out.append((_wrap_idxs(srcs), oh_pm))
    return cap, out


def _pack_host(inputs):
    inp = {k: np.asarray(v) for k, v in inputs.items()}
    rt2 = 1.0 / np.sqrt(HD)

    blocks, offs = [], {}

    def add(name, arr):
        arr = np.asarray(arr, np.float32)
        k, m = arr.shape
        buf = np.zeros((128, m), BF)
        buf[:k] = arr.astype(BF)
        offs[name] = sum(b.shape[1] for b in blocks)
        blocks.append(buf)

    add("emb", inp["node_emb_w"])
    add("pe", inp["pe_lin_w"])
    add("I2", 2.0 * np.eye(C))       # h2 fold: ACT scale s gives 2s*h
    add("I1", np.eye(C))             # transpose identity + acc fold
    add("ones32", np.ones((C, HD)))
    for l in range(L):
        aw = inp["attn_in_w"][l]
        add(f"gw1_{l}", inp["gin_w1"][l])
        add(f"gw2_{l}", inp["gin_w2"][l])
        add(f"qT_{l}", (aw[0:C] * rt2).T)
        for h in range(HEADS):
            km = aw[C:2 * C].T.copy()
            mask = np.zeros(C); mask[32 * h:32 * h + 32] = 1.0
            add(f"kTm{h}_{l}", km * mask[None, :])
        add(f"vT_{l}", aw[2 * C:3 * C].T)
        add(f"ow_{l}", inp["attn_out_w"][l].T)
        add(f"m1_{l}", inp["mlp_w1"][l])
        add(f"m2a_{l}", inp["mlp_w2"][l][0:C])
        add(f"m2b_{l}", inp["mlp_w2"][l][C:2 * C])
    wts = np.ascontiguousarray(np.concatenate(blocks, axis=1))

    bvecs, boffs = [], {}

    def addb(name, vec):
        vec = np.asarray(vec, np.float32).reshape(-1)
        assert vec.shape == (C,)
        boffs[name] = len(bvecs)
        bvecs.append(vec)

    addb("eb", inp["node_emb_b"] + inp["pe_lin_b"])
    for l in range(L):
        ab = inp["attn_in_b"][l]
        addb(f"gb1_{l}", inp["gin_b1"][l])
        addb(f"sgb2_{l}", S_BN * inp["gin_b2"][l])
        addb(f"qb_{l}", ab[0:C] * rt2)
        for h in range(HEADS):
            mask = np.zeros(C); mask[32 * h:32 * h + 32] = 1.0
            addb(f"kbm{h}_{l}", ab[C:2 * C] * mask)
        addb(f"sob_{l}", S_BN * inp["attn_out_b"][l])
        addb(f"m1ba_{l}", inp["mlp_b1"][l][0:C])
        addb(f"m1bb_{l}", inp["mlp_b1"][l][C:2 * C])
        addb(f"smb2_{l}", S_BN * inp["mlp_b2"][l])
    biases = np.ascontiguousarray(np.stack(bvecs, axis=1).astype(np.float32))

    vbias = np.ascontiguousarray(np.stack(
        [np.tile(inp["attn_in_b"][l][2 * C:3 * C], (128, 1)) for l in range(L)]
    ).astype(np.float32))

    caps, edges = [], []
    for key in ("edge_index", "edge_index1", "edge_index2"):
        cap, per_core = _prep_edges_stream(inp[key])
        caps.append(cap)
        edges.append(per_core)

    xs = [inp["x"], inp["x1"], inp["x2"]]
    in_maps = []
    for c in range(N_CORES):
        m = {"wts": wts, "biases": biases, "vbias": vbias}
        sl = slice(c * NPC, (c + 1) * NPC)
        for s in range(3):
            m[f"xT{s}"] = np.ascontiguousarray(xs[s][sl].T.astype(BF))
            m[f"gidx{s}"] = edges[s][c][0]
            m[f"onehot{s}"] = edges[s][c][1]
        m["peT"] = np.ascontiguousarray(inp["pe"][sl].T.astype(BF))
        in_maps.append(m)

    return in_maps, caps, offs, boffs, wts.shape[1], biases.shape[1]


# ---------------------------------------------------------------------------
# Kernel builder
# ---------------------------------------------------------------------------


def _mm(nc, out, lhsT, rhs, start, stop, nmax=512):
    """matmul with moving free dim split to <=512 (ISA limit)."""
    n = rhs.shape[-1]
    assert out.shape[-1] == n
    for i in range(0, n, nmax):
        j = min(i + nmax, n)
        nc.tensor.matmul(out[:, i:j], lhsT, rhs[:, i:j], start=start, stop=stop)


def build_kernel(caps, offs, boffs, wcols, bcols, n_layers=L, n_streams=3,
                 gcall=1024, dbg_stream=None):
    nc = bacc.Bacc("TRN2", target_bir_lowering=False, num_devices=N_CORES,
                   num_swdge_queues=4)

    wts_d = nc.dram_tensor("wts", [128, wcols], bdt, kind="ExternalInput")
    bias_d = nc.dram_tensor("biases", [128, bcols], fdt, kind="ExternalInput")
    vbias_d = nc.dram_tensor("vbias", [L, 128, 128], fdt, kind="ExternalInput")
    peT_d = nc.dram_tensor("peT", [PE_DIM, NPC], bdt, kind="ExternalInput")
    xT_d, gidx_d, oh_d = [], [], []
    for s in range(3):
        cap = caps[s]
        xT_d.append(nc.dram_tensor(f"xT{s}", [FEA_DIM, NPC], bdt,
                                   kind="ExternalInput"))
        gidx_d.append(nc.dram_tensor(f"gidx{s}", [128, NWIN * cap * 8], i16,
                                     kind="ExternalInput"))
        oh_d.append(nc.dram_tensor(f"onehot{s}", [128, NWIN * cap * WIN], bdt,
                                   kind="ExternalInput"))
    pool_out = nc.dram_tensor("pool_out", [3, C, NG_C], fdt,
                              kind="ExternalOutput")
    dbg_out = None
    if dbg_stream is not None:
        dbg_out = nc.dram_tensor("dbg_out", [C, NPC], fdt, kind="ExternalOutput")

    with tile.TileContext(nc) as tc:
        with (
            tc.tile_pool(name="const", bufs=1) as const_p,
            tc.tile_pool(name="hstate", bufs=1) as hstate_p,
            tc.tile_pool(name="big", bufs=1) as big_p,       # full-width tiles
            tc.tile_pool(name="chunk", bufs=2) as chunk_p,   # [C,1024]-ish tiles
            tc.tile_pool(name="gath", bufs=4) as gath_p,
            tc.tile_pool(name="ohp", bufs=2) as oh_p,
            tc.tile_pool(name="ps2", bufs=3, space="PSUM") as ps2,   # 2-bank
            tc.tile_pool(name="ps1", bufs=2, space="PSUM") as ps1,   # 1-bank
            tc.tile_pool(name="dram", bufs=4, space="DRAM") as dram_p,
        ):
            wts = const_p.tile([128, wcols], bdt)
            nc.sync.dma_start(wts[:], wts_d[:])
            bia = const_p.tile([128, bcols], fdt)
            nc.sync.dma_start(bia[:], bias_d[:])
            vbia = const_p.tile([128, L, 128], fdt)
            nc.sync.dma_start(vbia[:], vbias_d[:].rearrange("l p c -> p l c"))

            def W(name, width=128, rows=128):
                return wts[0:rows, offs[name]:offs[name] + width]

            def B(name):
                return bia[:, boffs[name]:boffs[name] + 1]

            hT = [hstate_p.tile([C, NPC], fdt, tag=f"hT{s}", name=f"hT{s}")
                  for s in range(3)]
            hB = [hstate_p.tile([C, NPC], bdt, tag=f"hB{s}", name=f"hB{s}")
                  for s in range(3)]
            gidx = []
            for s in range(n_streams):
                t = hstate_p.tile([128, NWIN * caps[s] * 8], i16,
                                  tag=f"gidx{s}", name=f"gidx{s}")
                nc.sync.dma_start(t[:], gidx_d[s][:])
                gidx.append(t)

            def emit_table(src_bf):
                """feature-major bf16 [C, NPC] -> p-major node table ->
                AllGather. Block layout: DRAM row p*NCHUNK+k = node k*128+p."""
                nm = big_p.tile([128, NCHUNK, C], bdt, tag="nm")
                for k in range(NCHUNK):
                    pt = ps1.tile([128, 128], bdt, tag="ps1", name="pt")
                    nc.tensor.transpose(pt[:], src_bf[:, k * 128:(k + 1) * 128],
                                        W("I1"))
                    nc.vector.tensor_copy(nm[:, k, :], pt[:])
                agi = dram_p.tile([128, NCHUNK * C], bdt, tag="agi")
                nc.sync.dma_start(agi[:], nm[:].rearrange("p k c -> p (k c)"))
                tab = dram_p.tile([N_NODES, C], bdt, tag="tab", name="tab")
                nc.gpsimd.collective_compute(
                    "AllGather", ALU.bypass, replica_groups=AG_GROUPS,
                    ins=[agi.opt()], outs=[tab.opt()])
                return tab

            # ---------------- embedding ----------------
            tabs = [None] * 3
            for s in range(n_streams):
                xT = chunk_p.tile([FEA_DIM, NPC], bdt, tag="xT")
                nc.sync.dma_start(xT[:], xT_d[s][:])
                peT = chunk_p.tile([PE_DIM, NPC], bdt, tag="peT")
                nc.sync.dma_start(peT[:], peT_d[:])
                for ch in range(2):
                    sl = slice(ch * 1024, (ch + 1) * 1024)
                    ep = ps2.tile([C, 1024], fdt, tag="ps2")
                    _mm(nc, ep, W("emb", rows=FEA_DIM), xT[:, sl],
                        start=True, stop=False)
                    _mm(nc, ep, W("pe", rows=PE_DIM), peT[:, sl],
                        start=False, stop=True)
                    nc.scalar.activation(hT[s][:, sl], ep[:], AF.Identity,
                                         bias=B("eb"))
                    nc.vector.tensor_copy(hB[s][:, sl], hT[s][:, sl])
                tabs[s] = emit_table(hB[s])

            # ---------------- layers ----------------
            for l in range(n_layers):
                for s in range(n_streams):
                    _gps_layer(nc, l, caps[s], hT[s], hB[s], tabs, s,
                               gidx[s], oh_d[s], W, B, vbia,
                               big_p, chunk_p, gath_p, oh_p, ps2, ps1,
                               emit_table, gcall, last=(l == n_layers - 1))

            # ---------------- pooling ----------------
            for s in range(n_streams):
                po = chunk_p.tile([C, NG_C], fdt, tag="pool")
                nc.vector.reduce_sum(
                    out=po[:],
                    in_=hT[s][:].rearrange("c (g n) -> c g n", g=NG_C),
                    axis=AX.X)
                nc.sync.dma_start(pool_out[s], po[:])

            if dbg_stream is not None:
                nc.sync.dma_start(dbg_out[:], hT[dbg_stream][:])

    nc.compile()
    return nc


def _gps_layer(nc, l, cap, hT, hB, tabs, s, gidx, oh_d, W, B, vbia,
               big_p, chunk_p, gath_p, oh_p, ps2, ps1, emit_table, gcall,
               last):
    AFI = AF.Identity

    # ---------------- attention (graph-local) ----------------
    qT = big_p.tile([C, NPC], bdt, tag="qT", name="qT")
    for ch in range(2):
        sl = slice(ch * 1024, (ch + 1) * 1024)
        qp = ps2.tile([C, 1024], fdt, tag="ps2")
        _mm(nc, qp, W(f"qT_{l}"), hB[:, sl], start=True, stop=True)
        nc.scalar.activation(qT[:, sl], qp[:], AFI, bias=B(f"qb_{l}"))
    v_sb = big_p.tile([128, NCHUNK, C], bdt, tag="v_sb", name="v_sb")
    for k in range(NCHUNK):
        vp = ps1.tile([128, C], fdt, tag="ps1")
        nc.tensor.matmul(vp[:], hB[:, k * 128:(k + 1) * 128], W(f"vT_{l}"),
                         start=True, stop=True)
        nc.vector.tensor_add(v_sb[:, k, :], vp[:], vbia[:, l, :])

    expT = big_p.tile([128, 2, HEADS, NG_C, NPG], bdt, tag="expT", name="expT")
    for h in range(HEADS):
        kTm = big_p.tile([C, NPC], bdt, tag="kTm", name="kTm")
        for ch in range(2):
            sl = slice(ch * 1024, (ch + 1) * 1024)
            kp = ps2.tile([C, 1024], fdt, tag="ps2")
            _mm(nc, kp, W(f"kTm{h}_{l}"), hB[:, sl], start=True, stop=True)
            nc.scalar.activation(kTm[:, sl], kp[:], AFI, bias=B(f"kbm{h}_{l}"))
        for kc in range(2):
            for gh in range(2):
                sp = ps2.tile([128, 4, NPG], fdt, tag="ps2")
                for gi in range(4):
                    g = gh * 4 + gi
                    ksl = kTm[:, g * NPG + kc * 128:g * NPG + kc * 128 + 128]
                    qsl = qT[:, g * NPG:(g + 1) * NPG]
                    nc.tensor.matmul(sp[:, gi, :], ksl, qsl,
                                     start=True, stop=True)
                nc.scalar.activation(expT[:, kc, h, gh * 4:gh * 4 + 4, :],
                                     sp[:], AF.Exp)

    recip = big_p.tile([128, NPC], bdt, tag="recip", name="recip")
    for half in range(2):
        sm = ps2.tile([128, 1024], fdt, tag="ps2")
        for h in range(HEADS):
            for qc in range(2):
                osl = sm[32 * h:32 * h + 32, qc * 512:(qc + 1) * 512]
                for kc in range(2):
                    rhs = expT[:, kc, h, :, :].rearrange("p g q -> p (g q)")
                    nc.tensor.matmul(
                        osl, W("ones32", width=HD),
                        rhs[:, half * 1024 + qc * 512:half * 1024 + (qc + 1) * 512],
                        start=(kc == 0), stop=(kc == 1),
                        tile_position=(0, 32 * h))
        with nc.allow_low_precision(reason="softmax recip bf16 ok at 2e-2 gate"):
            nc.vector.reciprocal(recip[:, half * 1024:(half + 1) * 1024], sm[:])

    o_bf = big_p.tile([C, NPC], bdt, tag="o_bf", name="o_bf")
    for g in range(NG_C):
        op = ps1.tile([128, NPG], fdt, tag="ps1")
        for h in range(HEADS):
            for kc in range(2):
                lhs = v_sb[:, g * 2 + kc, 32 * h:32 * h + 32]
                rhs = expT[:, kc, h, g, :]
                nc.tensor.matmul(op[32 * h:32 * h + 32, :], lhs, rhs,
                                 start=(kc == 0), stop=(kc == 1),
                                 tile_position=(0, 32 * h))
        nc.vector.tensor_mul(o_bf[:, g * NPG:(g + 1) * NPG], op[:],
                             recip[:, g * NPG:(g + 1) * NPG])

    # ---------------- GIN ----------------
    g_bf = big_p.tile([C, NPC], bdt, tag="g_bf")
    tab = tabs[s]
    cap_e = cap * 128                       # gathered rows per window
    qn = [0]
    for wg in range(NWIN // WGRP):
        nidx = WGRP * cap_e
        gath = gath_p.tile([128, WGRP * cap, C], bdt, tag="gath")
        base = wg * nidx
        off = 0
        while off < nidx:
            n = min(gcall, nidx - off)
            isl = gidx[:, (base + off) // 16:(base + off + n) // 16]
            nc.gpsimd.dma_gather(
                gath[:, off // 128:(off + n) // 128, :], tab[:], isl,
                n, n, C, queue_num=qn[0])
            qn[0] = (qn[0] + 1) % 4
            off += n
        for wi in range(WGRP):
            w = wg * WGRP + wi
            ohs = oh_p.tile([128, cap, WIN], bdt, tag="ohs")
            nc.sync.dma_start(
                ohs[:].rearrange("p t c -> p (t c)"),
                oh_d[:][:, w * cap * WIN:(w + 1) * cap * WIN])
            ap = ps1.tile([C, WIN], fdt, tag="ps1")
            for t in range(cap):
                nc.tensor.matmul(ap[:], gath[:, wi * cap + t, :], ohs[:, t, :],
                                 start=(t == 0), stop=(t == cap - 1))
            nc.vector.tensor_add(g_bf[:, w * WIN:(w + 1) * WIN], ap[:],
                                 hT[:, w * WIN:(w + 1) * WIN])

    # GIN MLP + combine with attention: acc = h1 + h2
    acc_bf = big_p.tile([C, NPC], bdt, tag="acc_bf")
    r_bf = big_p.tile([C, NPC], bdt, tag="r_bf")
    for ch in range(2):
        sl = slice(ch * 1024, (ch + 1) * 1024)
        tp = ps2.tile([C, 1024], fdt, tag="ps2")
        _mm(nc, tp, W(f"gw1_{l}"), g_bf[:, sl], start=True, stop=True)
        nc.scalar.activation(r_bf[:, sl], tp[:], AF.Relu, bias=B(f"gb1_{l}"))
        up = ps2.tile([C, 1024], fdt, tag="ps2")
        _mm(nc, up, W(f"gw2_{l}"), r_bf[:, sl], start=True, stop=True)
        h1 = chunk_p.tile([C, 1024], fdt, tag="h1")
        nc.scalar.activation(h1[:], up[:], AFI, bias=B(f"sgb2_{l}"), scale=S_BN)
        ap2 = ps2.tile([C, 1024], fdt, tag="ps2")
        _mm(nc, ap2, W(f"ow_{l}"), o_bf[:, sl], start=True, stop=False)
        _mm(nc, ap2, W("I2"), hB[:, sl], start=False, stop=True)
        h2 = chunk_p.tile([C, 1024], fdt, tag="h2")
        nc.scalar.activation(h2[:], ap2[:], AFI, bias=B(f"sob_{l}"), scale=S_BN)
        nc.vector.tensor_add(acc_bf[:, sl], h1[:], h2[:])

    # MLP (per-chunk to keep r2 small)
    for ch in range(2):
        sl = slice(ch * 1024, (ch + 1) * 1024)
        r2_bf = chunk_p.tile([C, 2, 1024], bdt, tag="r2_bf")
        for mh in range(2):
            mp = ps2.tile([C, 1024], fdt, tag="ps2")
            _mm(nc, mp, W(f"m1_{l}", width=256)[:, mh * 128:(mh + 1) * 128],
                acc_bf[:, sl], start=True, stop=True)
            bname = f"m1ba_{l}" if mh == 0 else f"m1bb_{l}"
            nc.scalar.activation(r2_bf[:, mh, :], mp[:], AF.Relu, bias=B(bname))
        m2p = ps2.tile([C, 1024], fdt, tag="ps2")
        _mm(nc, m2p, W(f"m2a_{l}"), r2_bf[:, 0, :], start=True, stop=False)
        _mm(nc, m2p, W(f"m2b_{l}"), r2_bf[:, 1, :], start=False, stop=False)
        _mm(nc, m2p, W("I1"), acc_bf[:, sl], start=False, stop=True)
        dh = chunk_p.tile([C, 1024], fdt, tag="dh")
        nc.scalar.activation(dh[:], m2p[:], AFI, bias=B(f"smb2_{l}"), scale=S_BN)
        nc.vector.tensor_add(hT[:, sl], hT[:, sl], dh[:])
        nc.vector.tensor_copy(hB[:, sl], hT[:, sl])

    if not last:
        tabs[s] = emit_table(hB)


# ---------------------------------------------------------------------------
# Entry point
# ---------------------------------------------------------------------------

_CACHE = {}


def _get_kernel(caps, offs, boffs, wcols, bcols, **kw):
    key = (tuple(caps), wcols, bcols, tuple(sorted(kw.items())))
    if key not in _CACHE:
        _CACHE[key] = build_kernel(caps, offs, boffs, wcols, bcols, **kw)
    return _CACHE[key]


def kernel(**inputs):
    in_maps, caps, offs, boffs, wcols, bcols = _pack_host(inputs)
    nc = _get_kernel(caps, offs, boffs, wcols, bcols)
    res = run_bass_kernel_spmd(nc, in_maps, core_ids=list(range(N_CORES)))
    pools = []
    for si in range(3):
        parts = [np.asarray(res.results[c]["pool_out"][si])
                 for c in range(N_CORES)]
        full = np.concatenate(parts, axis=1)          # [C, 64]
        pools.append(np.ascontiguousarray(full.T).astype(np.float32))
    return tuple(pools)


# revision 15
# speedup vs baseline: 1.3541x; 1.0431x over previous
"""Trainium2 Bass kernel for nn_CGT_21354577396059 (GPS-style GNN, 3 streams x 3 layers).

Strategy (8 NeuronCores, SPMD):
- Node-shard: core c owns nodes [2048c, 2048c+2048) = 8 graphs of 256 nodes.
- Activations feature-major in SBUF: hT [C=128 partitions, 2048 nodes] fp32,
  bf16 copies as matmul inputs.
- GIN segment_sum: edges dst-sorted per core, deduped by src per 128-dst
  window (multi-hot count columns); src rows gathered from a bf16 node-major
  DRAM table (gpsimd dma_gather, 256B rows, one call per window-group);
  scatter via count-matrix matmuls aggT += gathered_chunk.T @ onehot_chunk.
- The bf16 node table is stored P-MAJOR per core block (row p*16+k holds node
  k*128+p) so the SBUF->DRAM table write is contiguous; gather indices are
  host-permuted to match. Table rebuilt each layer via PE transpose + 8-core
  AllGather.
- One-hot count matrices are stored partition-major in DRAM so their loads are
  contiguous (4KB+ per-partition descriptors instead of 256B).
- Attention is graph-local: scoresT = kT.T @ qT per (graph, head, key-chunk)
  masked-K; exp on ACT; softmax sums via col-tiled ones-matmuls; o via
  col-tiled matmuls contracted over keys; normalization via ACT reciprocal.

kernel(**inputs) takes the FULL unsharded inputs and returns
(pool(h0), pool(ha), pool(hb)) -- tuple of [64, 128] float32 -- like the reference.
"""
import sys
import numpy as np
import ml_dtypes

if "/opt/trn_rl_repo" not in sys.path:
    sys.path.insert(0, "/opt/trn_rl_repo")

import concourse.bass as bass  # noqa: F401
import concourse.tile as tile
from concourse import bacc, mybir
from concourse.bass_utils import run_bass_kernel_spmd

BF = ml_dtypes.bfloat16

# Problem constants (self-contained; no reads of /root/problem/*)
N_NODES = 16384
N_GRAPHS = 64
NPG = 256
FEA_DIM = 32
PE_DIM = 20
C = 128
HEADS = 4
HD = C // HEADS
L = 3
BN_EPS = 1e-5
S_BN = float(1.0 / np.sqrt(1.0 + BN_EPS))

N_CORES = 8
NPC = N_NODES // N_CORES   # 2048
NG_C = NPC // NPG          # 8 graphs per core
WIN = 128
NWIN = NPC // WIN          # 16
NCHUNK = NPC // 128        # 16
WGRP = 1                   # windows per dma_gather call

fdt = mybir.dt.float32
bdt = mybir.dt.bfloat16
i16 = mybir.dt.int16
AF = mybir.ActivationFunctionType
AX = mybir.AxisListType
ALU = mybir.AluOpType
AG_GROUPS = [list(range(N_CORES))]


# ---------------------------------------------------------------------------
# Host-side data prep
# ---------------------------------------------------------------------------

def _perm_row(n):
    """Global table row for node n with p-major per-core blocks."""
    cb, m = n // NPC, n % NPC
    return cb * NPC + (m % 128) * NCHUNK + m // 128


def _wrap_idxs(idx):
    """dma_gather idx layout [128, n/16] int16: idx i at (i%16, i//16),
    replicated across the 8 16-partition blocks."""
    n = len(idx)
    a = np.asarray(idx, np.int16).reshape(n // 16, 16).T
    return np.ascontiguousarray(np.tile(a, (8, 1)))


def _prep_edges_stream(edge_index):
    """Dedup by src per (core, 128-dst window); count-matrix columns.

    Returns (cap_chunks, [(gidx_wrapped, oh_pmajor)] per core) where
    oh_pmajor is [128, NWIN*cap*WIN] bf16 (partition-major onehot)."""
    src = np.asarray(edge_index[0]).astype(np.int64)
    dst = np.asarray(edge_index[1]).astype(np.int64)
    per_core_wins = []
    max_w = 0
    for c in range(N_CORES):
        m = (dst >= c * NPC) & (dst < (c + 1) * NPC)
        s, d = src[m], dst[m] - c * NPC
        wins = []
        for w in range(NWIN):
            mw = (d >= w * WIN) & (d < (w + 1) * WIN)
            sw, dw = s[mw], d[mw] - w * WIN
            # dedup srcs within the window; multi-hot count columns
            uniq, inv = np.unique(sw, return_inverse=True)
            cnt = np.zeros((len(uniq), WIN), np.float32)
            np.add.at(cnt, (inv, dw), 1.0)
            # sort rows by permuted table address for DMA page locality
            order = np.argsort(_perm_row(uniq), kind="stable")
            uniq, cnt = uniq[order], cnt[order]
            wins.append((uniq, cnt))
            max_w = max(max_w, len(uniq))
        per_core_wins.append(wins)
    cap_e = ((max_w + 127) // 128) * 128     # rows per window, padded
    cap = cap_e // 128
    out = []
    for c in range(N_CORES):
        srcs = np.zeros(NWIN * cap_e, np.int64)
        oh = np.zeros((NWIN, cap_e, WIN), np.float32)
        for w in range(NWIN):
            uniq, cnt = per_core_wins[c][w]
            n = len(uniq)
            srcs[w * cap_e:w * cap_e + n] = _perm_row(uniq)
            oh[w, :n, :] = cnt
        # partition-major: oh_pm[p, (w, t, c)] = oh[w, t*128+p, c]
        oh_pm = np.ascontiguousarray(
            oh.reshape(NWIN, cap, 128, WIN).transpose(2, 0, 1, 3)
            .reshape(128, NWIN * cap * WIN).astype(BF))
        out.append((_wrap_idxs(srcs), oh_pm))
    return cap, out


def _pack_host(inputs):
    inp = {k: np.asarray(v) for k, v in inputs.items()}
    rt2 = 1.0 / np.sqrt(HD)

    blocks, offs = [], {}

    def add(name, arr):
        arr = np.asarray(arr, np.float32)
        k, m = arr.shape
        buf = np.zeros((128, m), BF)
        buf[:k] = arr.astype(BF)
        offs[name] = sum(b.shape[1] for b in blocks)
        blocks.append(buf)

    add("emb", inp["node_emb_w"])
    add("pe", inp["pe_lin_w"])
    add("I2", 2.0 * np.eye(C))       # h2 fold: ACT scale s gives 2s*h
    add("I1", np.eye(C))             # transpose identity + acc fold
    add("ones32", np.ones((C, HD)))
    for l in range(L):
        aw = inp["attn_in_w"][l]
        add(f"gw1_{l}", inp["gin_w1"][l])
        add(f"gw2_{l}", inp["gin_w2"][l])
        add(f"qT_{l}", (aw[0:C] * rt2).T)
        for h in range(HEADS):
            km = aw[C:2 * C].T.copy()
            mask = np.zeros(C); mask[32 * h:32 * h + 32] = 1.0
            add(f"kTm{h}_{l}", km * mask[None, :])
        add(f"vT_{l}", aw[2 * C:3 * C].T)
        add(f"ow_{l}", inp["attn_out_w"][l].T)
        add(f"m1_{l}", inp["mlp_w1"][l])
        add(f"m2a_{l}", inp["mlp_w2"][l][0:C])
        add(f"m2b_{l}", inp["mlp_w2"][l][C:2 * C])
    wts = np.ascontiguousarray(np.concatenate(blocks, axis=1))

    bvecs, boffs = [], {}

    def addb(name, vec):
        vec = np.asarray(vec, np.float32).reshape(-1)
        assert vec.shape == (C,)
        boffs[name] = len(bvecs)
        bvecs.append(vec)

    addb("eb", inp["node_emb_b"] + inp["pe_lin_b"])
    for l in range(L):
        ab = inp["attn_in_b"][l]
        addb(f"gb1_{l}", inp["gin_b1"][l])
        addb(f"sgb2_{l}", S_BN * inp["gin_b2"][l])
        addb(f"qb_{l}", ab[0:C] * rt2)
        for h in range(HEADS):
            mask = np.zeros(C); mask[32 * h:32 * h + 32] = 1.0
            addb(f"kbm{h}_{l}", ab[C:2 * C] * mask)
        addb(f"sob_{l}", S_BN * inp["attn_out_b"][l])
        addb(f"m1ba_{l}", inp["mlp_b1"][l][0:C])
        addb(f"m1bb_{l}", inp["mlp_b1"][l][C:2 * C])
        addb(f"smb2_{l}", S_BN * inp["mlp_b2"][l])
    biases = np.ascontiguousarray(np.stack(bvecs, axis=1).astype(np.float32))

    vbias = np.ascontiguousarray(np.stack(
        [np.tile(inp["attn_in_b"][l][2 * C:3 * C], (128, 1)) for l in range(L)]
    ).astype(np.float32))

    caps, edges = [], []
    for key in ("edge_index", "edge_index1", "edge_index2"):
        cap, per_core = _prep_edges_stream(inp[key])
        caps.append(cap)
        edges.append(per_core)

    xs = [inp["x"], inp["x1"], inp["x2"]]
    in_maps = []
    for c in range(N_CORES):
        m = {"wts": wts, "biases": biases, "vbias": vbias}
        sl = slice(c * NPC, (c + 1) * NPC)
        for s in range(3):
            m[f"xT{s}"] = np.ascontiguousarray(xs[s][sl].T.astype(BF))
            m[f"gidx{s}"] = edges[s][c][0]
            m[f"onehot{s}"] = edges[s][c][1]
        m["peT"] = np.ascontiguousarray(inp["pe"][sl].T.astype(BF))
        in_maps.append(m)

    return in_maps, caps, offs, boffs, wts.shape[1], biases.shape[1]


# ---------------------------------------------------------------------------
# Kernel builder
# ---------------------------------------------------------------------------


def _mm(nc, out, lhsT, rhs, start, stop, nmax=512):
    """matmul with moving free dim split to <=512 (ISA limit)."""
    n = rhs.shape[-1]
    assert out.shape[-1] == n
    for i in range(0, n, nmax):
        j = min(i + nmax, n)
        nc.tensor.matmul(out[:, i:j], lhsT, rhs[:, i:j], start=start, stop=stop)


def build_kernel(caps, offs, boffs, wcols, bcols, n_layers=L, n_streams=3,
                 gcall=1024, dbg_stream=None):
    nc = bacc.Bacc("TRN2", target_bir_lowering=False, num_devices=N_CORES,
                   num_swdge_queues=4)

    wts_d = nc.dram_tensor("wts", [128, wcols], bdt, kind="ExternalInput")
    bias_d = nc.dram_tensor("biases", [128, bcols], fdt, kind="ExternalInput")
    vbias_d = nc.dram_tensor("vbias", [L, 128, 128], fdt, kind="ExternalInput")
    peT_d = nc.dram_tensor("peT", [PE_DIM, NPC], bdt, kind="ExternalInput")
    xT_d, gidx_d, oh_d = [], [], []
    for s in range(3):
        cap = caps[s]
        xT_d.append(nc.dram_tensor(f"xT{s}", [FEA_DIM, NPC], bdt,
                                   kind="ExternalInput"))
        gidx_d.append(nc.dram_tensor(f"gidx{s}", [128, NWIN * cap * 8], i16,
                                     kind="ExternalInput"))
        oh_d.append(nc.dram_tensor(f"onehot{s}", [128, NWIN * cap * WIN], bdt,
                                   kind="ExternalInput"))
    pool_out = nc.dram_tensor("pool_out", [3, C, NG_C], fdt,
                              kind="ExternalOutput")
    dbg_out = None
    if dbg_stream is not None:
        dbg_out = nc.dram_tensor("dbg_out", [C, NPC], fdt, kind="ExternalOutput")

    with tile.TileContext(nc) as tc:
        with (
            tc.tile_pool(name="const", bufs=1) as const_p,
            tc.tile_pool(name="hstate", bufs=1) as hstate_p,
            tc.tile_pool(name="big", bufs=1) as big_p,       # full-width tiles
            tc.tile_pool(name="chunk", bufs=2) as chunk_p,   # [C,1024]-ish tiles
            tc.tile_pool(name="gath", bufs=4) as gath_p,
            tc.tile_pool(name="ohp", bufs=2) as oh_p,
            tc.tile_pool(name="ps2", bufs=3, space="PSUM") as ps2,   # 2-bank
            tc.tile_pool(name="ps1", bufs=2, space="PSUM") as ps1,   # 1-bank
            tc.tile_pool(name="dram", bufs=4, space="DRAM") as dram_p,
        ):
            wts = const_p.tile([128, wcols], bdt)
            nc.sync.dma_start(wts[:], wts_d[:])
            bia = const_p.tile([128, bcols], fdt)
            nc.sync.dma_start(bia[:], bias_d[:])
            vbia = const_p.tile([128, L, 128], fdt)
            nc.sync.dma_start(vbia[:], vbias_d[:].rearrange("l p c -> p l c"))

            def W(name, width=128, rows=128):
                return wts[0:rows, offs[name]:offs[name] + width]

            def B(name):
                return bia[:, boffs[name]:boffs[name] + 1]

            hT = [hstate_p.tile([C, NPC], fdt, tag=f"hT{s}", name=f"hT{s}")
                  for s in range(3)]
            hB = [hstate_p.tile([C, NPC], bdt, tag=f"hB{s}", name=f"hB{s}")
                  for s in range(3)]
            gidx = []
            for s in range(n_streams):
                t = hstate_p.tile([128, NWIN * caps[s] * 8], i16,
                                  tag=f"gidx{s}", name=f"gidx{s}")
                nc.sync.dma_start(t[:], gidx_d[s][:])
                gidx.append(t)

            def emit_table(src_bf):
                """feature-major bf16 [C, NPC] -> p-major node table ->
                AllGather. Block layout: DRAM row p*NCHUNK+k = node k*128+p."""
                nm = big_p.tile([128, NCHUNK, C], bdt, tag="nm")
                for k in range(NCHUNK):
                    pt = ps1.tile([128, 128], bdt, tag="ps1", name="pt")
                    nc.tensor.transpose(pt[:], src_bf[:, k * 128:(k + 1) * 128],
                                        W("I1"))
                    nc.vector.tensor_copy(nm[:, k, :], pt[:])
                agi = dram_p.tile([128, NCHUNK * C], bdt, tag="agi")
                nc.sync.dma_start(agi[:], nm[:].rearrange("p k c -> p (k c)"))
                tab = dram_p.tile([N_NODES, C], bdt, tag="tab", name="tab")
                nc.gpsimd.collective_compute(
                    "AllGather", ALU.bypass, replica_groups=AG_GROUPS,
                    ins=[agi.opt()], outs=[tab.opt()])
                return tab

            # ---------------- embedding ----------------
            tabs = [None] * 3
            pending = [None]   # deferred emit_table issue (keeps gpsimd queue hot)
            for s in range(n_streams):
                xT = chunk_p.tile([FEA_DIM, NPC], bdt, tag="xT")
                nc.sync.dma_start(xT[:], xT_d[s][:])
                peT = chunk_p.tile([PE_DIM, NPC], bdt, tag="peT")
                nc.sync.dma_start(peT[:], peT_d[:])
                for ch in range(2):
                    sl = slice(ch * 1024, (ch + 1) * 1024)
                    ep = ps2.tile([C, 1024], fdt, tag="ps2")
                    _mm(nc, ep, W("emb", rows=FEA_DIM), xT[:, sl],
                        start=True, stop=False)
                    _mm(nc, ep, W("pe", rows=PE_DIM), peT[:, sl],
                        start=False, stop=True)
                    nc.scalar.activation(hT[s][:, sl], ep[:], AF.Identity,
                                         bias=B("eb"))
                    nc.vector.tensor_copy(hB[s][:, sl], hT[s][:, sl])
                if s < n_streams - 1:
                    tabs[s] = emit_table(hB[s])
                else:
                    def _emit_last(s=s):
                        tabs[s] = emit_table(hB[s])
                    pending[0] = _emit_last

            # ---------------- layers ----------------
            for l in range(n_layers):
                for s in range(n_streams):
                    _gps_layer(nc, l, caps[s], hT[s], hB[s], tabs, s,
                               gidx[s], oh_d[s], W, B, vbia,
                               big_p, chunk_p, gath_p, oh_p, ps2, ps1,
                               emit_table, gcall, pending,
                               last=(l == n_layers - 1))

            # ---------------- pooling ----------------
            for s in range(n_streams):
                po = chunk_p.tile([C, NG_C], fdt, tag="pool")
                nc.vector.reduce_sum(
                    out=po[:],
                    in_=hT[s][:].rearrange("c (g n) -> c g n", g=NG_C),
                    axis=AX.X)
                nc.sync.dma_start(pool_out[s], po[:])

            if dbg_stream is not None:
                nc.sync.dma_start(dbg_out[:], hT[dbg_stream][:])

    nc.compile()
    return nc


def _gps_layer(nc, l, cap, hT, hB, tabs, s, gidx, oh_d, W, B, vbia,
               big_p, chunk_p, gath_p, oh_p, ps2, ps1, emit_table, gcall,
               pending, last):
    AFI = AF.Identity

    # ---------------- attention (graph-local) ----------------
    qT = big_p.tile([C, NPC], bdt, tag="qT", name="qT")
    for ch in range(2):
        sl = slice(ch * 1024, (ch + 1) * 1024)
        qp = ps2.tile([C, 1024], fdt, tag="ps2")
        _mm(nc, qp, W(f"qT_{l}"), hB[:, sl], start=True, stop=True)
        nc.scalar.activation(qT[:, sl], qp[:], AFI, bias=B(f"qb_{l}"))
    v_sb = big_p.tile([128, NCHUNK, C], bdt, tag="v_sb", name="v_sb")
    for k in range(NCHUNK):
        vp = ps1.tile([128, C], fdt, tag="ps1")
        nc.tensor.matmul(vp[:], hB[:, k * 128:(k + 1) * 128], W(f"vT_{l}"),
                         start=True, stop=True)
        nc.vector.tensor_add(v_sb[:, k, :], vp[:], vbia[:, l, :])

    expT = big_p.tile([128, 2, HEADS, NG_C, NPG], bdt, tag="expT", name="expT")
    for h in range(HEADS):
        kTm = big_p.tile([C, NPC], bdt, tag="kTm", name="kTm")
        for ch in range(2):
            sl = slice(ch * 1024, (ch + 1) * 1024)
            kp = ps2.tile([C, 1024], fdt, tag="ps2")
            _mm(nc, kp, W(f"kTm{h}_{l}"), hB[:, sl], start=True, stop=True)
            nc.scalar.activation(kTm[:, sl], kp[:], AFI, bias=B(f"kbm{h}_{l}"))
        for kc in range(2):
            for gh in range(2):
                sp = ps2.tile([128, 4, NPG], fdt, tag="ps2")
                for gi in range(4):
                    g = gh * 4 + gi
                    ksl = kTm[:, g * NPG + kc * 128:g * NPG + kc * 128 + 128]
                    qsl = qT[:, g * NPG:(g + 1) * NPG]
                    nc.tensor.matmul(sp[:, gi, :], ksl, qsl,
                                     start=True, stop=True)
                nc.scalar.activation(expT[:, kc, h, gh * 4:gh * 4 + 4, :],
                                     sp[:], AF.Exp)

    recip = big_p.tile([128, NPC], bdt, tag="recip", name="recip")
    for half in range(2):
        sm = ps2.tile([128, 1024], fdt, tag="ps2")
        for h in range(HEADS):
            for qc in range(2):
                osl = sm[32 * h:32 * h + 32, qc * 512:(qc + 1) * 512]
                for kc in range(2):
                    rhs = expT[:, kc, h, :, :].rearrange("p g q -> p (g q)")
                    nc.tensor.matmul(
                        osl, W("ones32", width=HD),
                        rhs[:, half * 1024 + qc * 512:half * 1024 + (qc + 1) * 512],
                        start=(kc == 0), stop=(kc == 1),
                        tile_position=(0, 32 * h))
        with nc.allow_low_precision(reason="softmax recip bf16 ok at 2e-2 gate"):
            nc.vector.reciprocal(recip[:, half * 1024:(half + 1) * 1024], sm[:])

    o_bf = big_p.tile([C, NPC], bdt, tag="o_bf", name="o_bf")
    for g in range(NG_C):
        op = ps1.tile([128, NPG], fdt, tag="ps1")
        for h in range(HEADS):
            for kc in range(2):
                lhs = v_sb[:, g * 2 + kc, 32 * h:32 * h + 32]
                rhs = expT[:, kc, h, g, :]
                nc.tensor.matmul(op[32 * h:32 * h + 32, :], lhs, rhs,
                                 start=(kc == 0), stop=(kc == 1),
                                 tile_position=(0, 32 * h))
        nc.vector.tensor_mul(o_bf[:, g * NPG:(g + 1) * NPG], op[:],
                             recip[:, g * NPG:(g + 1) * NPG])

    # ---------------- GIN ----------------
    g_bf = big_p.tile([C, NPC], bdt, tag="g_bf")
    tab = tabs[s]
    cap_e = cap * 128                       # gathered rows per window
    qn = [0]
    for wg in range(NWIN // WGRP):
        nidx = WGRP * cap_e
        gath = gath_p.tile([128, WGRP * cap, C], bdt, tag="gath")
        base = wg * nidx
        off = 0
        while off < nidx:
            n = min(gcall, nidx - off)
            isl = gidx[:, (base + off) // 16:(base + off + n) // 16]
            nc.gpsimd.dma_gather(
                gath[:, off // 128:(off + n) // 128, :], tab[:], isl,
                n, n, C, queue_num=qn[0])
            qn[0] = (qn[0] + 1) % 4
            off += n
        for wi in range(WGRP):
            w = wg * WGRP + wi
            ohs = oh_p.tile([128, cap, WIN], bdt, tag="ohs")
            nc.sync.dma_start(
                ohs[:].rearrange("p t c -> p (t c)"),
                oh_d[:][:, w * cap * WIN:(w + 1) * cap * WIN])
            ap = ps1.tile([C, WIN], fdt, tag="ps1")
            for t in range(cap):
                nc.tensor.matmul(ap[:], gath[:, wi * cap + t, :], ohs[:, t, :],
                                 start=(t == 0), stop=(t == cap - 1))
            nc.vector.tensor_add(g_bf[:, w * WIN:(w + 1) * WIN], ap[:],
                                 hT[:, w * WIN:(w + 1) * WIN])

    # issue the previous stream's table AllGather now: all of this stream's
    # gather calls are already queued on gpsimd, so the in-order queue does
    # not stall waiting for the previous stream's MLP/table write.
    if pending[0] is not None:
        pending[0]()
        pending[0] = None

    # GIN MLP + combine with attention: acc = h1 + h2
    acc_bf = big_p.tile([C, NPC], bdt, tag="acc_bf")
    r_bf = big_p.tile([C, NPC], bdt, tag="r_bf")
    for ch in range(2):
        sl = slice(ch * 1024, (ch + 1) * 1024)
        tp = ps2.tile([C, 1024], fdt, tag="ps2")
        _mm(nc, tp, W(f"gw1_{l}"), g_bf[:, sl], start=True, stop=True)
        nc.scalar.activation(r_bf[:, sl], tp[:], AF.Relu, bias=B(f"gb1_{l}"))
        up = ps2.tile([C, 1024], fdt, tag="ps2")
        _mm(nc, up, W(f"gw2_{l}"), r_bf[:, sl], start=True, stop=True)
        h1 = chunk_p.tile([C, 1024], fdt, tag="h1")
        nc.scalar.activation(h1[:], up[:], AFI, bias=B(f"sgb2_{l}"), scale=S_BN)
        ap2 = ps2.tile([C, 1024], fdt, tag="ps2")
        _mm(nc, ap2, W(f"ow_{l}"), o_bf[:, sl], start=True, stop=False)
        _mm(nc, ap2, W("I2"), hB[:, sl], start=False, stop=True)
        h2 = chunk_p.tile([C, 1024], fdt, tag="h2")
        nc.scalar.activation(h2[:], ap2[:], AFI, bias=B(f"sob_{l}"), scale=S_BN)
        nc.vector.tensor_add(acc_bf[:, sl], h1[:], h2[:])

    # MLP (per-chunk to keep r2 small)
    for ch in range(2):
        sl = slice(ch * 1024, (ch + 1) * 1024)
        r2_bf = chunk_p.tile([C, 2, 1024], bdt, tag="r2_bf")
        for mh in range(2):
            mp = ps2.tile([C, 1024], fdt, tag="ps2")
            _mm(nc, mp, W(f"m1_{l}", width=256)[:, mh * 128:(mh + 1) * 128],
                acc_bf[:, sl], start=True, stop=True)
            bname = f"m1ba_{l}" if mh == 0 else f"m1bb_{l}"
            nc.scalar.activation(r2_bf[:, mh, :], mp[:], AF.Relu, bias=B(bname))
        m2p = ps2.tile([C, 1024], fdt, tag="ps2")
        _mm(nc, m2p, W(f"m2a_{l}"), r2_bf[:, 0, :], start=True, stop=False)
        _mm(nc, m2p, W(f"m2b_{l}"), r2_bf[:, 1, :], start=False, stop=False)
        _mm(nc, m2p, W("I1"), acc_bf[:, sl], start=False, stop=True)
        dh = chunk_p.tile([C, 1024], fdt, tag="dh")
        nc.scalar.activation(dh[:], m2p[:], AFI, bias=B(f"smb2_{l}"), scale=S_BN)
        nc.vector.tensor_add(hT[:, sl], hT[:, sl], dh[:])
        nc.vector.tensor_copy(hB[:, sl], hT[:, sl])

    if not last:
        def _emit(s=s, hB=hB):
            tabs[s] = emit_table(hB)
        pending[0] = _emit


# ---------------------------------------------------------------------------
# Entry point
# ---------------------------------------------------------------------------

_CACHE = {}


def _get_kernel(caps, offs, boffs, wcols, bcols, **kw):
    key = (tuple(caps), wcols, bcols, tuple(sorted(kw.items())))
    if key not in _CACHE:
        _CACHE[key] = build_kernel(caps, offs, boffs, wcols, bcols, **kw)
    return _CACHE[key]


def kernel(**inputs):
    in_maps, caps, offs, boffs, wcols, bcols = _pack_host(inputs)
    nc = _get_kernel(caps, offs, boffs, wcols, bcols)
    res = run_bass_kernel_spmd(nc, in_maps, core_ids=list(range(N_CORES)))
    pools = []
    for si in range(3):
        parts = [np.asarray(res.results[c]["pool_out"][si])
                 for c in range(N_CORES)]
        full = np.concatenate(parts, axis=1)          # [C, 64]
        pools.append(np.ascontiguousarray(full.T).astype(np.float32))
    return tuple(pools)


# revision 17
# speedup vs baseline: 1.3967x; 1.0315x over previous
"""Trainium2 Bass kernel for nn_CGT_21354577396059 (GPS-style GNN, 3 streams x 3 layers).

Strategy (8 NeuronCores, SPMD):
- Node-shard: core c owns nodes [2048c, 2048c+2048) = 8 graphs of 256 nodes.
- Activations feature-major in SBUF: hT [C=128 partitions, 2048 nodes] fp32,
  bf16 copies as matmul inputs.
- GIN segment_sum: edges dst-sorted per core, deduped by src per 128-dst
  window (multi-hot count columns); src rows gathered from a bf16 node-major
  DRAM table (gpsimd dma_gather, 256B rows, one call per window-group);
  scatter via count-matrix matmuls aggT += gathered_chunk.T @ onehot_chunk.
- The bf16 node table is stored P-MAJOR per core block (row p*16+k holds node
  k*128+p) so the SBUF->DRAM table write is contiguous; gather indices are
  host-permuted to match. Table rebuilt each layer via PE transpose + 8-core
  AllGather.
- One-hot count matrices are stored partition-major in DRAM so their loads are
  contiguous (4KB+ per-partition descriptors instead of 256B).
- Attention is graph-local: scoresT = kT.T @ qT per (graph, head, key-chunk)
  masked-K; exp on ACT; softmax sums via col-tiled ones-matmuls; o via
  col-tiled matmuls contracted over keys; normalization via ACT reciprocal.

kernel(**inputs) takes the FULL unsharded inputs and returns
(pool(h0), pool(ha), pool(hb)) -- tuple of [64, 128] float32 -- like the reference.
"""
import sys
import numpy as np
import ml_dtypes

if "/opt/trn_rl_repo" not in sys.path:
    sys.path.insert(0, "/opt/trn_rl_repo")

import concourse.bass as bass  # noqa: F401
import concourse.tile as tile
from concourse import bacc, mybir
from concourse.bass_utils import run_bass_kernel_spmd

BF = ml_dtypes.bfloat16

# Problem constants (self-contained; no reads of /root/problem/*)
N_NODES = 16384
N_GRAPHS = 64
NPG = 256
FEA_DIM = 32
PE_DIM = 20
C = 128
HEADS = 4
HD = C // HEADS
L = 3
BN_EPS = 1e-5
S_BN = float(1.0 / np.sqrt(1.0 + BN_EPS))

N_CORES = 8
NPC = N_NODES // N_CORES   # 2048
NG_C = NPC // NPG          # 8 graphs per core
WIN = 128
NWIN = NPC // WIN          # 16
NCHUNK = NPC // 128        # 16
WGRP = 1                   # windows per dma_gather call

fdt = mybir.dt.float32
bdt = mybir.dt.bfloat16
i16 = mybir.dt.int16
AF = mybir.ActivationFunctionType
AX = mybir.AxisListType
ALU = mybir.AluOpType
AG_GROUPS = [list(range(N_CORES))]


# ---------------------------------------------------------------------------
# Host-side data prep
# ---------------------------------------------------------------------------

def _perm_row(n):
    """Global table row for node n with p-major per-core blocks."""
    cb, m = n // NPC, n % NPC
    return cb * NPC + (m % 128) * NCHUNK + m // 128


def _wrap_idxs(idx):
    """dma_gather idx layout [128, n/16] int16: idx i at (i%16, i//16),
    replicated across the 8 16-partition blocks."""
    n = len(idx)
    a = np.asarray(idx, np.int16).reshape(n // 16, 16).T
    return np.ascontiguousarray(np.tile(a, (8, 1)))


def _prep_edges_stream(edge_index):
    """Dedup by src per (core, 128-dst window); count-matrix columns.

    Returns (cap_chunks, [(gidx_wrapped, oh_pmajor)] per core) where
    oh_pmajor is [128, NWIN*cap*WIN] bf16 (partition-major onehot)."""
    src = np.asarray(edge_index[0]).astype(np.int64)
    dst = np.asarray(edge_index[1]).astype(np.int64)
    per_core_wins = []
    max_w = 0
    for c in range(N_CORES):
        m = (dst >= c * NPC) & (dst < (c + 1) * NPC)
        s, d = src[m], dst[m] - c * NPC
        wins = []
        for w in range(NWIN):
            mw = (d >= w * WIN) & (d < (w + 1) * WIN)
            sw, dw = s[mw], d[mw] - w * WIN
            # dedup srcs within the window; multi-hot count columns
            uniq, inv = np.unique(sw, return_inverse=True)
            cnt = np.zeros((len(uniq), WIN), np.float32)
            np.add.at(cnt, (inv, dw), 1.0)
            # sort rows by permuted table address for DMA page locality
            order = np.argsort(_perm_row(uniq), kind="stable")
            uniq, cnt = uniq[order], cnt[order]
            wins.append((uniq, cnt))
            max_w = max(max_w, len(uniq))
        per_core_wins.append(wins)
    cap_e = ((max_w + 127) // 128) * 128     # rows per window, padded
    cap = cap_e // 128
    out = []
    for c in range(N_CORES):
        srcs = np.zeros(NWIN * cap_e, np.int64)
        oh = np.zeros((NWIN, cap_e, WIN), np.float32)
        for w in range(NWIN):
            uniq, cnt = per_core_wins[c][w]
            n = len(uniq)
            srcs[w * cap_e:w * cap_e + n] = _perm_row(uniq)
            oh[w, :n, :] = cnt
        # partition-major: oh_pm[p, (w, t, c)] = oh[w, t*128+p, c]
        oh_pm = np.ascontiguousarray(
            oh.reshape(NWIN, cap, 128, WIN).transpose(2, 0, 1, 3)
            .reshape(128, NWIN * cap * WIN).astype(BF))
        out.append((_wrap_idxs(srcs), oh_pm))
    return cap, out


def _pack_host(inputs):
    inp = {k: np.asarray(v) for k, v in inputs.items()}
    rt2 = 1.0 / np.sqrt(HD)

    blocks, offs = [], {}

    def add(name, arr):
        arr = np.asarray(arr, np.float32)
        k, m = arr.shape
        buf = np.zeros((128, m), BF)
        buf[:k] = arr.astype(BF)
        offs[name] = sum(b.shape[1] for b in blocks)
        blocks.append(buf)

    add("emb", inp["node_emb_w"])
    add("pe", inp["pe_lin_w"])
    add("I2", 2.0 * np.eye(C))       # h2 fold: ACT scale s gives 2s*h
    add("I1", np.eye(C))             # transpose identity + acc fold
    add("ones32", np.ones((C, HD)))
    for l in range(L):
        aw = inp["attn_in_w"][l]
        add(f"gw1_{l}", inp["gin_w1"][l])
        add(f"gw2_{l}", inp["gin_w2"][l])
        add(f"qT_{l}", (aw[0:C] * rt2).T)
        for h in range(HEADS):
            km = aw[C:2 * C].T.copy()
            mask = np.zeros(C); mask[32 * h:32 * h + 32] = 1.0
            add(f"kTm{h}_{l}", km * mask[None, :])
        add(f"vT_{l}", aw[2 * C:3 * C].T)
        add(f"ow_{l}", inp["attn_out_w"][l].T)
        add(f"m1_{l}", inp["mlp_w1"][l])
        add(f"m2a_{l}", inp["mlp_w2"][l][0:C])
        add(f"m2b_{l}", inp["mlp_w2"][l][C:2 * C])
    wts = np.ascontiguousarray(np.concatenate(blocks, axis=1))

    bvecs, boffs = [], {}

    def addb(name, vec):
        vec = np.asarray(vec, np.float32).reshape(-1)
        assert vec.shape == (C,)
        boffs[name] = len(bvecs)
        bvecs.append(vec)

    addb("eb", inp["node_emb_b"] + inp["pe_lin_b"])
    for l in range(L):
        ab = inp["attn_in_b"][l]
        addb(f"gb1_{l}", inp["gin_b1"][l])
        addb(f"sgb2_{l}", S_BN * inp["gin_b2"][l])
        addb(f"qb_{l}", ab[0:C] * rt2)
        for h in range(HEADS):
            mask = np.zeros(C); mask[32 * h:32 * h + 32] = 1.0
            addb(f"kbm{h}_{l}", ab[C:2 * C] * mask)
        addb(f"sob_{l}", S_BN * inp["attn_out_b"][l])
        addb(f"m1ba_{l}", inp["mlp_b1"][l][0:C])
        addb(f"m1bb_{l}", inp["mlp_b1"][l][C:2 * C])
        addb(f"smb2_{l}", S_BN * inp["mlp_b2"][l])
    biases = np.ascontiguousarray(np.stack(bvecs, axis=1).astype(np.float32))

    vbias = np.ascontiguousarray(np.stack(
        [np.tile(inp["attn_in_b"][l][2 * C:3 * C], (128, 1)) for l in range(L)]
    ).astype(np.float32))

    caps, edges = [], []
    for key in ("edge_index", "edge_index1", "edge_index2"):
        cap, per_core = _prep_edges_stream(inp[key])
        caps.append(cap)
        edges.append(per_core)

    xs = [inp["x"], inp["x1"], inp["x2"]]
    in_maps = []
    for c in range(N_CORES):
        m = {"wts": wts, "biases": biases, "vbias": vbias}
        sl = slice(c * NPC, (c + 1) * NPC)
        for s in range(3):
            m[f"xT{s}"] = np.ascontiguousarray(xs[s][sl].T.astype(BF))
            m[f"gidx{s}"] = edges[s][c][0]
            m[f"onehot{s}"] = edges[s][c][1]
        m["peT"] = np.ascontiguousarray(inp["pe"][sl].T.astype(BF))
        in_maps.append(m)

    return in_maps, caps, offs, boffs, wts.shape[1], biases.shape[1]


# ---------------------------------------------------------------------------
# Kernel builder
# ---------------------------------------------------------------------------


def _mm(nc, out, lhsT, rhs, start, stop, nmax=512):
    """matmul with moving free dim split to <=512 (ISA limit)."""
    n = rhs.shape[-1]
    assert out.shape[-1] == n
    for i in range(0, n, nmax):
        j = min(i + nmax, n)
        nc.tensor.matmul(out[:, i:j], lhsT, rhs[:, i:j], start=start, stop=stop)


def build_kernel(caps, offs, boffs, wcols, bcols, n_layers=L, n_streams=3,
                 gcall=1024, dbg_stream=None):
    nc = bacc.Bacc("TRN2", target_bir_lowering=False, num_devices=N_CORES,
                   num_swdge_queues=4)

    wts_d = nc.dram_tensor("wts", [128, wcols], bdt, kind="ExternalInput")
    bias_d = nc.dram_tensor("biases", [128, bcols], fdt, kind="ExternalInput")
    vbias_d = nc.dram_tensor("vbias", [L, 128, 128], fdt, kind="ExternalInput")
    peT_d = nc.dram_tensor("peT", [PE_DIM, NPC], bdt, kind="ExternalInput")
    xT_d, gidx_d, oh_d = [], [], []
    for s in range(3):
        cap = caps[s]
        xT_d.append(nc.dram_tensor(f"xT{s}", [FEA_DIM, NPC], bdt,
                                   kind="ExternalInput"))
        gidx_d.append(nc.dram_tensor(f"gidx{s}", [128, NWIN * cap * 8], i16,
                                     kind="ExternalInput"))
        oh_d.append(nc.dram_tensor(f"onehot{s}", [128, NWIN * cap * WIN], bdt,
                                   kind="ExternalInput"))
    pool_out = nc.dram_tensor("pool_out", [3, C, NG_C], fdt,
                              kind="ExternalOutput")
    dbg_out = None
    if dbg_stream is not None:
        dbg_out = nc.dram_tensor("dbg_out", [C, NPC], fdt, kind="ExternalOutput")

    with tile.TileContext(nc) as tc:
        with (
            tc.tile_pool(name="const", bufs=1) as const_p,
            tc.tile_pool(name="hstate", bufs=1) as hstate_p,
            tc.tile_pool(name="big", bufs=1) as big_p,       # full-width tiles
            tc.tile_pool(name="chunk", bufs=2) as chunk_p,   # [C,1024]-ish tiles
            tc.tile_pool(name="gath", bufs=4) as gath_p,
            tc.tile_pool(name="ohp", bufs=2) as oh_p,
            tc.tile_pool(name="ps2", bufs=3, space="PSUM") as ps2,   # 2-bank
            tc.tile_pool(name="ps1", bufs=2, space="PSUM") as ps1,   # 1-bank
            tc.tile_pool(name="dram", bufs=4, space="DRAM") as dram_p,
        ):
            wts = const_p.tile([128, wcols], bdt)
            nc.sync.dma_start(wts[:], wts_d[:])
            bia = const_p.tile([128, bcols], fdt)
            nc.sync.dma_start(bia[:], bias_d[:])
            vbia = const_p.tile([128, L, 128], fdt)
            nc.sync.dma_start(vbia[:], vbias_d[:].rearrange("l p c -> p l c"))

            def W(name, width=128, rows=128):
                return wts[0:rows, offs[name]:offs[name] + width]

            def B(name):
                return bia[:, boffs[name]:boffs[name] + 1]

            hT = [hstate_p.tile([C, NPC], fdt, tag=f"hT{s}", name=f"hT{s}")
                  for s in range(3)]
            hB = [hstate_p.tile([C, NPC], bdt, tag=f"hB{s}", name=f"hB{s}")
                  for s in range(3)]
            gidx = []
            for s in range(n_streams):
                t = hstate_p.tile([128, NWIN * caps[s] * 8], i16,
                                  tag=f"gidx{s}", name=f"gidx{s}")
                nc.sync.dma_start(t[:], gidx_d[s][:])
                gidx.append(t)

            def emit_table(src_bf):
                """feature-major bf16 [C, NPC] -> p-major node table ->
                AllGather. Block layout: DRAM row p*NCHUNK+k = node k*128+p."""
                nm = big_p.tile([128, NCHUNK, C], bdt, tag="nm")
                for k in range(NCHUNK):
                    pt = ps1.tile([128, 128], bdt, tag="ps1", name="pt")
                    nc.tensor.transpose(pt[:], src_bf[:, k * 128:(k + 1) * 128],
                                        W("I1"))
                    nc.vector.tensor_copy(nm[:, k, :], pt[:])
                agi = dram_p.tile([128, NCHUNK * C], bdt, tag="agi")
                nc.sync.dma_start(agi[:], nm[:].rearrange("p k c -> p (k c)"))
                tab = dram_p.tile([N_NODES, C], bdt, tag="tab", name="tab",
                                  addr_space="Shared")
                nc.gpsimd.collective_compute(
                    "AllGather", ALU.bypass, replica_groups=AG_GROUPS,
                    ins=[agi.opt()], outs=[tab.opt()])
                return tab

            # ---------------- embedding ----------------
            tabs = [None] * 3
            pending = [None]   # deferred emit_table issue (keeps gpsimd queue hot)
            for s in range(n_streams):
                xT = chunk_p.tile([FEA_DIM, NPC], bdt, tag="xT")
                nc.sync.dma_start(xT[:], xT_d[s][:])
                peT = chunk_p.tile([PE_DIM, NPC], bdt, tag="peT")
                nc.sync.dma_start(peT[:], peT_d[:])
                for ch in range(2):
                    sl = slice(ch * 1024, (ch + 1) * 1024)
                    ep = ps2.tile([C, 1024], fdt, tag="ps2")
                    _mm(nc, ep, W("emb", rows=FEA_DIM), xT[:, sl],
                        start=True, stop=False)
                    _mm(nc, ep, W("pe", rows=PE_DIM), peT[:, sl],
                        start=False, stop=True)
                    nc.scalar.activation(hT[s][:, sl], ep[:], AF.Identity,
                                         bias=B("eb"))
                    nc.vector.tensor_copy(hB[s][:, sl], hT[s][:, sl])
                if s < n_streams - 1:
                    tabs[s] = emit_table(hB[s])
                else:
                    def _emit_last(s=s):
                        tabs[s] = emit_table(hB[s])
                    pending[0] = _emit_last

            # ---------------- layers ----------------
            for l in range(n_layers):
                for s in range(n_streams):
                    _gps_layer(nc, l, caps[s], hT[s], hB[s], tabs, s,
                               gidx[s], oh_d[s], W, B, vbia,
                               big_p, chunk_p, gath_p, oh_p, ps2, ps1,
                               emit_table, gcall, pending,
                               last=(l == n_layers - 1))

            # ---------------- pooling ----------------
            for s in range(n_streams):
                po = chunk_p.tile([C, NG_C], fdt, tag="pool")
                nc.vector.reduce_sum(
                    out=po[:],
                    in_=hT[s][:].rearrange("c (g n) -> c g n", g=NG_C),
                    axis=AX.X)
                nc.sync.dma_start(pool_out[s], po[:])

            if dbg_stream is not None:
                nc.sync.dma_start(dbg_out[:], hT[dbg_stream][:])

    nc.compile()
    return nc


def _gps_layer(nc, l, cap, hT, hB, tabs, s, gidx, oh_d, W, B, vbia,
               big_p, chunk_p, gath_p, oh_p, ps2, ps1, emit_table, gcall,
               pending, last):
    AFI = AF.Identity

    # ---------------- attention (graph-local) ----------------
    qT = big_p.tile([C, NPC], bdt, tag="qT", name="qT")
    for ch in range(2):
        sl = slice(ch * 1024, (ch + 1) * 1024)
        qp = ps2.tile([C, 1024], fdt, tag="ps2")
        _mm(nc, qp, W(f"qT_{l}"), hB[:, sl], start=True, stop=True)
        nc.scalar.activation(qT[:, sl], qp[:], AFI, bias=B(f"qb_{l}"))
    v_sb = big_p.tile([128, NCHUNK, C], bdt, tag="v_sb", name="v_sb")
    for k in range(NCHUNK):
        vp = ps1.tile([128, C], fdt, tag="ps1")
        nc.tensor.matmul(vp[:], hB[:, k * 128:(k + 1) * 128], W(f"vT_{l}"),
                         start=True, stop=True)
        nc.vector.tensor_add(v_sb[:, k, :], vp[:], vbia[:, l, :])

    expT = big_p.tile([128, 2, HEADS, NG_C, NPG], bdt, tag="expT", name="expT")
    for h in range(HEADS):
        kTm = big_p.tile([C, NPC], bdt, tag="kTm", name="kTm")
        for ch in range(2):
            sl = slice(ch * 1024, (ch + 1) * 1024)
            kp = ps2.tile([C, 1024], fdt, tag="ps2")
            _mm(nc, kp, W(f"kTm{h}_{l}"), hB[:, sl], start=True, stop=True)
            nc.scalar.activation(kTm[:, sl], kp[:], AFI, bias=B(f"kbm{h}_{l}"))
        for kc in range(2):
            for gh in range(2):
                sp = ps2.tile([128, 4, NPG], fdt, tag="ps2")
                for gi in range(4):
                    g = gh * 4 + gi
                    ksl = kTm[:, g * NPG + kc * 128:g * NPG + kc * 128 + 128]
                    qsl = qT[:, g * NPG:(g + 1) * NPG]
                    nc.tensor.matmul(sp[:, gi, :], ksl, qsl,
                                     start=True, stop=True)
                nc.scalar.activation(expT[:, kc, h, gh * 4:gh * 4 + 4, :],
                                     sp[:], AF.Exp)

    recip = big_p.tile([128, NPC], bdt, tag="recip", name="recip")
    for half in range(2):
        sm = ps2.tile([128, 1024], fdt, tag="ps2")
        for h in range(HEADS):
            for qc in range(2):
                osl = sm[32 * h:32 * h + 32, qc * 512:(qc + 1) * 512]
                for kc in range(2):
                    rhs = expT[:, kc, h, :, :].rearrange("p g q -> p (g q)")
                    nc.tensor.matmul(
                        osl, W("ones32", width=HD),
                        rhs[:, half * 1024 + qc * 512:half * 1024 + (qc + 1) * 512],
                        start=(kc == 0), stop=(kc == 1),
                        tile_position=(0, 32 * h))
        with nc.allow_low_precision(reason="softmax recip bf16 ok at 2e-2 gate"):
            nc.vector.reciprocal(recip[:, half * 1024:(half + 1) * 1024], sm[:])

    o_bf = big_p.tile([C, NPC], bdt, tag="o_bf", name="o_bf")
    for g in range(NG_C):
        op = ps1.tile([128, NPG], fdt, tag="ps1")
        for h in range(HEADS):
            for kc in range(2):
                lhs = v_sb[:, g * 2 + kc, 32 * h:32 * h + 32]
                rhs = expT[:, kc, h, g, :]
                nc.tensor.matmul(op[32 * h:32 * h + 32, :], lhs, rhs,
                                 start=(kc == 0), stop=(kc == 1),
                                 tile_position=(0, 32 * h))
        nc.vector.tensor_mul(o_bf[:, g * NPG:(g + 1) * NPG], op[:],
                             recip[:, g * NPG:(g + 1) * NPG])

    # ---------------- GIN ----------------
    g_bf = big_p.tile([C, NPC], bdt, tag="g_bf")
    tab = tabs[s]
    cap_e = cap * 128                       # gathered rows per window
    qn = [0]
    for wg in range(NWIN // WGRP):
        nidx = WGRP * cap_e
        gath = gath_p.tile([128, WGRP * cap, C], bdt, tag="gath")
        base = wg * nidx
        off = 0
        while off < nidx:
            n = min(gcall, nidx - off)
            isl = gidx[:, (base + off) // 16:(base + off + n) // 16]
            nc.gpsimd.dma_gather(
                gath[:, off // 128:(off + n) // 128, :], tab[:], isl,
                n, n, C, queue_num=qn[0])
            qn[0] = (qn[0] + 1) % 4
            off += n
        for wi in range(WGRP):
            w = wg * WGRP + wi
            ohs = oh_p.tile([128, cap, WIN], bdt, tag="ohs")
            nc.sync.dma_start(
                ohs[:].rearrange("p t c -> p (t c)"),
                oh_d[:][:, w * cap * WIN:(w + 1) * cap * WIN])
            ap = ps1.tile([C, WIN], fdt, tag="ps1")
            for t in range(cap):
                nc.tensor.matmul(ap[:], gath[:, wi * cap + t, :], ohs[:, t, :],
                                 start=(t == 0), stop=(t == cap - 1))
            nc.vector.tensor_add(g_bf[:, w * WIN:(w + 1) * WIN], ap[:],
                                 hT[:, w * WIN:(w + 1) * WIN])

        # Issue the previous stream's table AllGather late in this stream's
        # gather issue (not after it): the CC blocks the in-order gpsimd
        # queue for its ~35us duration, and mid-stream the SWDGE rings still
        # hold a backlog of queued gather calls whose transfers keep the DMA
        # engines busy underneath the block.
        if wg == NWIN // WGRP - 5 and pending[0] is not None:
            pending[0]()
            pending[0] = None

    if pending[0] is not None:
        pending[0]()
        pending[0] = None

    # GIN MLP + combine with attention: acc = h1 + h2
    acc_bf = big_p.tile([C, NPC], bdt, tag="acc_bf")
    r_bf = big_p.tile([C, NPC], bdt, tag="r_bf")
    for ch in range(2):
        sl = slice(ch * 1024, (ch + 1) * 1024)
        tp = ps2.tile([C, 1024], fdt, tag="ps2")
        _mm(nc, tp, W(f"gw1_{l}"), g_bf[:, sl], start=True, stop=True)
        nc.scalar.activation(r_bf[:, sl], tp[:], AF.Relu, bias=B(f"gb1_{l}"))
        up = ps2.tile([C, 1024], fdt, tag="ps2")
        _mm(nc, up, W(f"gw2_{l}"), r_bf[:, sl], start=True, stop=True)
        h1 = chunk_p.tile([C, 1024], fdt, tag="h1")
        nc.scalar.activation(h1[:], up[:], AFI, bias=B(f"sgb2_{l}"), scale=S_BN)
        ap2 = ps2.tile([C, 1024], fdt, tag="ps2")
        _mm(nc, ap2, W(f"ow_{l}"), o_bf[:, sl], start=True, stop=False)
        _mm(nc, ap2, W("I2"), hB[:, sl], start=False, stop=True)
        h2 = chunk_p.tile([C, 1024], fdt, tag="h2")
        nc.scalar.activation(h2[:], ap2[:], AFI, bias=B(f"sob_{l}"), scale=S_BN)
        nc.vector.tensor_add(acc_bf[:, sl], h1[:], h2[:])

    # MLP (per-chunk to keep r2 small)
    for ch in range(2):
        sl = slice(ch * 1024, (ch + 1) * 1024)
        r2_bf = chunk_p.tile([C, 2, 1024], bdt, tag="r2_bf")
        for mh in range(2):
            mp = ps2.tile([C, 1024], fdt, tag="ps2")
            _mm(nc, mp, W(f"m1_{l}", width=256)[:, mh * 128:(mh + 1) * 128],
                acc_bf[:, sl], start=True, stop=True)
            bname = f"m1ba_{l}" if mh == 0 else f"m1bb_{l}"
            nc.scalar.activation(r2_bf[:, mh, :], mp[:], AF.Relu, bias=B(bname))
        m2p = ps2.tile([C, 1024], fdt, tag="ps2")
        _mm(nc, m2p, W(f"m2a_{l}"), r2_bf[:, 0, :], start=True, stop=False)
        _mm(nc, m2p, W(f"m2b_{l}"), r2_bf[:, 1, :], start=False, stop=False)
        _mm(nc, m2p, W("I1"), acc_bf[:, sl], start=False, stop=True)
        dh = chunk_p.tile([C, 1024], fdt, tag="dh")
        nc.scalar.activation(dh[:], m2p[:], AFI, bias=B(f"smb2_{l}"), scale=S_BN)
        nc.vector.tensor_add(hT[:, sl], hT[:, sl], dh[:])
        nc.vector.tensor_copy(hB[:, sl], hT[:, sl])

    if not last:
        def _emit(s=s, hB=hB):
            tabs[s] = emit_table(hB)
        pending[0] = _emit


# ---------------------------------------------------------------------------
# Entry point
# ---------------------------------------------------------------------------

_CACHE = {}


def _get_kernel(caps, offs, boffs, wcols, bcols, **kw):
    key = (tuple(caps), wcols, bcols, tuple(sorted(kw.items())))
    if key not in _CACHE:
        _CACHE[key] = build_kernel(caps, offs, boffs, wcols, bcols, **kw)
    return _CACHE[key]


def kernel(**inputs):
    in_maps, caps, offs, boffs, wcols, bcols = _pack_host(inputs)
    nc = _get_kernel(caps, offs, boffs, wcols, bcols)
    res = run_bass_kernel_spmd(nc, in_maps, core_ids=list(range(N_CORES)))
    pools = []
    for si in range(3):
        parts = [np.asarray(res.results[c]["pool_out"][si])
                 for c in range(N_CORES)]
        full = np.concatenate(parts, axis=1)          # [C, 64]
        pools.append(np.ascontiguousarray(full.T).astype(np.float32))
    return tuple(pools)


# revision 21
# speedup vs baseline: 1.4641x; 1.0482x over previous
"""Trainium2 Bass kernel for nn_CGT_21354577396059 (GPS-style GNN, 3 streams x 3 layers).

Strategy (8 NeuronCores, SPMD):
- Node-shard: core c owns nodes [2048c, 2048c+2048) = 8 graphs of 256 nodes.
- Activations feature-major in SBUF: hT [C=128 partitions, 2048 nodes] fp32,
  bf16 copies as matmul inputs.
- GIN segment_sum: edges dst-sorted per core, deduped by src per 128-dst
  window (multi-hot count columns); src rows gathered from a bf16 node-major
  DRAM table (gpsimd dma_gather, 256B rows, one call per window-group);
  scatter via count-matrix matmuls aggT += gathered_chunk.T @ onehot_chunk.
- The bf16 node table is stored P-MAJOR per core block (row p*16+k holds node
  k*128+p) so the SBUF->DRAM table write is contiguous; gather indices are
  host-permuted to match. Table rebuilt each layer via PE transpose + 8-core
  AllGather.
- One-hot count matrices are stored partition-major in DRAM so their loads are
  contiguous (4KB+ per-partition descriptors instead of 256B).
- Attention is graph-local: scoresT = kT.T @ qT per (graph, head, key-chunk)
  masked-K; exp on ACT; softmax sums via col-tiled ones-matmuls; o via
  col-tiled matmuls contracted over keys; normalization via ACT reciprocal.

kernel(**inputs) takes the FULL unsharded inputs and returns
(pool(h0), pool(ha), pool(hb)) -- tuple of [64, 128] float32 -- like the reference.
"""
import sys
import numpy as np
import ml_dtypes

if "/opt/trn_rl_repo" not in sys.path:
    sys.path.insert(0, "/opt/trn_rl_repo")

import concourse.bass as bass  # noqa: F401
import concourse.tile as tile
from concourse import bacc, mybir
from concourse.bass_utils import run_bass_kernel_spmd

BF = ml_dtypes.bfloat16

# Problem constants (self-contained; no reads of /root/problem/*)
N_NODES = 16384
N_GRAPHS = 64
NPG = 256
FEA_DIM = 32
PE_DIM = 20
C = 128
HEADS = 4
HD = C // HEADS
L = 3
BN_EPS = 1e-5
S_BN = float(1.0 / np.sqrt(1.0 + BN_EPS))

N_CORES = 8
NPC = N_NODES // N_CORES   # 2048
NG_C = NPC // NPG          # 8 graphs per core
WIN = 128
NWIN = NPC // WIN          # 16
NCHUNK = NPC // 128        # 16
WGRP = 1                   # windows per dma_gather call

fdt = mybir.dt.float32
bdt = mybir.dt.bfloat16
i16 = mybir.dt.int16
AF = mybir.ActivationFunctionType
AX = mybir.AxisListType
ALU = mybir.AluOpType
AG_GROUPS = [list(range(N_CORES))]


# ---------------------------------------------------------------------------
# Host-side data prep
# ---------------------------------------------------------------------------

def _perm_row(n):
    """Global table row for node n with p-major per-core blocks."""
    cb, m = n // NPC, n % NPC
    return cb * NPC + (m % 128) * NCHUNK + m // 128


def _wrap_idxs(idx):
    """dma_gather idx layout [128, n/16] int16: idx i at (i%16, i//16),
    replicated across the 8 16-partition blocks."""
    n = len(idx)
    a = np.asarray(idx, np.int16).reshape(n // 16, 16).T
    return np.ascontiguousarray(np.tile(a, (8, 1)))


def _prep_edges_stream(edge_index):
    """Dedup by src per (core, 128-dst window); count-matrix columns.

    Returns (cap_chunks, [(gidx_wrapped, oh_pmajor)] per core) where
    oh_pmajor is [128, NWIN*cap*WIN] bf16 (partition-major onehot)."""
    src = np.asarray(edge_index[0]).astype(np.int64)
    dst = np.asarray(edge_index[1]).astype(np.int64)
    per_core_wins = []
    max_w = 0
    for c in range(N_CORES):
        m = (dst >= c * NPC) & (dst < (c + 1) * NPC)
        s, d = src[m], dst[m] - c * NPC
        wins = []
        for w in range(NWIN):
            mw = (d >= w * WIN) & (d < (w + 1) * WIN)
            sw, dw = s[mw], d[mw] - w * WIN
            # dedup srcs within the window; multi-hot count columns
            uniq, inv = np.unique(sw, return_inverse=True)
            cnt = np.zeros((len(uniq), WIN), np.float32)
            np.add.at(cnt, (inv, dw), 1.0)
            # sort rows by permuted table address for DMA page locality
            order = np.argsort(_perm_row(uniq), kind="stable")
            uniq, cnt = uniq[order], cnt[order]
            wins.append((uniq, cnt))
            max_w = max(max_w, len(uniq))
        per_core_wins.append(wins)
    cap_e = ((max_w + 127) // 128) * 128     # rows per window, padded
    cap = cap_e // 128
    out = []
    for c in range(N_CORES):
        srcs = np.zeros(NWIN * cap_e, np.int64)
        oh = np.zeros((NWIN, cap_e, WIN), np.float32)
        for w in range(NWIN):
            uniq, cnt = per_core_wins[c][w]
            n = len(uniq)
            srcs[w * cap_e:w * cap_e + n] = _perm_row(uniq)
            oh[w, :n, :] = cnt
        # partition-major: oh_pm[p, (w, t, c)] = oh[w, t*128+p, c]
        # fp8e4: counts 1..3 are exact; halves the DMA bytes
        oh_pm = np.ascontiguousarray(
            oh.reshape(NWIN, cap, 128, WIN).transpose(2, 0, 1, 3)
            .reshape(128, NWIN * cap * WIN).astype(ml_dtypes.float8_e4m3fn))
        out.append((_wrap_idxs(srcs), oh_pm))
    return cap, out


def _pack_host(inputs):
    inp = {k: np.asarray(v) for k, v in inputs.items()}
    rt2 = 1.0 / np.sqrt(HD)

    blocks, offs = [], {}

    def add(name, arr):
        arr = np.asarray(arr, np.float32)
        k, m = arr.shape
        buf = np.zeros((128, m), BF)
        buf[:k] = arr.astype(BF)
        offs[name] = sum(b.shape[1] for b in blocks)
        blocks.append(buf)

    add("emb", inp["node_emb_w"])
    add("pe", inp["pe_lin_w"])
    add("I2", 2.0 * np.eye(C))       # h2 fold: ACT scale s gives 2s*h
    add("I1", np.eye(C))             # transpose identity + acc fold
    add("ones32", np.ones((C, HD)))
    for l in range(L):
        aw = inp["attn_in_w"][l]
        add(f"gw1_{l}", inp["gin_w1"][l])
        add(f"gw2_{l}", inp["gin_w2"][l])
        add(f"qT_{l}", (aw[0:C] * rt2).T)
        for h in range(HEADS):
            km = aw[C:2 * C].T.copy()
            mask = np.zeros(C); mask[32 * h:32 * h + 32] = 1.0
            add(f"kTm{h}_{l}", km * mask[None, :])
        add(f"vT_{l}", aw[2 * C:3 * C].T)
        add(f"ow_{l}", inp["attn_out_w"][l].T)
        add(f"m1_{l}", inp["mlp_w1"][l])
        add(f"m2a_{l}", inp["mlp_w2"][l][0:C])
        add(f"m2b_{l}", inp["mlp_w2"][l][C:2 * C])
    wts = np.ascontiguousarray(np.concatenate(blocks, axis=1))

    bvecs, boffs = [], {}

    def addb(name, vec):
        vec = np.asarray(vec, np.float32).reshape(-1)
        assert vec.shape == (C,)
        boffs[name] = len(bvecs)
        bvecs.append(vec)

    addb("eb", inp["node_emb_b"] + inp["pe_lin_b"])
    for l in range(L):
        ab = inp["attn_in_b"][l]
        addb(f"gb1_{l}", inp["gin_b1"][l])
        addb(f"sgb2_{l}", S_BN * inp["gin_b2"][l])
        addb(f"qb_{l}", ab[0:C] * rt2)
        for h in range(HEADS):
            mask = np.zeros(C); mask[32 * h:32 * h + 32] = 1.0
            addb(f"kbm{h}_{l}", ab[C:2 * C] * mask)
        addb(f"sob_{l}", S_BN * inp["attn_out_b"][l])
        addb(f"m1ba_{l}", inp["mlp_b1"][l][0:C])
        addb(f"m1bb_{l}", inp["mlp_b1"][l][C:2 * C])
        addb(f"smb2_{l}", S_BN * inp["mlp_b2"][l])
    biases = np.ascontiguousarray(np.stack(bvecs, axis=1).astype(np.float32))

    vbias = np.ascontiguousarray(np.stack(
        [np.tile(inp["attn_in_b"][l][2 * C:3 * C], (128, 1)) for l in range(L)]
    ).astype(np.float32))

    caps, edges = [], []
    for key in ("edge_index", "edge_index1", "edge_index2"):
        cap, per_core = _prep_edges_stream(inp[key])
        caps.append(cap)
        edges.append(per_core)

    xs = [inp["x"], inp["x1"], inp["x2"]]
    in_maps = []
    for c in range(N_CORES):
        m = {"wts": wts, "biases": biases, "vbias": vbias}
        sl = slice(c * NPC, (c + 1) * NPC)
        for s in range(3):
            m[f"xT{s}"] = np.ascontiguousarray(xs[s][sl].T.astype(BF))
            m[f"gidx{s}"] = edges[s][c][0]
            m[f"onehot{s}"] = edges[s][c][1]
        m["peT"] = np.ascontiguousarray(inp["pe"][sl].T.astype(BF))
        in_maps.append(m)

    return in_maps, caps, offs, boffs, wts.shape[1], biases.shape[1]


# ---------------------------------------------------------------------------
# Kernel builder
# ---------------------------------------------------------------------------


def _mm(nc, out, lhsT, rhs, start, stop, nmax=512):
    """matmul with moving free dim split to <=512 (ISA limit)."""
    n = rhs.shape[-1]
    assert out.shape[-1] == n
    for i in range(0, n, nmax):
        j = min(i + nmax, n)
        nc.tensor.matmul(out[:, i:j], lhsT, rhs[:, i:j], start=start, stop=stop)


def build_kernel(caps, offs, boffs, wcols, bcols, n_layers=L, n_streams=3,
                 gcall=1024, dbg_stream=None):
    nc = bacc.Bacc("TRN2", target_bir_lowering=False, num_devices=N_CORES,
                   num_swdge_queues=4)

    wts_d = nc.dram_tensor("wts", [128, wcols], bdt, kind="ExternalInput")
    bias_d = nc.dram_tensor("biases", [128, bcols], fdt, kind="ExternalInput")
    vbias_d = nc.dram_tensor("vbias", [L, 128, 128], fdt, kind="ExternalInput")
    peT_d = nc.dram_tensor("peT", [PE_DIM, NPC], bdt, kind="ExternalInput")
    xT_d, gidx_d, oh_d = [], [], []
    for s in range(3):
        cap = caps[s]
        xT_d.append(nc.dram_tensor(f"xT{s}", [FEA_DIM, NPC], bdt,
                                   kind="ExternalInput"))
        gidx_d.append(nc.dram_tensor(f"gidx{s}", [128, NWIN * cap * 8], i16,
                                     kind="ExternalInput"))
        oh_d.append(nc.dram_tensor(f"onehot{s}", [128, NWIN * cap * WIN],
                                   mybir.dt.float8e4, kind="ExternalInput"))
    pool_out = nc.dram_tensor("pool_out", [3, C, NG_C], fdt,
                              kind="ExternalOutput")
    dbg_out = None
    if dbg_stream is not None:
        dbg_out = nc.dram_tensor("dbg_out", [C, NPC], fdt, kind="ExternalOutput")

    with tile.TileContext(nc) as tc:
        with (
            tc.tile_pool(name="const", bufs=1) as const_p,
            tc.tile_pool(name="hstate", bufs=1) as hstate_p,
            tc.tile_pool(name="big", bufs=1) as big_p,       # full-width tiles
            tc.tile_pool(name="chunk", bufs=2) as chunk_p,   # [C,1024]-ish tiles
            tc.tile_pool(name="gath", bufs=4) as gath_p,
            tc.tile_pool(name="ohp", bufs=2) as oh_p,
            tc.tile_pool(name="ps2", bufs=3, space="PSUM") as ps2,   # 2-bank
            tc.tile_pool(name="ps1", bufs=2, space="PSUM") as ps1,   # 1-bank
            tc.tile_pool(name="dram", bufs=4, space="DRAM") as dram_p,
        ):
            wts = const_p.tile([128, wcols], bdt)
            nc.sync.dma_start(wts[:], wts_d[:])
            bia = const_p.tile([128, bcols], fdt)
            nc.sync.dma_start(bia[:], bias_d[:])
            vbia = const_p.tile([128, L, 128], fdt)
            nc.sync.dma_start(vbia[:], vbias_d[:].rearrange("l p c -> p l c"))

            def W(name, width=128, rows=128):
                return wts[0:rows, offs[name]:offs[name] + width]

            def B(name):
                return bia[:, boffs[name]:boffs[name] + 1]

            hT = [hstate_p.tile([C, NPC], fdt, tag=f"hT{s}", name=f"hT{s}")
                  for s in range(3)]
            hB = [hstate_p.tile([C, NPC], bdt, tag=f"hB{s}", name=f"hB{s}")
                  for s in range(3)]
            gidx = []
            for s in range(n_streams):
                t = hstate_p.tile([128, NWIN * caps[s] * 8], i16,
                                  tag=f"gidx{s}", name=f"gidx{s}")
                nc.sync.dma_start(t[:], gidx_d[s][:])
                gidx.append(t)

            def emit_table(src_bf):
                """feature-major bf16 [C, NPC] -> p-major node table ->
                AllGather. Block layout: DRAM row p*NCHUNK+k = node k*128+p."""
                nm = big_p.tile([128, NCHUNK, C], bdt, tag="nm")
                for k in range(NCHUNK):
                    pt = ps1.tile([128, 128], bdt, tag="ps1", name="pt")
                    nc.tensor.transpose(pt[:], src_bf[:, k * 128:(k + 1) * 128],
                                        W("I1"))
                    nc.vector.tensor_copy(nm[:, k, :], pt[:])
                agi = dram_p.tile([128, NCHUNK * C], bdt, tag="agi")
                nc.sync.dma_start(agi[:], nm[:].rearrange("p k c -> p (k c)"))
                tab = dram_p.tile([N_NODES, C], bdt, tag="tab", name="tab",
                                  addr_space="Shared")
                nc.gpsimd.collective_compute(
                    "AllGather", ALU.bypass, replica_groups=AG_GROUPS,
                    ins=[agi.opt()], outs=[tab.opt()])
                return tab

            # ---------------- embedding ----------------
            tabs = [None] * 3
            pending = [None]   # deferred emit_table issue (keeps gpsimd queue hot)
            for s in range(n_streams):
                xT = chunk_p.tile([FEA_DIM, NPC], bdt, tag="xT")
                nc.sync.dma_start(xT[:], xT_d[s][:])
                peT = chunk_p.tile([PE_DIM, NPC], bdt, tag="peT")
                nc.sync.dma_start(peT[:], peT_d[:])
                for ch in range(2):
                    sl = slice(ch * 1024, (ch + 1) * 1024)
                    ep = ps2.tile([C, 1024], fdt, tag="ps2")
                    _mm(nc, ep, W("emb", rows=FEA_DIM), xT[:, sl],
                        start=True, stop=False)
                    _mm(nc, ep, W("pe", rows=PE_DIM), peT[:, sl],
                        start=False, stop=True)
                    nc.scalar.activation(hT[s][:, sl], ep[:], AF.Identity,
                                         bias=B("eb"))
                    nc.vector.tensor_copy(hB[s][:, sl], hT[s][:, sl])
                if s < n_streams - 1:
                    tabs[s] = emit_table(hB[s])
                else:
                    def _emit_last(s=s):
                        tabs[s] = emit_table(hB[s])
                    pending[0] = _emit_last

            # ---------------- layers ----------------
            for l in range(n_layers):
                for s in range(n_streams):
                    _gps_layer(nc, l, caps[s], hT[s], hB[s], tabs, s,
                               gidx[s], oh_d[s], W, B, vbia,
                               big_p, chunk_p, gath_p, oh_p, ps2, ps1,
                               emit_table, gcall, pending,
                               last=(l == n_layers - 1))

            # ---------------- pooling ----------------
            for s in range(n_streams):
                po = chunk_p.tile([C, NG_C], fdt, tag="pool")
                nc.vector.reduce_sum(
                    out=po[:],
                    in_=hT[s][:].rearrange("c (g n) -> c g n", g=NG_C),
                    axis=AX.X)
                nc.sync.dma_start(pool_out[s], po[:])

            if dbg_stream is not None:
                nc.sync.dma_start(dbg_out[:], hT[dbg_stream][:])

    nc.compile()
    return nc


def _gps_layer(nc, l, cap, hT, hB, tabs, s, gidx, oh_d, W, B, vbia,
               big_p, chunk_p, gath_p, oh_p, ps2, ps1, emit_table, gcall,
               pending, last):
    AFI = AF.Identity

    # ---------------- attention (graph-local) ----------------
    qT = big_p.tile([C, NPC], bdt, tag="qT", name="qT")
    for ch in range(2):
        sl = slice(ch * 1024, (ch + 1) * 1024)
        qp = ps2.tile([C, 1024], fdt, tag="ps2")
        _mm(nc, qp, W(f"qT_{l}"), hB[:, sl], start=True, stop=True)
        nc.scalar.activation(qT[:, sl], qp[:], AFI, bias=B(f"qb_{l}"))
    v_sb = big_p.tile([128, NCHUNK, C], bdt, tag="v_sb", name="v_sb")
    for k in range(NCHUNK):
        vp = ps1.tile([128, C], fdt, tag="ps1")
        nc.tensor.matmul(vp[:], hB[:, k * 128:(k + 1) * 128], W(f"vT_{l}"),
                         start=True, stop=True)
        nc.vector.tensor_add(v_sb[:, k, :], vp[:], vbia[:, l, :])

    expT = big_p.tile([128, 2, HEADS, NG_C, NPG], bdt, tag="expT", name="expT")
    for h in range(HEADS):
        kTm = big_p.tile([C, NPC], bdt, tag="kTm", name="kTm")
        for ch in range(2):
            sl = slice(ch * 1024, (ch + 1) * 1024)
            kp = ps2.tile([C, 1024], fdt, tag="ps2")
            _mm(nc, kp, W(f"kTm{h}_{l}"), hB[:, sl], start=True, stop=True)
            nc.scalar.activation(kTm[:, sl], kp[:], AFI, bias=B(f"kbm{h}_{l}"))
        for kc in range(2):
            for gh in range(2):
                sp = ps2.tile([128, 4, NPG], fdt, tag="ps2")
                for gi in range(4):
                    g = gh * 4 + gi
                    ksl = kTm[:, g * NPG + kc * 128:g * NPG + kc * 128 + 128]
                    qsl = qT[:, g * NPG:(g + 1) * NPG]
                    nc.tensor.matmul(sp[:, gi, :], ksl, qsl,
                                     start=True, stop=True)
                nc.scalar.activation(expT[:, kc, h, gh * 4:gh * 4 + 4, :],
                                     sp[:], AF.Exp)

    recip = big_p.tile([128, NPC], bdt, tag="recip", name="recip")
    for half in range(2):
        sm = ps2.tile([128, 1024], fdt, tag="ps2")
        for h in range(HEADS):
            for qc in range(2):
                osl = sm[32 * h:32 * h + 32, qc * 512:(qc + 1) * 512]
                for kc in range(2):
                    rhs = expT[:, kc, h, :, :].rearrange("p g q -> p (g q)")
                    nc.tensor.matmul(
                        osl, W("ones32", width=HD),
                        rhs[:, half * 1024 + qc * 512:half * 1024 + (qc + 1) * 512],
                        start=(kc == 0), stop=(kc == 1),
                        tile_position=(0, 32 * h))
        with nc.allow_low_precision(reason="softmax recip bf16 ok at 2e-2 gate"):
            nc.vector.reciprocal(recip[:, half * 1024:(half + 1) * 1024], sm[:])

    o_bf = big_p.tile([C, NPC], bdt, tag="o_bf", name="o_bf")
    for g in range(NG_C):
        op = ps1.tile([128, NPG], fdt, tag="ps1")
        for h in range(HEADS):
            for kc in range(2):
                lhs = v_sb[:, g * 2 + kc, 32 * h:32 * h + 32]
                rhs = expT[:, kc, h, g, :]
                nc.tensor.matmul(op[32 * h:32 * h + 32, :], lhs, rhs,
                                 start=(kc == 0), stop=(kc == 1),
                                 tile_position=(0, 32 * h))
        nc.vector.tensor_mul(o_bf[:, g * NPG:(g + 1) * NPG], op[:],
                             recip[:, g * NPG:(g + 1) * NPG])

    # ---------------- GIN ----------------
    g_bf = big_p.tile([C, NPC], bdt, tag="g_bf")
    tab = tabs[s]
    cap_e = cap * 128                       # gathered rows per window
    qn = [0]
    for wg in range(NWIN // WGRP):
        nidx = WGRP * cap_e
        gath = gath_p.tile([128, WGRP * cap, C], bdt, tag="gath")
        base = wg * nidx
        off = 0
        while off < nidx:
            n = min(gcall, nidx - off)
            isl = gidx[:, (base + off) // 16:(base + off + n) // 16]
            nc.gpsimd.dma_gather(
                gath[:, off // 128:(off + n) // 128, :], tab[:], isl,
                n, n, C, queue_num=qn[0])
            qn[0] = (qn[0] + 1) % 4
            off += n
        for wi in range(WGRP):
            w = wg * WGRP + wi
            ohs = oh_p.tile([128, cap, WIN], mybir.dt.float8e4, tag="ohs")
            nc.sync.dma_start(
                ohs[:].rearrange("p t c -> p (t c)"),
                oh_d[:][:, w * cap * WIN:(w + 1) * cap * WIN])
            ap = ps1.tile([C, WIN], fdt, tag="ps1")
            for t in range(cap):
                nc.tensor.matmul(ap[:], gath[:, wi * cap + t, :], ohs[:, t, :],
                                 start=(t == 0), stop=(t == cap - 1))
            nc.vector.tensor_add(g_bf[:, w * WIN:(w + 1) * WIN], ap[:],
                                 hT[:, w * WIN:(w + 1) * WIN])

    # issue the previous stream's table AllGather now: all of this stream's
    # gather calls are already queued on gpsimd, so the in-order queue does
    # not stall waiting for the previous stream's MLP/table write.
    if pending[0] is not None:
        pending[0]()
        pending[0] = None

    # GIN MLP + combine with attention: acc = h1 + h2
    acc_bf = big_p.tile([C, NPC], bdt, tag="acc_bf")
    r_bf = big_p.tile([C, NPC], bdt, tag="r_bf")
    for ch in range(2):
        sl = slice(ch * 1024, (ch + 1) * 1024)
        tp = ps2.tile([C, 1024], fdt, tag="ps2")
        _mm(nc, tp, W(f"gw1_{l}"), g_bf[:, sl], start=True, stop=True)
        nc.scalar.activation(r_bf[:, sl], tp[:], AF.Relu, bias=B(f"gb1_{l}"))
        up = ps2.tile([C, 1024], fdt, tag="ps2")
        _mm(nc, up, W(f"gw2_{l}"), r_bf[:, sl], start=True, stop=True)
        h1 = chunk_p.tile([C, 1024], fdt, tag="h1")
        nc.scalar.activation(h1[:], up[:], AFI, bias=B(f"sgb2_{l}"), scale=S_BN)
        ap2 = ps2.tile([C, 1024], fdt, tag="ps2")
        _mm(nc, ap2, W(f"ow_{l}"), o_bf[:, sl], start=True, stop=False)
        _mm(nc, ap2, W("I2"), hB[:, sl], start=False, stop=True)
        h2 = chunk_p.tile([C, 1024], fdt, tag="h2")
        nc.scalar.activation(h2[:], ap2[:], AFI, bias=B(f"sob_{l}"), scale=S_BN)
        nc.vector.tensor_add(acc_bf[:, sl], h1[:], h2[:])

    # MLP (per-chunk to keep r2 small)
    for ch in range(2):
        sl = slice(ch * 1024, (ch + 1) * 1024)
        r2_bf = chunk_p.tile([C, 2, 1024], bdt, tag="r2_bf")
        for mh in range(2):
            mp = ps2.tile([C, 1024], fdt, tag="ps2")
            _mm(nc, mp, W(f"m1_{l}", width=256)[:, mh * 128:(mh + 1) * 128],
                acc_bf[:, sl], start=True, stop=True)
            bname = f"m1ba_{l}" if mh == 0 else f"m1bb_{l}"
            nc.scalar.activation(r2_bf[:, mh, :], mp[:], AF.Relu, bias=B(bname))
        m2p = ps2.tile([C, 1024], fdt, tag="ps2")
        _mm(nc, m2p, W(f"m2a_{l}"), r2_bf[:, 0, :], start=True, stop=False)
        _mm(nc, m2p, W(f"m2b_{l}"), r2_bf[:, 1, :], start=False, stop=False)
        _mm(nc, m2p, W("I1"), acc_bf[:, sl], start=False, stop=True)
        dh = chunk_p.tile([C, 1024], fdt, tag="dh")
        nc.scalar.activation(dh[:], m2p[:], AFI, bias=B(f"smb2_{l}"), scale=S_BN)
        nc.vector.tensor_add(hT[:, sl], hT[:, sl], dh[:])
        nc.vector.tensor_copy(hB[:, sl], hT[:, sl])

    if not last:
        def _emit(s=s, hB=hB):
            tabs[s] = emit_table(hB)
        pending[0] = _emit


# ---------------------------------------------------------------------------
# Entry point
# ---------------------------------------------------------------------------

_CACHE = {}


def _get_kernel(caps, offs, boffs, wcols, bcols, **kw):
    key = (tuple(caps), wcols, bcols, tuple(sorted(kw.items())))
    if key not in _CACHE:
        _CACHE[key] = build_kernel(caps, offs, boffs, wcols, bcols, **kw)
    return _CACHE[key]


def kernel(**inputs):
    in_maps, caps, offs, boffs, wcols, bcols = _pack_host(inputs)
    nc = _get_kernel(caps, offs, boffs, wcols, bcols)
    res = run_bass_kernel_spmd(nc, in_maps, core_ids=list(range(N_CORES)))
    pools = []
    for si in range(3):
        parts = [np.asarray(res.results[c]["pool_out"][si])
                 for c in range(N_CORES)]
        full = np.concatenate(parts, axis=1)          # [C, 64]
        pools.append(np.ascontiguousarray(full.T).astype(np.float32))
    return tuple(pools)


# revision 24
# speedup vs baseline: 1.5139x; 1.0340x over previous
"""Trainium2 Bass kernel for nn_CGT_21354577396059 (GPS-style GNN, 3 streams x 3 layers).

Strategy (8 NeuronCores, SPMD):
- Node-shard: core c owns nodes [2048c, 2048c+2048) = 8 graphs of 256 nodes.
- Activations feature-major in SBUF: hT [C=128 partitions, 2048 nodes] fp32,
  bf16 copies as matmul inputs.
- GIN segment_sum: edges dst-sorted per core, deduped by src per 128-dst
  window (multi-hot count columns); src rows gathered from a bf16 node-major
  DRAM table (gpsimd dma_gather, 256B rows, one call per window-group);
  scatter via count-matrix matmuls aggT += gathered_chunk.T @ onehot_chunk.
- The bf16 node table is stored P-MAJOR per core block (row p*16+k holds node
  k*128+p) so the SBUF->DRAM table write is contiguous; gather indices are
  host-permuted to match. Table rebuilt each layer via PE transpose + 8-core
  AllGather.
- One-hot count matrices are stored partition-major in DRAM so their loads are
  contiguous (4KB+ per-partition descriptors instead of 256B).
- Attention is graph-local: scoresT = kT.T @ qT per (graph, head, key-chunk)
  masked-K; exp on ACT; softmax sums via col-tiled ones-matmuls; o via
  col-tiled matmuls contracted over keys; normalization via ACT reciprocal.

kernel(**inputs) takes the FULL unsharded inputs and returns
(pool(h0), pool(ha), pool(hb)) -- tuple of [64, 128] float32 -- like the reference.
"""
import sys
import numpy as np
import ml_dtypes

if "/opt/trn_rl_repo" not in sys.path:
    sys.path.insert(0, "/opt/trn_rl_repo")

import concourse.bass as bass  # noqa: F401
import concourse.tile as tile
from concourse import bacc, mybir
from concourse.bass_utils import run_bass_kernel_spmd

BF = ml_dtypes.bfloat16

# Problem constants (self-contained; no reads of /root/problem/*)
N_NODES = 16384
N_GRAPHS = 64
NPG = 256
FEA_DIM = 32
PE_DIM = 20
C = 128
HEADS = 4
HD = C // HEADS
L = 3
BN_EPS = 1e-5
S_BN = float(1.0 / np.sqrt(1.0 + BN_EPS))

N_CORES = 8
NPC = N_NODES // N_CORES   # 2048
NG_C = NPC // NPG          # 8 graphs per core
WIN = 128
NWIN = NPC // WIN          # 16
NCHUNK = NPC // 128        # 16
WGRP = 1                   # windows per dma_gather call

fdt = mybir.dt.float32
bdt = mybir.dt.bfloat16
i16 = mybir.dt.int16
AF = mybir.ActivationFunctionType
AX = mybir.AxisListType
ALU = mybir.AluOpType
AG_GROUPS = [list(range(N_CORES))]


# ---------------------------------------------------------------------------
# Host-side data prep
# ---------------------------------------------------------------------------

def _perm_row(n):
    """Global table row for node n with p-major per-core blocks."""
    cb, m = n // NPC, n % NPC
    return cb * NPC + (m % 128) * NCHUNK + m // 128


def _wrap_idxs(idx):
    """dma_gather idx layout [128, n/16] int16: idx i at (i%16, i//16),
    replicated across the 8 16-partition blocks."""
    n = len(idx)
    a = np.asarray(idx, np.int16).reshape(n // 16, 16).T
    return np.ascontiguousarray(np.tile(a, (8, 1)))


def _prep_edges_stream(edge_index):
    """Dedup by src per (core, 128-dst window); count-matrix columns.

    Returns (cap_chunks, [(gidx_wrapped, oh_pmajor)] per core) where
    oh_pmajor is [128, NWIN*cap*WIN] bf16 (partition-major onehot)."""
    src = np.asarray(edge_index[0]).astype(np.int64)
    dst = np.asarray(edge_index[1]).astype(np.int64)
    per_core_wins = []
    max_w = 0
    for c in range(N_CORES):
        m = (dst >= c * NPC) & (dst < (c + 1) * NPC)
        s, d = src[m], dst[m] - c * NPC
        wins = []
        for w in range(NWIN):
            mw = (d >= w * WIN) & (d < (w + 1) * WIN)
            sw, dw = s[mw], d[mw] - w * WIN
            # dedup srcs within the window; multi-hot count columns
            uniq, inv = np.unique(sw, return_inverse=True)
            cnt = np.zeros((len(uniq), WIN), np.float32)
            np.add.at(cnt, (inv, dw), 1.0)
            # sort rows by permuted table address for DMA page locality
            order = np.argsort(_perm_row(uniq), kind="stable")
            uniq, cnt = uniq[order], cnt[order]
            wins.append((uniq, cnt))
            max_w = max(max_w, len(uniq))
        per_core_wins.append(wins)
    cap_e = ((max_w + 127) // 128) * 128     # rows per window, padded
    cap = cap_e // 128
    out = []
    for c in range(N_CORES):
        srcs = np.zeros(NWIN * cap_e, np.int64)
        oh = np.zeros((NWIN, cap_e, WIN), np.float32)
        for w in range(NWIN):
            uniq, cnt = per_core_wins[c][w]
            n = len(uniq)
            srcs[w * cap_e:w * cap_e + n] = _perm_row(uniq)
            oh[w, :n, :] = cnt
        # partition-major: oh_pm[p, (w, t, c)] = oh[w, t*128+p, c]
        # fp8e4: counts 1..3 are exact; halves the DMA bytes
        oh_pm = np.ascontiguousarray(
            oh.reshape(NWIN, cap, 128, WIN).transpose(2, 0, 1, 3)
            .reshape(128, NWIN * cap * WIN).astype(ml_dtypes.float8_e4m3fn))
        out.append((_wrap_idxs(srcs), oh_pm))
    return cap, out


def _pack_host(inputs):
    inp = {k: np.asarray(v) for k, v in inputs.items()}
    rt2 = 1.0 / np.sqrt(HD)

    blocks, offs = [], {}

    def add(name, arr):
        arr = np.asarray(arr, np.float32)
        k, m = arr.shape
        buf = np.zeros((128, m), BF)
        buf[:k] = arr.astype(BF)
        offs[name] = sum(b.shape[1] for b in blocks)
        blocks.append(buf)

    add("emb", inp["node_emb_w"])
    add("pe", inp["pe_lin_w"])
    add("I2", 2.0 * np.eye(C))       # h2 fold: ACT scale s gives 2s*h
    add("I1", np.eye(C))             # transpose identity + acc fold
    add("ones32", np.ones((C, HD)))
    for l in range(L):
        aw = inp["attn_in_w"][l]
        add(f"gw1_{l}", inp["gin_w1"][l])
        add(f"gw2_{l}", inp["gin_w2"][l])
        add(f"qT_{l}", (aw[0:C] * rt2).T)
        add(f"kT_{l}", aw[C:2 * C].T)
        add(f"vT_{l}", aw[2 * C:3 * C].T)
        add(f"ow_{l}", inp["attn_out_w"][l].T)
        add(f"m1_{l}", inp["mlp_w1"][l])
        add(f"m2a_{l}", inp["mlp_w2"][l][0:C])
        add(f"m2b_{l}", inp["mlp_w2"][l][C:2 * C])
    wts = np.ascontiguousarray(np.concatenate(blocks, axis=1))

    bvecs, boffs = [], {}

    def addb(name, vec):
        vec = np.asarray(vec, np.float32).reshape(-1)
        assert vec.shape == (C,)
        boffs[name] = len(bvecs)
        bvecs.append(vec)

    addb("eb", inp["node_emb_b"] + inp["pe_lin_b"])
    for l in range(L):
        ab = inp["attn_in_b"][l]
        addb(f"gb1_{l}", inp["gin_b1"][l])
        addb(f"sgb2_{l}", S_BN * inp["gin_b2"][l])
        addb(f"qb_{l}", ab[0:C] * rt2)
        addb(f"kb_{l}", ab[C:2 * C])
        addb(f"sob_{l}", S_BN * inp["attn_out_b"][l])
        addb(f"m1ba_{l}", inp["mlp_b1"][l][0:C])
        addb(f"m1bb_{l}", inp["mlp_b1"][l][C:2 * C])
        addb(f"smb2_{l}", S_BN * inp["mlp_b2"][l])
    biases = np.ascontiguousarray(np.stack(bvecs, axis=1).astype(np.float32))

    vbias = np.ascontiguousarray(np.stack(
        [np.tile(inp["attn_in_b"][l][2 * C:3 * C], (128, 1)) for l in range(L)]
    ).astype(np.float32))

    caps, edges = [], []
    for key in ("edge_index", "edge_index1", "edge_index2"):
        cap, per_core = _prep_edges_stream(inp[key])
        caps.append(cap)
        edges.append(per_core)

    xs = [inp["x"], inp["x1"], inp["x2"]]
    in_maps = []
    for c in range(N_CORES):
        m = {"wts": wts, "biases": biases, "vbias": vbias}
        sl = slice(c * NPC, (c + 1) * NPC)
        for s in range(3):
            m[f"xT{s}"] = np.ascontiguousarray(xs[s][sl].T.astype(BF))
            m[f"gidx{s}"] = edges[s][c][0]
            m[f"onehot{s}"] = edges[s][c][1]
        m["peT"] = np.ascontiguousarray(inp["pe"][sl].T.astype(BF))
        in_maps.append(m)

    return in_maps, caps, offs, boffs, wts.shape[1], biases.shape[1]


# ---------------------------------------------------------------------------
# Kernel builder
# ---------------------------------------------------------------------------


def _mm(nc, out, lhsT, rhs, start, stop, nmax=512):
    """matmul with moving free dim split to <=512 (ISA limit)."""
    n = rhs.shape[-1]
    assert out.shape[-1] == n
    for i in range(0, n, nmax):
        j = min(i + nmax, n)
        nc.tensor.matmul(out[:, i:j], lhsT, rhs[:, i:j], start=start, stop=stop)


def build_kernel(caps, offs, boffs, wcols, bcols, n_layers=L, n_streams=3,
                 gcall=1024, dbg_stream=None):
    nc = bacc.Bacc("TRN2", target_bir_lowering=False, num_devices=N_CORES,
                   num_swdge_queues=4)

    wts_d = nc.dram_tensor("wts", [128, wcols], bdt, kind="ExternalInput")
    bias_d = nc.dram_tensor("biases", [128, bcols], fdt, kind="ExternalInput")
    vbias_d = nc.dram_tensor("vbias", [L, 128, 128], fdt, kind="ExternalInput")
    peT_d = nc.dram_tensor("peT", [PE_DIM, NPC], bdt, kind="ExternalInput")
    xT_d, gidx_d, oh_d = [], [], []
    for s in range(3):
        cap = caps[s]
        xT_d.append(nc.dram_tensor(f"xT{s}", [FEA_DIM, NPC], bdt,
                                   kind="ExternalInput"))
        gidx_d.append(nc.dram_tensor(f"gidx{s}", [128, NWIN * cap * 8], i16,
                                     kind="ExternalInput"))
        oh_d.append(nc.dram_tensor(f"onehot{s}", [128, NWIN * cap * WIN],
                                   mybir.dt.float8e4, kind="ExternalInput"))
    pool_out = nc.dram_tensor("pool_out", [3, C, NG_C], fdt,
                              kind="ExternalOutput")
    dbg_out = None
    if dbg_stream is not None:
        dbg_out = nc.dram_tensor("dbg_out", [C, NPC], fdt, kind="ExternalOutput")

    with tile.TileContext(nc) as tc:
        with (
            tc.tile_pool(name="const", bufs=1) as const_p,
            tc.tile_pool(name="hstate", bufs=1) as hstate_p,
            tc.tile_pool(name="big", bufs=1) as big_p,       # full-width tiles
            tc.tile_pool(name="chunk", bufs=2) as chunk_p,   # [C,1024]-ish tiles
            tc.tile_pool(name="gath", bufs=4) as gath_p,
            tc.tile_pool(name="ohp", bufs=2) as oh_p,
            tc.tile_pool(name="ps2", bufs=3, space="PSUM") as ps2,   # 2-bank
            tc.tile_pool(name="ps1", bufs=2, space="PSUM") as ps1,   # 1-bank
            tc.tile_pool(name="dram", bufs=4, space="DRAM") as dram_p,
        ):
            wts = const_p.tile([128, wcols], bdt)
            nc.sync.dma_start(wts[:], wts_d[:])
            bia = const_p.tile([128, bcols], fdt)
            nc.sync.dma_start(bia[:], bias_d[:])
            vbia = const_p.tile([128, L, 128], fdt)
            nc.sync.dma_start(vbia[:], vbias_d[:].rearrange("l p c -> p l c"))

            def W(name, width=128, rows=128):
                return wts[0:rows, offs[name]:offs[name] + width]

            def B(name):
                return bia[:, boffs[name]:boffs[name] + 1]

            hT = [hstate_p.tile([C, NPC], fdt, tag=f"hT{s}", name=f"hT{s}")
                  for s in range(3)]
            hB = [hstate_p.tile([C, NPC], bdt, tag=f"hB{s}", name=f"hB{s}")
                  for s in range(3)]
            gidx = []
            for s in range(n_streams):
                t = hstate_p.tile([128, NWIN * caps[s] * 8], i16,
                                  tag=f"gidx{s}", name=f"gidx{s}")
                nc.sync.dma_start(t[:], gidx_d[s][:])
                gidx.append(t)

            def emit_table(src_bf):
                """feature-major bf16 [C, NPC] -> p-major node table ->
                AllGather. Block layout: DRAM row p*NCHUNK+k = node k*128+p."""
                nm = big_p.tile([128, NCHUNK, C], bdt, tag="nm")
                for k in range(NCHUNK):
                    pt = ps1.tile([128, 128], bdt, tag="ps1", name="pt")
                    nc.tensor.transpose(pt[:], src_bf[:, k * 128:(k + 1) * 128],
                                        W("I1"))
                    nc.vector.tensor_copy(nm[:, k, :], pt[:])
                agi = dram_p.tile([128, NCHUNK * C], bdt, tag="agi")
                nc.sync.dma_start(agi[:], nm[:].rearrange("p k c -> p (k c)"))
                tab = dram_p.tile([N_NODES, C], bdt, tag="tab", name="tab",
                                  addr_space="Shared")
                nc.gpsimd.collective_compute(
                    "AllGather", ALU.bypass, replica_groups=AG_GROUPS,
                    ins=[agi.opt()], outs=[tab.opt()])
                return tab

            # ---------------- embedding ----------------
            tabs = [None] * 3
            pending = [None]   # deferred emit_table issue (keeps gpsimd queue hot)
            for s in range(n_streams):
                xT = chunk_p.tile([FEA_DIM, NPC], bdt, tag="xT")
                nc.sync.dma_start(xT[:], xT_d[s][:])
                peT = chunk_p.tile([PE_DIM, NPC], bdt, tag="peT")
                nc.sync.dma_start(peT[:], peT_d[:])
                for ch in range(2):
                    sl = slice(ch * 1024, (ch + 1) * 1024)
                    ep = ps2.tile([C, 1024], fdt, tag="ps2")
                    _mm(nc, ep, W("emb", rows=FEA_DIM), xT[:, sl],
                        start=True, stop=False)
                    _mm(nc, ep, W("pe", rows=PE_DIM), peT[:, sl],
                        start=False, stop=True)
                    nc.scalar.activation(hT[s][:, sl], ep[:], AF.Identity,
                                         bias=B("eb"))
                    nc.vector.tensor_copy(hB[s][:, sl], hT[s][:, sl])
                if s < n_streams - 1:
                    tabs[s] = emit_table(hB[s])
                else:
                    def _emit_last(s=s):
                        tabs[s] = emit_table(hB[s])
                    pending[0] = _emit_last

            # ---------------- layers ----------------
            for l in range(n_layers):
                for s in range(n_streams):
                    _gps_layer(nc, l, caps[s], hT[s], hB[s], tabs, s,
                               gidx[s], oh_d[s], W, B, vbia,
                               big_p, chunk_p, gath_p, oh_p, ps2, ps1,
                               emit_table, gcall, pending,
                               last=(l == n_layers - 1))

            # ---------------- pooling ----------------
            for s in range(n_streams):
                po = chunk_p.tile([C, NG_C], fdt, tag="pool")
                nc.vector.reduce_sum(
                    out=po[:],
                    in_=hT[s][:].rearrange("c (g n) -> c g n", g=NG_C),
                    axis=AX.X)
                nc.sync.dma_start(pool_out[s], po[:])

            if dbg_stream is not None:
                nc.sync.dma_start(dbg_out[:], hT[dbg_stream][:])

    nc.compile()
    return nc


def _gps_layer(nc, l, cap, hT, hB, tabs, s, gidx, oh_d, W, B, vbia,
               big_p, chunk_p, gath_p, oh_p, ps2, ps1, emit_table, gcall,
               pending, last):
    AFI = AF.Identity

    # ---------------- attention (graph-local) ----------------
    qT = big_p.tile([C, NPC], bdt, tag="qT", name="qT")
    for ch in range(2):
        sl = slice(ch * 1024, (ch + 1) * 1024)
        qp = ps2.tile([C, 1024], fdt, tag="ps2")
        _mm(nc, qp, W(f"qT_{l}"), hB[:, sl], start=True, stop=True)
        nc.scalar.activation(qT[:, sl], qp[:], AFI, bias=B(f"qb_{l}"))
    v_sb = big_p.tile([128, NCHUNK, C], bdt, tag="v_sb", name="v_sb")
    for k in range(NCHUNK):
        vp = ps1.tile([128, C], fdt, tag="ps1")
        nc.tensor.matmul(vp[:], hB[:, k * 128:(k + 1) * 128], W(f"vT_{l}"),
                         start=True, stop=True)
        nc.vector.tensor_add(v_sb[:, k, :], vp[:], vbia[:, l, :])

    kT = big_p.tile([C, NPC], bdt, tag="kTm", name="kT")
    for ch in range(2):
        sl = slice(ch * 1024, (ch + 1) * 1024)
        kp = ps2.tile([C, 1024], fdt, tag="ps2")
        _mm(nc, kp, W(f"kT_{l}"), hB[:, sl], start=True, stop=True)
        nc.scalar.activation(kT[:, sl], kp[:], AFI, bias=B(f"kb_{l}"))
    expT = big_p.tile([128, 2, HEADS, NG_C, NPG], bdt, tag="expT", name="expT")
    for h in range(HEADS):
        for kc in range(2):
            for gh in range(2):
                sp = ps2.tile([128, 4, NPG], fdt, tag="ps2")
                for gi in range(4):
                    g = gh * 4 + gi
                    # per-head K=32 contraction: both operands sliced to the
                    # head's 32 channels, weight tile placed at array row 32h
                    ksl = kT[32 * h:32 * h + 32,
                             g * NPG + kc * 128:g * NPG + kc * 128 + 128]
                    qsl = qT[32 * h:32 * h + 32, g * NPG:(g + 1) * NPG]
                    nc.tensor.matmul(sp[:, gi, :], ksl, qsl,
                                     start=True, stop=True,
                                     tile_position=(32 * h, 0))
                nc.scalar.activation(expT[:, kc, h, gh * 4:gh * 4 + 4, :],
                                     sp[:], AF.Exp)

    recip = big_p.tile([128, NPC], bdt, tag="recip", name="recip")
    for half in range(2):
        sm = ps2.tile([128, 1024], fdt, tag="ps2")
        for h in range(HEADS):
            for qc in range(2):
                osl = sm[32 * h:32 * h + 32, qc * 512:(qc + 1) * 512]
                for kc in range(2):
                    rhs = expT[:, kc, h, :, :].rearrange("p g q -> p (g q)")
                    nc.tensor.matmul(
                        osl, W("ones32", width=HD),
                        rhs[:, half * 1024 + qc * 512:half * 1024 + (qc + 1) * 512],
                        start=(kc == 0), stop=(kc == 1),
                        tile_position=(0, 32 * h))
        with nc.allow_low_precision(reason="softmax recip bf16 ok at 2e-2 gate"):
            nc.vector.reciprocal(recip[:, half * 1024:(half + 1) * 1024], sm[:])

    o_bf = big_p.tile([C, NPC], bdt, tag="o_bf", name="o_bf")
    for g in range(NG_C):
        op = ps1.tile([128, NPG], fdt, tag="ps1")
        for h in range(HEADS):
            for kc in range(2):
                lhs = v_sb[:, g * 2 + kc, 32 * h:32 * h + 32]
                rhs = expT[:, kc, h, g, :]
                nc.tensor.matmul(op[32 * h:32 * h + 32, :], lhs, rhs,
                                 start=(kc == 0), stop=(kc == 1),
                                 tile_position=(0, 32 * h))
        nc.vector.tensor_mul(o_bf[:, g * NPG:(g + 1) * NPG], op[:],
                             recip[:, g * NPG:(g + 1) * NPG])

    # ---------------- GIN ----------------
    g_bf = big_p.tile([C, NPC], bdt, tag="g_bf")
    tab = tabs[s]
    cap_e = cap * 128                       # gathered rows per window
    qn = [0]
    for wg in range(NWIN // WGRP):
        nidx = WGRP * cap_e
        gath = gath_p.tile([128, WGRP * cap, C], bdt, tag="gath")
        base = wg * nidx
        off = 0
        while off < nidx:
            n = min(gcall, nidx - off)
            isl = gidx[:, (base + off) // 16:(base + off + n) // 16]
            nc.gpsimd.dma_gather(
                gath[:, off // 128:(off + n) // 128, :], tab[:], isl,
                n, n, C, queue_num=qn[0])
            qn[0] = (qn[0] + 1) % 4
            off += n
        for wi in range(WGRP):
            w = wg * WGRP + wi
            ohs = oh_p.tile([128, cap, WIN], mybir.dt.float8e4, tag="ohs")
            nc.sync.dma_start(
                ohs[:].rearrange("p t c -> p (t c)"),
                oh_d[:][:, w * cap * WIN:(w + 1) * cap * WIN])
            ap = ps1.tile([C, WIN], fdt, tag="ps1")
            for t in range(cap):
                nc.tensor.matmul(ap[:], gath[:, wi * cap + t, :], ohs[:, t, :],
                                 start=(t == 0), stop=(t == cap - 1))
            nc.vector.tensor_add(g_bf[:, w * WIN:(w + 1) * WIN], ap[:],
                                 hT[:, w * WIN:(w + 1) * WIN])

    # issue the previous stream's table AllGather now: all of this stream's
    # gather calls are already queued on gpsimd, so the in-order queue does
    # not stall waiting for the previous stream's MLP/table write.
    if pending[0] is not None:
        pending[0]()
        pending[0] = None

    # GIN MLP + combine with attention: acc = h1 + h2
    acc_bf = big_p.tile([C, NPC], bdt, tag="acc_bf")
    r_bf = big_p.tile([C, NPC], bdt, tag="r_bf")
    for ch in range(2):
        sl = slice(ch * 1024, (ch + 1) * 1024)
        tp = ps2.tile([C, 1024], fdt, tag="ps2")
        _mm(nc, tp, W(f"gw1_{l}"), g_bf[:, sl], start=True, stop=True)
        nc.scalar.activation(r_bf[:, sl], tp[:], AF.Relu, bias=B(f"gb1_{l}"))
        up = ps2.tile([C, 1024], fdt, tag="ps2")
        _mm(nc, up, W(f"gw2_{l}"), r_bf[:, sl], start=True, stop=True)
        h1 = chunk_p.tile([C, 1024], fdt, tag="h1")
        nc.scalar.activation(h1[:], up[:], AFI, bias=B(f"sgb2_{l}"), scale=S_BN)
        ap2 = ps2.tile([C, 1024], fdt, tag="ps2")
        _mm(nc, ap2, W(f"ow_{l}"), o_bf[:, sl], start=True, stop=False)
        _mm(nc, ap2, W("I2"), hB[:, sl], start=False, stop=True)
        h2 = chunk_p.tile([C, 1024], fdt, tag="h2")
        nc.scalar.activation(h2[:], ap2[:], AFI, bias=B(f"sob_{l}"), scale=S_BN)
        nc.vector.tensor_add(acc_bf[:, sl], h1[:], h2[:])

    # MLP (per-chunk to keep r2 small)
    for ch in range(2):
        sl = slice(ch * 1024, (ch + 1) * 1024)
        r2_bf = chunk_p.tile([C, 2, 1024], bdt, tag="r2_bf")
        for mh in range(2):
            mp = ps2.tile([C, 1024], fdt, tag="ps2")
            _mm(nc, mp, W(f"m1_{l}", width=256)[:, mh * 128:(mh + 1) * 128],
                acc_bf[:, sl], start=True, stop=True)
            bname = f"m1ba_{l}" if mh == 0 else f"m1bb_{l}"
            nc.scalar.activation(r2_bf[:, mh, :], mp[:], AF.Relu, bias=B(bname))
        m2p = ps2.tile([C, 1024], fdt, tag="ps2")
        _mm(nc, m2p, W(f"m2a_{l}"), r2_bf[:, 0, :], start=True, stop=False)
        _mm(nc, m2p, W(f"m2b_{l}"), r2_bf[:, 1, :], start=False, stop=False)
        _mm(nc, m2p, W("I1"), acc_bf[:, sl], start=False, stop=True)
        dh = chunk_p.tile([C, 1024], fdt, tag="dh")
        nc.scalar.activation(dh[:], m2p[:], AFI, bias=B(f"smb2_{l}"), scale=S_BN)
        nc.vector.tensor_add(hT[:, sl], hT[:, sl], dh[:])
        nc.vector.tensor_copy(hB[:, sl], hT[:, sl])

    if not last:
        def _emit(s=s, hB=hB):
            tabs[s] = emit_table(hB)
        pending[0] = _emit


# ---------------------------------------------------------------------------
# Entry point
# ---------------------------------------------------------------------------

_CACHE = {}


def _get_kernel(caps, offs, boffs, wcols, bcols, **kw):
    key = (tuple(caps), wcols, bcols, tuple(sorted(kw.items())))
    if key not in _CACHE:
        _CACHE[key] = build_kernel(caps, offs, boffs, wcols, bcols, **kw)
    return _CACHE[key]


def kernel(**inputs):
    in_maps, caps, offs, boffs, wcols, bcols = _pack_host(inputs)
    nc = _get_kernel(caps, offs, boffs, wcols, bcols)
    res = run_bass_kernel_spmd(nc, in_maps, core_ids=list(range(N_CORES)))
    pools = []
    for si in range(3):
        parts = [np.asarray(res.results[c]["pool_out"][si])
                 for c in range(N_CORES)]
        full = np.concatenate(parts, axis=1)          # [C, 64]
        pools.append(np.ascontiguousarray(full.T).astype(np.float32))
    return tuple(pools)


# revision 28
# speedup vs baseline: 1.6178x; 1.0687x over previous
"""Trainium2 Bass kernel for nn_CGT_21354577396059 (GPS-style GNN, 3 streams x 3 layers).

Strategy (8 NeuronCores, SPMD):
- Node-shard: core c owns nodes [2048c, 2048c+2048) = 8 graphs of 256 nodes.
- Activations feature-major in SBUF: hT [C=128 partitions, 2048 nodes] fp32,
  bf16 copies as matmul inputs.
- GIN segment_sum: edges dst-sorted per core, deduped by src per 128-dst
  window (multi-hot count columns); src rows gathered from a bf16 node-major
  DRAM table (gpsimd dma_gather, 256B rows, one call per window-group);
  scatter via count-matrix matmuls aggT += gathered_chunk.T @ onehot_chunk.
- The bf16 node table is stored P-MAJOR per core block (row p*16+k holds node
  k*128+p) so the SBUF->DRAM table write is contiguous; gather indices are
  host-permuted to match. Table rebuilt each layer via PE transpose + 8-core
  AllGather.
- One-hot count matrices are stored partition-major in DRAM so their loads are
  contiguous (4KB+ per-partition descriptors instead of 256B).
- Attention is graph-local: scoresT = kT.T @ qT per (graph, head, key-chunk)
  masked-K; exp on ACT; softmax sums via col-tiled ones-matmuls; o via
  col-tiled matmuls contracted over keys; normalization via ACT reciprocal.

kernel(**inputs) takes the FULL unsharded inputs and returns
(pool(h0), pool(ha), pool(hb)) -- tuple of [64, 128] float32 -- like the reference.
"""
import sys
import numpy as np
import ml_dtypes

if "/opt/trn_rl_repo" not in sys.path:
    sys.path.insert(0, "/opt/trn_rl_repo")

import concourse.bass as bass  # noqa: F401
import concourse.tile as tile
from concourse import bacc, mybir
from concourse.bass_utils import run_bass_kernel_spmd

BF = ml_dtypes.bfloat16

# Problem constants (self-contained; no reads of /root/problem/*)
N_NODES = 16384
N_GRAPHS = 64
NPG = 256
FEA_DIM = 32
PE_DIM = 20
C = 128
HEADS = 4
HD = C // HEADS
L = 3
BN_EPS = 1e-5
S_BN = float(1.0 / np.sqrt(1.0 + BN_EPS))

N_CORES = 8
NPC = N_NODES // N_CORES   # 2048
NG_C = NPC // NPG          # 8 graphs per core
WIN = 128
NWIN = NPC // WIN          # 16
NCHUNK = NPC // 128        # 16
WGRP = 1                   # windows per dma_gather call

fdt = mybir.dt.float32
bdt = mybir.dt.bfloat16
i16 = mybir.dt.int16
AF = mybir.ActivationFunctionType
AX = mybir.AxisListType
ALU = mybir.AluOpType
AG_GROUPS = [list(range(N_CORES))]


# ---------------------------------------------------------------------------
# Host-side data prep
# ---------------------------------------------------------------------------

def _perm_row(n):
    """Global table row for node n with p-major per-core blocks."""
    cb, m = n // NPC, n % NPC
    return cb * NPC + (m % 128) * NCHUNK + m // 128


def _wrap_idxs(idx):
    """dma_gather idx layout [128, n/16] int16: idx i at (i%16, i//16),
    replicated across the 8 16-partition blocks."""
    n = len(idx)
    a = np.asarray(idx, np.int16).reshape(n // 16, 16).T
    return np.ascontiguousarray(np.tile(a, (8, 1)))


def _prep_edges_stream(edge_index):
    """Dedup by src per (core, 128-dst window); count-matrix columns.

    Returns (cap_chunks, [(gidx_wrapped, oh_pmajor)] per core) where
    oh_pmajor is [128, NWIN*cap*WIN] bf16 (partition-major onehot)."""
    src = np.asarray(edge_index[0]).astype(np.int64)
    dst = np.asarray(edge_index[1]).astype(np.int64)
    per_core_wins = []
    max_w = 0
    for c in range(N_CORES):
        m = (dst >= c * NPC) & (dst < (c + 1) * NPC)
        s, d = src[m], dst[m] - c * NPC
        wins = []
        for w in range(NWIN):
            mw = (d >= w * WIN) & (d < (w + 1) * WIN)
            sw, dw = s[mw], d[mw] - w * WIN
            # dedup srcs within the window; multi-hot count columns
            uniq, inv = np.unique(sw, return_inverse=True)
            cnt = np.zeros((len(uniq), WIN), np.float32)
            np.add.at(cnt, (inv, dw), 1.0)
            # sort rows by permuted table address for DMA page locality
            order = np.argsort(_perm_row(uniq), kind="stable")
            uniq, cnt = uniq[order], cnt[order]
            wins.append((uniq, cnt))
            max_w = max(max_w, len(uniq))
        per_core_wins.append(wins)
    cap_e = ((max_w + 127) // 128) * 128     # rows per window, padded
    cap = cap_e // 128
    out = []
    for c in range(N_CORES):
        srcs = np.zeros(NWIN * cap_e, np.int64)
        oh = np.zeros((NWIN, cap_e, WIN), np.float32)
        for w in range(NWIN):
            uniq, cnt = per_core_wins[c][w]
            n = len(uniq)
            srcs[w * cap_e:w * cap_e + n] = _perm_row(uniq)
            oh[w, :n, :] = cnt
        # partition-major: oh_pm[p, (w, t, c)] = oh[w, t*128+p, c]
        # fp8e4: counts 1..3 are exact; halves the DMA bytes
        oh_pm = np.ascontiguousarray(
            oh.reshape(NWIN, cap, 128, WIN).transpose(2, 0, 1, 3)
            .reshape(128, NWIN * cap * WIN).astype(ml_dtypes.float8_e4m3fn))
        out.append((_wrap_idxs(srcs), oh_pm))
    return cap, out


def _pack_host(inputs):
    inp = {k: np.asarray(v) for k, v in inputs.items()}
    rt2 = 1.0 / np.sqrt(HD)

    blocks, offs = [], {}

    def add(name, arr):
        arr = np.asarray(arr, np.float32)
        k, m = arr.shape
        buf = np.zeros((128, m), BF)
        buf[:k] = arr.astype(BF)
        offs[name] = sum(b.shape[1] for b in blocks)
        blocks.append(buf)

    add("emb", inp["node_emb_w"])
    add("pe", inp["pe_lin_w"])
    add("I2", 2.0 * np.eye(C))       # h2 fold: ACT scale s gives 2s*h
    add("I1", np.eye(C))             # transpose identity + acc fold
    add("ones32", np.ones((C, HD)))
    for l in range(L):
        aw = inp["attn_in_w"][l]
        add(f"gw1_{l}", inp["gin_w1"][l])
        add(f"gw2_{l}", inp["gin_w2"][l])
        add(f"qT_{l}", (aw[0:C] * rt2).T)
        add(f"kT_{l}", aw[C:2 * C].T)
        add(f"vT_{l}", aw[2 * C:3 * C].T)
        add(f"ow_{l}", inp["attn_out_w"][l].T)
        add(f"m1_{l}", inp["mlp_w1"][l])
        add(f"m2a_{l}", inp["mlp_w2"][l][0:C])
        add(f"m2b_{l}", inp["mlp_w2"][l][C:2 * C])
    wts = np.ascontiguousarray(np.concatenate(blocks, axis=1))

    bvecs, boffs = [], {}

    def addb(name, vec):
        vec = np.asarray(vec, np.float32).reshape(-1)
        assert vec.shape == (C,)
        boffs[name] = len(bvecs)
        bvecs.append(vec)

    addb("eb", inp["node_emb_b"] + inp["pe_lin_b"])
    for l in range(L):
        ab = inp["attn_in_b"][l]
        addb(f"gb1_{l}", inp["gin_b1"][l])
        addb(f"sgb2_{l}", S_BN * inp["gin_b2"][l])
        addb(f"qb_{l}", ab[0:C] * rt2)
        addb(f"kb_{l}", ab[C:2 * C])
        addb(f"sob_{l}", S_BN * inp["attn_out_b"][l])
        addb(f"m1ba_{l}", inp["mlp_b1"][l][0:C])
        addb(f"m1bb_{l}", inp["mlp_b1"][l][C:2 * C])
        addb(f"smb2_{l}", S_BN * inp["mlp_b2"][l])
    biases = np.ascontiguousarray(np.stack(bvecs, axis=1).astype(np.float32))

    vbias = np.ascontiguousarray(np.stack(
        [np.tile(inp["attn_in_b"][l][2 * C:3 * C], (128, 1)) for l in range(L)]
    ).astype(np.float32))

    caps, edges = [], []
    for key in ("edge_index", "edge_index1", "edge_index2"):
        cap, per_core = _prep_edges_stream(inp[key])
        caps.append(cap)
        edges.append(per_core)

    xs = [inp["x"], inp["x1"], inp["x2"]]
    in_maps = []
    for c in range(N_CORES):
        m = {"wts": wts, "biases": biases, "vbias": vbias}
        sl = slice(c * NPC, (c + 1) * NPC)
        for s in range(3):
            m[f"xT{s}"] = np.ascontiguousarray(xs[s][sl].T.astype(BF))
            m[f"gidx{s}"] = edges[s][c][0]
            m[f"onehot{s}"] = edges[s][c][1]
        m["peT"] = np.ascontiguousarray(inp["pe"][sl].T.astype(BF))
        in_maps.append(m)

    return in_maps, caps, offs, boffs, wts.shape[1], biases.shape[1]


# ---------------------------------------------------------------------------
# Kernel builder
# ---------------------------------------------------------------------------


def _mm(nc, out, lhsT, rhs, start, stop, nmax=512):
    """matmul with moving free dim split to <=512 (ISA limit)."""
    n = rhs.shape[-1]
    assert out.shape[-1] == n
    for i in range(0, n, nmax):
        j = min(i + nmax, n)
        nc.tensor.matmul(out[:, i:j], lhsT, rhs[:, i:j], start=start, stop=stop)


def build_kernel(caps, offs, boffs, wcols, bcols, n_layers=L, n_streams=3,
                 gcall=1024, dbg_stream=None):
    nc = bacc.Bacc("TRN2", target_bir_lowering=False, num_devices=N_CORES,
                   num_swdge_queues=4)

    wts_d = nc.dram_tensor("wts", [128, wcols], bdt, kind="ExternalInput")
    bias_d = nc.dram_tensor("biases", [128, bcols], fdt, kind="ExternalInput")
    vbias_d = nc.dram_tensor("vbias", [L, 128, 128], fdt, kind="ExternalInput")
    peT_d = nc.dram_tensor("peT", [PE_DIM, NPC], bdt, kind="ExternalInput")
    xT_d, gidx_d, oh_d = [], [], []
    for s in range(3):
        cap = caps[s]
        xT_d.append(nc.dram_tensor(f"xT{s}", [FEA_DIM, NPC], bdt,
                                   kind="ExternalInput"))
        gidx_d.append(nc.dram_tensor(f"gidx{s}", [128, NWIN * cap * 8], i16,
                                     kind="ExternalInput"))
        oh_d.append(nc.dram_tensor(f"onehot{s}", [128, NWIN * cap * WIN],
                                   mybir.dt.float8e4, kind="ExternalInput"))
    pool_out = nc.dram_tensor("pool_out", [3, C, NG_C], fdt,
                              kind="ExternalOutput")
    dbg_out = None
    if dbg_stream is not None:
        dbg_out = nc.dram_tensor("dbg_out", [C, NPC], fdt, kind="ExternalOutput")

    with tile.TileContext(nc) as tc:
        with (
            tc.tile_pool(name="const", bufs=1) as const_p,
            tc.tile_pool(name="hstate", bufs=1) as hstate_p,
            tc.tile_pool(name="big", bufs=1) as big_p,       # full-width tiles
            tc.tile_pool(name="chunk", bufs=2) as chunk_p,   # [C,1024]-ish tiles
            tc.tile_pool(name="gath", bufs=6) as gath_p,
            tc.tile_pool(name="ohp", bufs=3) as oh_p,
            tc.tile_pool(name="ps2", bufs=3, space="PSUM") as ps2,   # 2-bank
            tc.tile_pool(name="ps1", bufs=2, space="PSUM") as ps1,   # 1-bank
            tc.tile_pool(name="dram", bufs=4, space="DRAM") as dram_p,
        ):
            wts = const_p.tile([128, wcols], bdt)
            nc.sync.dma_start(wts[:], wts_d[:])
            bia = const_p.tile([128, bcols], fdt)
            nc.sync.dma_start(bia[:], bias_d[:])
            vbia = const_p.tile([128, L, 128], fdt)
            nc.sync.dma_start(vbia[:], vbias_d[:].rearrange("l p c -> p l c"))

            def W(name, width=128, rows=128):
                return wts[0:rows, offs[name]:offs[name] + width]

            def B(name):
                return bia[:, boffs[name]:boffs[name] + 1]

            hT = [hstate_p.tile([C, NPC], fdt, tag=f"hT{s}", name=f"hT{s}")
                  for s in range(3)]
            hB = [hstate_p.tile([C, NPC], bdt, tag=f"hB{s}", name=f"hB{s}")
                  for s in range(3)]
            gidx = [hstate_p.tile([128, NWIN * caps[s] * 8], i16,
                                  tag=f"gidx{s}", name=f"gidx{s}")
                    for s in range(n_streams)]

            def emit_table(src_bf):
                """feature-major bf16 [C, NPC] -> p-major node table ->
                AllGather. Block layout: DRAM row p*NCHUNK+k = node k*128+p."""
                nm = big_p.tile([128, NCHUNK, C], bdt, tag="nm")
                for k in range(NCHUNK):
                    pt = ps1.tile([128, 128], bdt, tag="ps1", name="pt")
                    nc.tensor.transpose(pt[:], src_bf[:, k * 128:(k + 1) * 128],
                                        W("I1"))
                    nc.vector.tensor_copy(nm[:, k, :], pt[:])
                agi = dram_p.tile([128, NCHUNK * C], bdt, tag="agi")
                nc.sync.dma_start(agi[:], nm[:].rearrange("p k c -> p (k c)"))
                tab = dram_p.tile([N_NODES, C], bdt, tag="tab", name="tab",
                                  addr_space="Shared")
                nc.gpsimd.collective_compute(
                    "AllGather", ALU.bypass, replica_groups=AG_GROUPS,
                    ins=[agi.opt()], outs=[tab.opt()])
                return tab

            # ---------------- embedding ----------------
            tabs = [None] * 3
            pending = [None]   # deferred emit_table issue (keeps gpsimd queue hot)
            for s in range(n_streams):
                xT = chunk_p.tile([FEA_DIM, NPC], bdt, tag="xT")
                nc.sync.dma_start(xT[:], xT_d[s][:])
                # gather idxs are not needed until this stream's first GIN
                # window; keep them off the startup critical path
                nc.sync.dma_start(gidx[s][:], gidx_d[s][:])
                peT = chunk_p.tile([PE_DIM, NPC], bdt, tag="peT")
                nc.sync.dma_start(peT[:], peT_d[:])
                for ch in range(2):
                    sl = slice(ch * 1024, (ch + 1) * 1024)
                    ep = ps2.tile([C, 1024], fdt, tag="ps2")
                    _mm(nc, ep, W("emb", rows=FEA_DIM), xT[:, sl],
                        start=True, stop=False)
                    _mm(nc, ep, W("pe", rows=PE_DIM), peT[:, sl],
                        start=False, stop=True)
                    nc.scalar.activation(hT[s][:, sl], ep[:], AF.Identity,
                                         bias=B("eb"))
                    nc.vector.tensor_copy(hB[s][:, sl], hT[s][:, sl])
                if s < n_streams - 1:
                    tabs[s] = emit_table(hB[s])
                else:
                    def _emit_last(s=s):
                        tabs[s] = emit_table(hB[s])
                    pending[0] = _emit_last

            # ---------------- layers ----------------
            for l in range(n_layers):
                for s in range(n_streams):
                    _gps_layer(nc, l, caps[s], hT[s], hB[s], tabs, s,
                               gidx[s], oh_d[s], W, B, vbia,
                               big_p, chunk_p, gath_p, oh_p, ps2, ps1,
                               emit_table, gcall, pending,
                               last=(l == n_layers - 1))

            # ---------------- pooling ----------------
            for s in range(n_streams):
                po = chunk_p.tile([C, NG_C], fdt, tag="pool")
                nc.vector.reduce_sum(
                    out=po[:],
                    in_=hT[s][:].rearrange("c (g n) -> c g n", g=NG_C),
                    axis=AX.X)
                nc.sync.dma_start(pool_out[s], po[:])

            if dbg_stream is not None:
                nc.sync.dma_start(dbg_out[:], hT[dbg_stream][:])

    nc.compile()
    return nc


def _gps_layer(nc, l, cap, hT, hB, tabs, s, gidx, oh_d, W, B, vbia,
               big_p, chunk_p, gath_p, oh_p, ps2, ps1, emit_table, gcall,
               pending, last):
    AFI = AF.Identity

    # ---------------- attention (graph-local) ----------------
    qT = big_p.tile([C, NPC], bdt, tag="qT", name="qT")
    for ch in range(2):
        sl = slice(ch * 1024, (ch + 1) * 1024)
        qp = ps2.tile([C, 1024], fdt, tag="ps2")
        _mm(nc, qp, W(f"qT_{l}"), hB[:, sl], start=True, stop=True)
        nc.scalar.activation(qT[:, sl], qp[:], AFI, bias=B(f"qb_{l}"))
    v_sb = big_p.tile([128, NCHUNK, C], bdt, tag="v_sb", name="v_sb")
    for k in range(NCHUNK):
        vp = ps1.tile([128, C], fdt, tag="ps1")
        nc.tensor.matmul(vp[:], hB[:, k * 128:(k + 1) * 128], W(f"vT_{l}"),
                         start=True, stop=True)
        nc.vector.tensor_add(v_sb[:, k, :], vp[:], vbia[:, l, :])

    kT = big_p.tile([C, NPC], bdt, tag="kTm", name="kT")
    for ch in range(2):
        sl = slice(ch * 1024, (ch + 1) * 1024)
        kp = ps2.tile([C, 1024], fdt, tag="ps2")
        _mm(nc, kp, W(f"kT_{l}"), hB[:, sl], start=True, stop=True)
        nc.scalar.activation(kT[:, sl], kp[:], AFI, bias=B(f"kb_{l}"))
    expT = big_p.tile([128, 2, HEADS, NG_C, NPG], bdt, tag="expT", name="expT")
    for h in range(HEADS):
        for kc in range(2):
            for gh in range(2):
                sp = ps2.tile([128, 4, NPG], fdt, tag="ps2")
                for gi in range(4):
                    g = gh * 4 + gi
                    # per-head K=32 contraction: both operands sliced to the
                    # head's 32 channels, weight tile placed at array row 32h
                    ksl = kT[32 * h:32 * h + 32,
                             g * NPG + kc * 128:g * NPG + kc * 128 + 128]
                    qsl = qT[32 * h:32 * h + 32, g * NPG:(g + 1) * NPG]
                    nc.tensor.matmul(sp[:, gi, :], ksl, qsl,
                                     start=True, stop=True,
                                     tile_position=(32 * h, 0))
                nc.scalar.activation(expT[:, kc, h, gh * 4:gh * 4 + 4, :],
                                     sp[:], AF.Exp)

    recip = big_p.tile([128, NPC], bdt, tag="recip", name="recip")
    for half in range(2):
        sm = ps2.tile([128, 1024], fdt, tag="ps2")
        for h in range(HEADS):
            for qc in range(2):
                osl = sm[32 * h:32 * h + 32, qc * 512:(qc + 1) * 512]
                for kc in range(2):
                    rhs = expT[:, kc, h, :, :].rearrange("p g q -> p (g q)")
                    nc.tensor.matmul(
                        osl, W("ones32", width=HD),
                        rhs[:, half * 1024 + qc * 512:half * 1024 + (qc + 1) * 512],
                        start=(kc == 0), stop=(kc == 1),
                        tile_position=(0, 32 * h))
        with nc.allow_low_precision(reason="softmax recip bf16 ok at 2e-2 gate"):
            nc.vector.reciprocal(recip[:, half * 1024:(half + 1) * 1024], sm[:])

    o_bf = big_p.tile([C, NPC], bdt, tag="o_bf", name="o_bf")
    for g in range(NG_C):
        op = ps1.tile([128, NPG], fdt, tag="ps1")
        for h in range(HEADS):
            for kc in range(2):
                lhs = v_sb[:, g * 2 + kc, 32 * h:32 * h + 32]
                rhs = expT[:, kc, h, g, :]
                nc.tensor.matmul(op[32 * h:32 * h + 32, :], lhs, rhs,
                                 start=(kc == 0), stop=(kc == 1),
                                 tile_position=(0, 32 * h))
        nc.vector.tensor_mul(o_bf[:, g * NPG:(g + 1) * NPG], op[:],
                             recip[:, g * NPG:(g + 1) * NPG])

    # ---------------- GIN ----------------
    g_bf = big_p.tile([C, NPC], bdt, tag="g_bf")
    tab = tabs[s]
    cap_e = cap * 128                       # gathered rows per window
    qn = [0]
    for wg in range(NWIN // WGRP):
        nidx = WGRP * cap_e
        gath = gath_p.tile([128, WGRP * cap, C], bdt, tag="gath")
        base = wg * nidx
        off = 0
        while off < nidx:
            n = min(gcall, nidx - off)
            isl = gidx[:, (base + off) // 16:(base + off + n) // 16]
            nc.gpsimd.dma_gather(
                gath[:, off // 128:(off + n) // 128, :], tab[:], isl,
                n, n, C, queue_num=qn[0])
            qn[0] = (qn[0] + 1) % 4
            off += n
        for wi in range(WGRP):
            w = wg * WGRP + wi
            ohs = oh_p.tile([128, cap, WIN], mybir.dt.float8e4, tag="ohs")
            nc.sync.dma_start(
                ohs[:].rearrange("p t c -> p (t c)"),
                oh_d[:][:, w * cap * WIN:(w + 1) * cap * WIN])
            ap = ps1.tile([C, WIN], fdt, tag="ps1")
            for t in range(cap):
                nc.tensor.matmul(ap[:], gath[:, wi * cap + t, :], ohs[:, t, :],
                                 start=(t == 0), stop=(t == cap - 1))
            nc.vector.tensor_add(g_bf[:, w * WIN:(w + 1) * WIN], ap[:],
                                 hT[:, w * WIN:(w + 1) * WIN])

    # issue the previous stream's table AllGather now: all of this stream's
    # gather calls are already queued on gpsimd, so the in-order queue does
    # not stall waiting for the previous stream's MLP/table write.
    if pending[0] is not None:
        pending[0]()
        pending[0] = None

    # GIN MLP + combine with attention: acc = h1 + h2
    acc_bf = big_p.tile([C, NPC], bdt, tag="acc_bf")
    r_bf = big_p.tile([C, NPC], bdt, tag="r_bf")
    for ch in range(2):
        sl = slice(ch * 1024, (ch + 1) * 1024)
        tp = ps2.tile([C, 1024], fdt, tag="ps2")
        _mm(nc, tp, W(f"gw1_{l}"), g_bf[:, sl], start=True, stop=True)
        nc.scalar.activation(r_bf[:, sl], tp[:], AF.Relu, bias=B(f"gb1_{l}"))
        up = ps2.tile([C, 1024], fdt, tag="ps2")
        _mm(nc, up, W(f"gw2_{l}"), r_bf[:, sl], start=True, stop=True)
        h1 = chunk_p.tile([C, 1024], fdt, tag="h1")
        nc.scalar.activation(h1[:], up[:], AFI, bias=B(f"sgb2_{l}"), scale=S_BN)
        ap2 = ps2.tile([C, 1024], fdt, tag="ps2")
        _mm(nc, ap2, W(f"ow_{l}"), o_bf[:, sl], start=True, stop=False)
        _mm(nc, ap2, W("I2"), hB[:, sl], start=False, stop=True)
        h2 = chunk_p.tile([C, 1024], fdt, tag="h2")
        nc.scalar.activation(h2[:], ap2[:], AFI, bias=B(f"sob_{l}"), scale=S_BN)
        nc.vector.tensor_add(acc_bf[:, sl], h1[:], h2[:])

    # MLP (per-chunk to keep r2 small)
    for ch in range(2):
        sl = slice(ch * 1024, (ch + 1) * 1024)
        r2_bf = chunk_p.tile([C, 2, 1024], bdt, tag="r2_bf")
        for mh in range(2):
            mp = ps2.tile([C, 1024], fdt, tag="ps2")
            _mm(nc, mp, W(f"m1_{l}", width=256)[:, mh * 128:(mh + 1) * 128],
                acc_bf[:, sl], start=True, stop=True)
            bname = f"m1ba_{l}" if mh == 0 else f"m1bb_{l}"
            nc.scalar.activation(r2_bf[:, mh, :], mp[:], AF.Relu, bias=B(bname))
        m2p = ps2.tile([C, 1024], fdt, tag="ps2")
        _mm(nc, m2p, W(f"m2a_{l}"), r2_bf[:, 0, :], start=True, stop=False)
        _mm(nc, m2p, W(f"m2b_{l}"), r2_bf[:, 1, :], start=False, stop=False)
        _mm(nc, m2p, W("I1"), acc_bf[:, sl], start=False, stop=True)
        dh = chunk_p.tile([C, 1024], fdt, tag="dh")
        nc.scalar.activation(dh[:], m2p[:], AFI, bias=B(f"smb2_{l}"), scale=S_BN)
        nc.vector.tensor_add(hT[:, sl], hT[:, sl], dh[:])
        nc.vector.tensor_copy(hB[:, sl], hT[:, sl])

    if not last:
        def _emit(s=s, hB=hB):
            tabs[s] = emit_table(hB)
        pending[0] = _emit


# ---------------------------------------------------------------------------
# Entry point
# ---------------------------------------------------------------------------

_CACHE = {}


def _get_kernel(caps, offs, boffs, wcols, bcols, **kw):
    key = (tuple(caps), wcols, bcols, tuple(sorted(kw.items())))
    if key not in _CACHE:
        _CACHE[key] = build_kernel(caps, offs, boffs, wcols, bcols, **kw)
    return _CACHE[key]


def kernel(**inputs):
    in_maps, caps, offs, boffs, wcols, bcols = _pack_host(inputs)
    nc = _get_kernel(caps, offs, boffs, wcols, bcols)
    res = run_bass_kernel_spmd(nc, in_maps, core_ids=list(range(N_CORES)))
    pools = []
    for si in range(3):
        parts = [np.asarray(res.results[c]["pool_out"][si])
                 for c in range(N_CORES)]
        full = np.concatenate(parts, axis=1)          # [C, 64]
        pools.append(np.ascontiguousarray(full.T).astype(np.float32))
    return tuple(pools)


# revision 31
# speedup vs baseline: 1.7682x; 1.0929x over previous
"""Trainium2 Bass kernel for nn_CGT_21354577396059 (GPS-style GNN, 3 streams x 3 layers).

Strategy (8 NeuronCores, SPMD):
- Node-shard: core c owns nodes [2048c, 2048c+2048) = 8 graphs of 256 nodes.
- Activations feature-major in SBUF: hT [C=128 partitions, 2048 nodes] fp32,
  bf16 copies as matmul inputs.
- GIN segment_sum: edges dst-sorted per core, deduped by src per 128-dst
  window (multi-hot count columns); src rows gathered from a bf16 node-major
  DRAM table (gpsimd dma_gather, 256B rows, one call per window-group);
  scatter via count-matrix matmuls aggT += gathered_chunk.T @ onehot_chunk.
- The bf16 node table is stored P-MAJOR per core block (row p*16+k holds node
  k*128+p) so the SBUF->DRAM table write is contiguous; gather indices are
  host-permuted to match. Table rebuilt each layer via PE transpose + 8-core
  AllGather.
- One-hot count matrices are stored partition-major in DRAM so their loads are
  contiguous (4KB+ per-partition descriptors instead of 256B).
- Attention is graph-local: scoresT = kT.T @ qT per (graph, head, key-chunk)
  masked-K; exp on ACT; softmax sums via col-tiled ones-matmuls; o via
  col-tiled matmuls contracted over keys; normalization via ACT reciprocal.

kernel(**inputs) takes the FULL unsharded inputs and returns
(pool(h0), pool(ha), pool(hb)) -- tuple of [64, 128] float32 -- like the reference.
"""
import sys
import numpy as np
import ml_dtypes

if "/opt/trn_rl_repo" not in sys.path:
    sys.path.insert(0, "/opt/trn_rl_repo")

import concourse.bass as bass  # noqa: F401
import concourse.tile as tile
from concourse import bacc, mybir
from concourse.bass_utils import run_bass_kernel_spmd

BF = ml_dtypes.bfloat16

# Problem constants (self-contained; no reads of /root/problem/*)
N_NODES = 16384
N_GRAPHS = 64
NPG = 256
FEA_DIM = 32
PE_DIM = 20
C = 128
HEADS = 4
HD = C // HEADS
L = 3
BN_EPS = 1e-5
S_BN = float(1.0 / np.sqrt(1.0 + BN_EPS))

N_CORES = 8
NPC = N_NODES // N_CORES   # 2048
NG_C = NPC // NPG          # 8 graphs per core
WIN = 128
NWIN = NPC // WIN          # 16
NCHUNK = NPC // 128        # 16
WGRP = 1                   # windows per dma_gather call

fdt = mybir.dt.float32
bdt = mybir.dt.bfloat16
i16 = mybir.dt.int16
AF = mybir.ActivationFunctionType
AX = mybir.AxisListType
ALU = mybir.AluOpType
AG_GROUPS = [list(range(N_CORES))]


# ---------------------------------------------------------------------------
# Host-side data prep
# ---------------------------------------------------------------------------

def _perm_row(n):
    """Global table row for node n with p-major per-core blocks."""
    cb, m = n // NPC, n % NPC
    return cb * NPC + (m % 128) * NCHUNK + m // 128


def _wrap_idxs(idx):
    """dma_gather idx layout [128, n/16] int16: idx i at (i%16, i//16),
    replicated across the 8 16-partition blocks."""
    n = len(idx)
    a = np.asarray(idx, np.int16).reshape(n // 16, 16).T
    return np.ascontiguousarray(np.tile(a, (8, 1)))


def _prep_edges_stream(edge_index):
    """Dedup by src per (core, 128-dst window); count-matrix columns.

    Returns (cap_chunks, [(gidx_wrapped, oh_pmajor)] per core) where
    oh_pmajor is [128, NWIN*cap*WIN] bf16 (partition-major onehot)."""
    src = np.asarray(edge_index[0]).astype(np.int64)
    dst = np.asarray(edge_index[1]).astype(np.int64)
    per_core_wins = []
    max_w = 0
    for c in range(N_CORES):
        m = (dst >= c * NPC) & (dst < (c + 1) * NPC)
        s, d = src[m], dst[m] - c * NPC
        wins = []
        for w in range(NWIN):
            mw = (d >= w * WIN) & (d < (w + 1) * WIN)
            sw, dw = s[mw], d[mw] - w * WIN
            # dedup srcs within the window; multi-hot count columns
            uniq, inv = np.unique(sw, return_inverse=True)
            cnt = np.zeros((len(uniq), WIN), np.float32)
            np.add.at(cnt, (inv, dw), 1.0)
            # sort rows by permuted table address for DMA page locality
            order = np.argsort(_perm_row(uniq), kind="stable")
            uniq, cnt = uniq[order], cnt[order]
            wins.append((uniq, cnt))
            max_w = max(max_w, len(uniq))
        per_core_wins.append(wins)
    cap_e = ((max_w + 127) // 128) * 128     # rows per window, padded
    cap = cap_e // 128
    out = []
    for c in range(N_CORES):
        srcs = np.zeros(NWIN * cap_e, np.int64)
        oh = np.zeros((NWIN, cap_e, WIN), np.float32)
        for w in range(NWIN):
            uniq, cnt = per_core_wins[c][w]
            n = len(uniq)
            srcs[w * cap_e:w * cap_e + n] = _perm_row(uniq)
            oh[w, :n, :] = cnt
        # partition-major: oh_pm[p, (w, t, c)] = oh[w, t*128+p, c]
        # fp8e4: counts 1..3 are exact; halves the DMA bytes
        oh_pm = np.ascontiguousarray(
            oh.reshape(NWIN, cap, 128, WIN).transpose(2, 0, 1, 3)
            .reshape(128, NWIN * cap * WIN).astype(ml_dtypes.float8_e4m3fn))
        out.append((_wrap_idxs(srcs), oh_pm))
    return cap, out


def _pack_host(inputs):
    inp = {k: np.asarray(v) for k, v in inputs.items()}
    rt2 = 1.0 / np.sqrt(HD)

    blocks, offs = [], {}

    def add(name, arr):
        arr = np.asarray(arr, np.float32)
        k, m = arr.shape
        buf = np.zeros((128, m), BF)
        buf[:k] = arr.astype(BF)
        offs[name] = sum(b.shape[1] for b in blocks)
        blocks.append(buf)

    add("emb", inp["node_emb_w"])
    add("pe", inp["pe_lin_w"])
    add("I2", 2.0 * np.eye(C))       # h2 fold: ACT scale s gives 2s*h
    add("I1", np.eye(C))             # transpose identity + acc fold
    add("ones32", np.ones((C, HD)))
    for l in range(L):
        aw = inp["attn_in_w"][l]
        add(f"gw1_{l}", inp["gin_w1"][l])
        add(f"gw2_{l}", inp["gin_w2"][l])
        add(f"qT_{l}", (aw[0:C] * rt2).T)
        add(f"kT_{l}", aw[C:2 * C].T)
        add(f"vT_{l}", aw[2 * C:3 * C].T)
        add(f"ow_{l}", inp["attn_out_w"][l].T)
        add(f"m1_{l}", inp["mlp_w1"][l])
        add(f"m2a_{l}", inp["mlp_w2"][l][0:C])
        add(f"m2b_{l}", inp["mlp_w2"][l][C:2 * C])
    wts = np.ascontiguousarray(np.concatenate(blocks, axis=1))

    bvecs, boffs = [], {}

    def addb(name, vec):
        vec = np.asarray(vec, np.float32).reshape(-1)
        assert vec.shape == (C,)
        boffs[name] = len(bvecs)
        bvecs.append(vec)

    addb("eb", inp["node_emb_b"] + inp["pe_lin_b"])
    for l in range(L):
        ab = inp["attn_in_b"][l]
        addb(f"gb1_{l}", inp["gin_b1"][l])
        addb(f"sgb2_{l}", S_BN * inp["gin_b2"][l])
        addb(f"qb_{l}", ab[0:C] * rt2)
        addb(f"kb_{l}", ab[C:2 * C])
        addb(f"sob_{l}", S_BN * inp["attn_out_b"][l])
        addb(f"m1ba_{l}", inp["mlp_b1"][l][0:C])
        addb(f"m1bb_{l}", inp["mlp_b1"][l][C:2 * C])
        addb(f"smb2_{l}", S_BN * inp["mlp_b2"][l])
    biases = np.ascontiguousarray(np.stack(bvecs, axis=1).astype(np.float32))

    vbias = np.ascontiguousarray(np.stack(
        [np.tile(inp["attn_in_b"][l][2 * C:3 * C], (128, 1)) for l in range(L)]
    ).astype(np.float32))

    caps, edges = [], []
    for key in ("edge_index", "edge_index1", "edge_index2"):
        cap, per_core = _prep_edges_stream(inp[key])
        caps.append(cap)
        edges.append(per_core)

    xs = [inp["x"], inp["x1"], inp["x2"]]
    in_maps = []
    for c in range(N_CORES):
        m = {"wts": wts, "biases": biases, "vbias": vbias}
        sl = slice(c * NPC, (c + 1) * NPC)
        for s in range(3):
            m[f"xT{s}"] = np.ascontiguousarray(xs[s][sl].T.astype(BF))
            m[f"gidx{s}"] = edges[s][c][0]
            m[f"onehot{s}"] = edges[s][c][1]
        m["peT"] = np.ascontiguousarray(inp["pe"][sl].T.astype(BF))
        in_maps.append(m)

    return in_maps, caps, offs, boffs, wts.shape[1], biases.shape[1]


# ---------------------------------------------------------------------------
# Kernel builder
# ---------------------------------------------------------------------------


def _mm(nc, out, lhsT, rhs, start, stop, nmax=512):
    """matmul with moving free dim split to <=512 (ISA limit)."""
    n = rhs.shape[-1]
    assert out.shape[-1] == n
    for i in range(0, n, nmax):
        j = min(i + nmax, n)
        nc.tensor.matmul(out[:, i:j], lhsT, rhs[:, i:j], start=start, stop=stop)


def build_kernel(caps, offs, boffs, wcols, bcols, n_layers=L, n_streams=3,
                 gcall=1024, dbg_stream=None):
    nc = bacc.Bacc("TRN2", target_bir_lowering=False, num_devices=N_CORES,
                   num_swdge_queues=4)

    wts_d = nc.dram_tensor("wts", [128, wcols], bdt, kind="ExternalInput")
    bias_d = nc.dram_tensor("biases", [128, bcols], fdt, kind="ExternalInput")
    vbias_d = nc.dram_tensor("vbias", [L, 128, 128], fdt, kind="ExternalInput")
    peT_d = nc.dram_tensor("peT", [PE_DIM, NPC], bdt, kind="ExternalInput")
    xT_d, gidx_d, oh_d = [], [], []
    for s in range(3):
        cap = caps[s]
        xT_d.append(nc.dram_tensor(f"xT{s}", [FEA_DIM, NPC], bdt,
                                   kind="ExternalInput"))
        gidx_d.append(nc.dram_tensor(f"gidx{s}", [128, NWIN * cap * 8], i16,
                                     kind="ExternalInput"))
        oh_d.append(nc.dram_tensor(f"onehot{s}", [128, NWIN * cap * WIN],
                                   mybir.dt.float8e4, kind="ExternalInput"))
    pool_out = nc.dram_tensor("pool_out", [3, C, NG_C], fdt,
                              kind="ExternalOutput")
    dbg_out = None
    if dbg_stream is not None:
        dbg_out = nc.dram_tensor("dbg_out", [C, NPC], fdt, kind="ExternalOutput")

    with tile.TileContext(nc) as tc:
        with (
            tc.tile_pool(name="const", bufs=1) as const_p,
            tc.tile_pool(name="hstate", bufs=1) as hstate_p,
            tc.tile_pool(name="big", bufs=1) as big_p,       # full-width tiles
            tc.tile_pool(name="chunk", bufs=2) as chunk_p,   # [C,1024]-ish tiles
            tc.tile_pool(name="gath", bufs=8) as gath_p,
            tc.tile_pool(name="ohp", bufs=5) as oh_p,
            tc.tile_pool(name="ps2", bufs=3, space="PSUM") as ps2,   # 2-bank
            tc.tile_pool(name="ps1", bufs=2, space="PSUM") as ps1,   # 1-bank
            tc.tile_pool(name="dram", bufs=4, space="DRAM") as dram_p,
        ):
            wts = const_p.tile([128, wcols], bdt)
            nc.sync.dma_start(wts[:], wts_d[:])
            bia = const_p.tile([128, bcols], fdt)
            nc.sync.dma_start(bia[:], bias_d[:])
            vbia = const_p.tile([128, L, 128], fdt)
            nc.sync.dma_start(vbia[:], vbias_d[:].rearrange("l p c -> p l c"))

            def W(name, width=128, rows=128):
                return wts[0:rows, offs[name]:offs[name] + width]

            def B(name):
                return bia[:, boffs[name]:boffs[name] + 1]

            hT = [hstate_p.tile([C, NPC], bdt, tag=f"hT{s}", name=f"hT{s}")
                  for s in range(3)]
            gidx = []
            for s in range(n_streams):
                t = hstate_p.tile([128, NWIN * caps[s] * 8], i16,
                                  tag=f"gidx{s}", name=f"gidx{s}")
                nc.sync.dma_start(t[:], gidx_d[s][:])
                gidx.append(t)

            def emit_table(src_bf):
                """feature-major bf16 [C, NPC] -> p-major node table ->
                AllGather. Block layout: DRAM row p*NCHUNK+k = node k*128+p."""
                nm = big_p.tile([128, NCHUNK, C], bdt, tag="nm")
                for k in range(NCHUNK):
                    pt = ps1.tile([128, 128], bdt, tag="ps1", name="pt")
                    nc.tensor.transpose(pt[:], src_bf[:, k * 128:(k + 1) * 128],
                                        W("I1"))
                    nc.vector.tensor_copy(nm[:, k, :], pt[:])
                agi = dram_p.tile([128, NCHUNK * C], bdt, tag="agi")
                nc.sync.dma_start(agi[:], nm[:].rearrange("p k c -> p (k c)"))
                tab = dram_p.tile([N_NODES, C], bdt, tag="tab", name="tab",
                                  addr_space="Shared")
                nc.gpsimd.collective_compute(
                    "AllGather", ALU.bypass, replica_groups=AG_GROUPS,
                    ins=[agi.opt()], outs=[tab.opt()])
                return tab

            # ---------------- embedding ----------------
            tabs = [None] * 3
            pending = [None]   # deferred emit_table issue (keeps gpsimd queue hot)
            for s in range(n_streams):
                xT = chunk_p.tile([FEA_DIM, NPC], bdt, tag="xT")
                nc.sync.dma_start(xT[:], xT_d[s][:])
                peT = chunk_p.tile([PE_DIM, NPC], bdt, tag="peT")
                nc.sync.dma_start(peT[:], peT_d[:])
                for ch in range(2):
                    sl = slice(ch * 1024, (ch + 1) * 1024)
                    ep = ps2.tile([C, 1024], fdt, tag="ps2")
                    _mm(nc, ep, W("emb", rows=FEA_DIM), xT[:, sl],
                        start=True, stop=False)
                    _mm(nc, ep, W("pe", rows=PE_DIM), peT[:, sl],
                        start=False, stop=True)
                    nc.scalar.activation(hT[s][:, sl], ep[:], AF.Identity,
                                         bias=B("eb"))
                if s < n_streams - 1:
                    tabs[s] = emit_table(hT[s])
                else:
                    def _emit_last(s=s):
                        tabs[s] = emit_table(hT[s])
                    pending[0] = _emit_last

            # ---------------- layers ----------------
            for l in range(n_layers):
                for s in range(n_streams):
                    _gps_layer(nc, l, caps[s], hT[s], tabs, s,
                               gidx[s], oh_d[s], W, B, vbia,
                               big_p, chunk_p, gath_p, oh_p, ps2, ps1,
                               emit_table, gcall, pending,
                               last=(l == n_layers - 1))

            # ---------------- pooling ----------------
            for s in range(n_streams):
                po = chunk_p.tile([C, NG_C], fdt, tag="pool")
                nc.vector.reduce_sum(
                    out=po[:],
                    in_=hT[s][:].rearrange("c (g n) -> c g n", g=NG_C),
                    axis=AX.X)
                nc.sync.dma_start(pool_out[s], po[:])

            if dbg_stream is not None:
                nc.sync.dma_start(dbg_out[:], hT[dbg_stream][:])

    nc.compile()
    return nc


def _gps_layer(nc, l, cap, hT, tabs, s, gidx, oh_d, W, B, vbia,
               big_p, chunk_p, gath_p, oh_p, ps2, ps1, emit_table, gcall,
               pending, last):
    AFI = AF.Identity

    # ---------------- attention (graph-local) ----------------
    qT = big_p.tile([C, NPC], bdt, tag="qT", name="qT")
    for ch in range(2):
        sl = slice(ch * 1024, (ch + 1) * 1024)
        qp = ps2.tile([C, 1024], fdt, tag="ps2")
        _mm(nc, qp, W(f"qT_{l}"), hT[:, sl], start=True, stop=True)
        nc.scalar.activation(qT[:, sl], qp[:], AFI, bias=B(f"qb_{l}"))
    v_sb = big_p.tile([128, NCHUNK, C], bdt, tag="v_sb", name="v_sb")
    for k in range(NCHUNK):
        vp = ps1.tile([128, C], fdt, tag="ps1")
        nc.tensor.matmul(vp[:], hT[:, k * 128:(k + 1) * 128], W(f"vT_{l}"),
                         start=True, stop=True)
        nc.vector.tensor_add(v_sb[:, k, :], vp[:], vbia[:, l, :])

    kT = big_p.tile([C, NPC], bdt, tag="kTm", name="kT")
    for ch in range(2):
        sl = slice(ch * 1024, (ch + 1) * 1024)
        kp = ps2.tile([C, 1024], fdt, tag="ps2")
        _mm(nc, kp, W(f"kT_{l}"), hT[:, sl], start=True, stop=True)
        nc.scalar.activation(kT[:, sl], kp[:], AFI, bias=B(f"kb_{l}"))
    expT = big_p.tile([128, 2, HEADS, NG_C, NPG], bdt, tag="expT", name="expT")
    for h in range(HEADS):
        for kc in range(2):
            for gh in range(2):
                sp = ps2.tile([128, 4, NPG], fdt, tag="ps2")
                for gi in range(4):
                    g = gh * 4 + gi
                    # per-head K=32 contraction: both operands sliced to the
                    # head's 32 channels, weight tile placed at array row 32h
                    ksl = kT[32 * h:32 * h + 32,
                             g * NPG + kc * 128:g * NPG + kc * 128 + 128]
                    qsl = qT[32 * h:32 * h + 32, g * NPG:(g + 1) * NPG]
                    nc.tensor.matmul(sp[:, gi, :], ksl, qsl,
                                     start=True, stop=True,
                                     tile_position=(32 * h, 0))
                nc.scalar.activation(expT[:, kc, h, gh * 4:gh * 4 + 4, :],
                                     sp[:], AF.Exp)

    recip = big_p.tile([128, NPC], bdt, tag="recip", name="recip")
    for half in range(2):
        sm = ps2.tile([128, 1024], fdt, tag="ps2")
        for h in range(HEADS):
            for qc in range(2):
                osl = sm[32 * h:32 * h + 32, qc * 512:(qc + 1) * 512]
                for kc in range(2):
                    rhs = expT[:, kc, h, :, :].rearrange("p g q -> p (g q)")
                    nc.tensor.matmul(
                        osl, W("ones32", width=HD),
                        rhs[:, half * 1024 + qc * 512:half * 1024 + (qc + 1) * 512],
                        start=(kc == 0), stop=(kc == 1),
                        tile_position=(0, 32 * h))
        with nc.allow_low_precision(reason="softmax recip bf16 ok at 2e-2 gate"):
            nc.vector.reciprocal(recip[:, half * 1024:(half + 1) * 1024], sm[:])

    o_bf = big_p.tile([C, NPC], bdt, tag="o_bf", name="o_bf")
    for g in range(NG_C):
        op = ps1.tile([128, NPG], fdt, tag="ps1")
        for h in range(HEADS):
            for kc in range(2):
                lhs = v_sb[:, g * 2 + kc, 32 * h:32 * h + 32]
                rhs = expT[:, kc, h, g, :]
                nc.tensor.matmul(op[32 * h:32 * h + 32, :], lhs, rhs,
                                 start=(kc == 0), stop=(kc == 1),
                                 tile_position=(0, 32 * h))
        nc.vector.tensor_mul(o_bf[:, g * NPG:(g + 1) * NPG], op[:],
                             recip[:, g * NPG:(g + 1) * NPG])

    # ---------------- GIN ----------------
    g_bf = big_p.tile([C, NPC], bdt, tag="g_bf")
    tab = tabs[s]
    cap_e = cap * 128                       # gathered rows per window
    qn = [0]
    for wg in range(NWIN // WGRP):
        nidx = WGRP * cap_e
        gath = gath_p.tile([128, WGRP * cap, C], bdt, tag="gath")
        base = wg * nidx
        off = 0
        while off < nidx:
            n = min(gcall, nidx - off)
            isl = gidx[:, (base + off) // 16:(base + off + n) // 16]
            nc.gpsimd.dma_gather(
                gath[:, off // 128:(off + n) // 128, :], tab[:], isl,
                n, n, C, queue_num=qn[0])
            qn[0] = (qn[0] + 1) % 4
            off += n
        for wi in range(WGRP):
            w = wg * WGRP + wi
            ohs = oh_p.tile([128, cap, WIN], mybir.dt.float8e4, tag="ohs")
            nc.sync.dma_start(
                ohs[:].rearrange("p t c -> p (t c)"),
                oh_d[:][:, w * cap * WIN:(w + 1) * cap * WIN])
            ap = ps1.tile([C, WIN], fdt, tag="ps1")
            for t in range(cap):
                nc.tensor.matmul(ap[:], gath[:, wi * cap + t, :], ohs[:, t, :],
                                 start=(t == 0), stop=(t == cap - 1))
            nc.vector.tensor_add(g_bf[:, w * WIN:(w + 1) * WIN], ap[:],
                                 hT[:, w * WIN:(w + 1) * WIN])

    # issue the previous stream's table AllGather now: all of this stream's
    # gather calls are already queued on gpsimd, so the in-order queue does
    # not stall waiting for the previous stream's MLP/table write.
    if pending[0] is not None:
        pending[0]()
        pending[0] = None

    # GIN MLP + combine with attention: acc = h1 + h2
    acc_bf = big_p.tile([C, NPC], bdt, tag="acc_bf")
    r_bf = big_p.tile([C, NPC], bdt, tag="r_bf")
    for ch in range(2):
        sl = slice(ch * 1024, (ch + 1) * 1024)
        tp = ps2.tile([C, 1024], fdt, tag="ps2")
        _mm(nc, tp, W(f"gw1_{l}"), g_bf[:, sl], start=True, stop=True)
        nc.scalar.activation(r_bf[:, sl], tp[:], AF.Relu, bias=B(f"gb1_{l}"))
        up = ps2.tile([C, 1024], fdt, tag="ps2")
        _mm(nc, up, W(f"gw2_{l}"), r_bf[:, sl], start=True, stop=True)
        h1 = chunk_p.tile([C, 1024], fdt, tag="h1")
        nc.scalar.activation(h1[:], up[:], AFI, bias=B(f"sgb2_{l}"), scale=S_BN)
        ap2 = ps2.tile([C, 1024], fdt, tag="ps2")
        _mm(nc, ap2, W(f"ow_{l}"), o_bf[:, sl], start=True, stop=False)
        _mm(nc, ap2, W("I2"), hT[:, sl], start=False, stop=True)
        h2 = chunk_p.tile([C, 1024], fdt, tag="h2")
        nc.scalar.activation(h2[:], ap2[:], AFI, bias=B(f"sob_{l}"), scale=S_BN)
        nc.vector.tensor_add(acc_bf[:, sl], h1[:], h2[:])

    # MLP (per-chunk to keep r2 small)
    for ch in range(2):
        sl = slice(ch * 1024, (ch + 1) * 1024)
        r2_bf = chunk_p.tile([C, 2, 1024], bdt, tag="r2_bf")
        for mh in range(2):
            mp = ps2.tile([C, 1024], fdt, tag="ps2")
            _mm(nc, mp, W(f"m1_{l}", width=256)[:, mh * 128:(mh + 1) * 128],
                acc_bf[:, sl], start=True, stop=True)
            bname = f"m1ba_{l}" if mh == 0 else f"m1bb_{l}"
            nc.scalar.activation(r2_bf[:, mh, :], mp[:], AF.Relu, bias=B(bname))
        m2p = ps2.tile([C, 1024], fdt, tag="ps2")
        _mm(nc, m2p, W(f"m2a_{l}"), r2_bf[:, 0, :], start=True, stop=False)
        _mm(nc, m2p, W(f"m2b_{l}"), r2_bf[:, 1, :], start=False, stop=False)
        _mm(nc, m2p, W("I1"), acc_bf[:, sl], start=False, stop=True)
        dh = chunk_p.tile([C, 1024], fdt, tag="dh")
        nc.scalar.activation(dh[:], m2p[:], AFI, bias=B(f"smb2_{l}"), scale=S_BN)
        nc.vector.tensor_add(hT[:, sl], hT[:, sl], dh[:])

    if not last:
        def _emit(s=s, hT=hT):
            tabs[s] = emit_table(hT)
        pending[0] = _emit


# ---------------------------------------------------------------------------
# Entry point
# ---------------------------------------------------------------------------

_CACHE = {}


def _get_kernel(caps, offs, boffs, wcols, bcols, **kw):
    key = (tuple(caps), wcols, bcols, tuple(sorted(kw.items())))
    if key not in _CACHE:
        _CACHE[key] = build_kernel(caps, offs, boffs, wcols, bcols, **kw)
    return _CACHE[key]


def kernel(**inputs):
    in_maps, caps, offs, boffs, wcols, bcols = _pack_host(inputs)
    nc = _get_kernel(caps, offs, boffs, wcols, bcols)
    res = run_bass_kernel_spmd(nc, in_maps, core_ids=list(range(N_CORES)))
    pools = []
    for si in range(3):
        parts = [np.asarray(res.results[c]["pool_out"][si])
                 for c in range(N_CORES)]
        full = np.concatenate(parts, axis=1)          # [C, 64]
        pools.append(np.ascontiguousarray(full.T).astype(np.float32))
    return tuple(pools)
